# revision 2
# baseline (speedup 1.0000x reference)
"""Trainium2 Bass kernel for nn_Net_50620484551136 (gnn_message_passing).

Network (see problem reference):
  h  = MLP(x)                     # 4652 -> 256 -> 256
  h1 = relu(GCN(h, e1)); h2 = relu(GCN(h, e2))
  h  = MLP([h1, h2])              # 512 -> 256 -> 256
  h1 = relu(GCN(h, e1)); h2 = relu(GCN(h, e2))
  h  = MLP([h1, h2])
  r1 = scatter_mean(h, index_1, N); r2 = scatter_mean(h, index_2, N)
  out = log_softmax(MLP([r1, r2]))

Strategy (8 NeuronCores, SPMD single program):
  - Tuple nodes sharded contiguously across cores (6250/core, padded to 6272).
  - All dense matmuls run feature-major (h^T: [feat, node]) in bf16, fp32 PSUM.
  - GCN: matmul commutes with aggregation, so we aggregate g = h * dinv[src]
    (node-major, bf16) and apply the conv weight after.  Each round: write
    g1|g2 locally, AllGather (4 node-range chunks, overlapped with the MLP
    that produces them) to a full [50176, 512] chunk-major buffer, then each
    core gathers its incoming-edge rows (sorted by dst) with
    gpsimd.dma_gather and segment-sums them with PE matmuls against one-hot
    SEG blocks built ON DEVICE from compact (dd, scale) pairs via
    DVE iota==dd * scale (SEG carries dinv[dst]).
  - dma_gather indices are int16, so gathers are split into a low range
    (rows < 32768) and a high range; block counts are per-tile (max over the
    8 cores) so one static program serves all cores with minimal padding.
  - Scatter-mean readout: output bins sharded across cores (625/core, padded
    to 640); same gather+SEG machinery against the AllGathered final h, with
    1/count folded into the SEG scale.  Final MLP + log_softmax on device;
    host concatenates the 8 output shards.
"""

import numpy as np
import ml_dtypes

BF16 = ml_dtypes.bfloat16

# Problem constants (hardcoded per harness contract).
T = 50000
N_BINS = 5000
F_IN = 4652
DIM = 256
N_CLASSES = 5
NCORES = 8
SPLIT = 32768  # int16 gather index limit
NCHUNK = 4     # AllGather chunks per buffer


def _ceil_to(x, m):
    return (x + m - 1) // m * m


def _wrap_idx(v):
    """int16 index vector (len % 16 == 0) -> [128, len/16] wrapped layout."""
    assert len(v) % 16 == 0
    w = v.reshape(-1, 16).T.astype(np.int16)  # [16, len/16]
    return np.tile(w, (8, 1))  # [128, len/16]


def _chunk_weight(w, dtype=BF16):
    """[K, M] -> [128, ceil(K/128), M] (partition = k%128, block = k//128)."""
    k, m = w.shape
    kp = _ceil_to(k, 128)
    wp = np.zeros((kp, m), np.float32)
    wp[:k] = w
    return np.ascontiguousarray(
        wp.reshape(kp // 128, 128, m).transpose(1, 0, 2)
    ).astype(dtype)


def _chunk_bias(b):
    """[M] -> [128, ceil(M/128)] f32 (partition = m%128, col = m//128)."""
    m = len(b)
    mp = _ceil_to(m, 128)
    bp = np.zeros(mp, np.float32)
    bp[:m] = b
    return np.ascontiguousarray(bp.reshape(mp // 128, 128).T).astype(np.float32)


def _chunk_widths(pad):
    """Split `pad` (multiple of 128) into NCHUNK widths, each mult of 128."""
    ntile = pad // 128
    per = ntile // NCHUNK
    ws = [per * 128] * (NCHUNK - 1)
    ws.append(pad - sum(ws))
    return ws


def _cm_rows(src, spc, spad, ncores):
    """Chunk-major global row id for each source node (vectorized).

    Layout: for chunk c (widths from _chunk_widths(spad)), rows
    [ncores*cum[c], ncores*cum[c+1]) hold [rank0 rows, rank1 rows, ...].
    """
    ws = _chunk_widths(spad)
    cum = np.cumsum([0] + ws)  # [NCHUNK+1]
    p = src // spc
    l = src % spc
    c = np.minimum(np.searchsorted(cum, l, side="right") - 1, NCHUNK - 1)
    return ncores * cum[c] + p * np.array(ws)[c] + (l - cum[c])


def _prep_edges(src, dst, dpc, dpad, spc, spad, ncores, seg_scale):
    """Per-core gather indices + compact SEG inputs for one (src -> dst)
    relation.  dst space is sharded dpc-per-core (padded dpad); src rows live
    in a chunk-major AllGathered buffer (see _cm_rows).  Aggregation output
    for dst d is sum over edges e with dst==d of seg_scale[d] * g[src_e].

    Per-tile block counts are variable (max over cores).  Returns dict with
    per-core idx/ddsc arrays plus global per-tile nb_lo/nb_hi lists.
    """
    nt = dpad // 128
    order = np.argsort(dst, kind="stable")
    src = src[order]
    dst = dst[order]
    core_of = dst // dpc
    gsrc = _cm_rows(src, spc, spad, ncores)

    per_core = []  # [p][t] = (lo_gs, hi_gs, lo_dd, hi_dd)
    cnt_lo = np.zeros((ncores, nt), np.int64)
    cnt_hi = np.zeros((ncores, nt), np.int64)
    for p in range(ncores):
        sel = core_of == p
        sp = gsrc[sel]
        ld = dst[sel] - p * dpc
        tiles = []
        for t in range(nt):
            m = (ld // 128) == t
            st = sp[m]
            dd = (ld[m] - t * 128).astype(np.int64)
            lo = st < SPLIT
            tiles.append((st[lo], st[~lo] - SPLIT, dd[lo], dd[~lo]))
            cnt_lo[p, t] = lo.sum()
            cnt_hi[p, t] = (~lo).sum()
        per_core.append(tiles)

    nb_lo = [int(_ceil_to(max(cnt_lo[:, t].max(), 1), 128) // 128)
             for t in range(nt)]
    nb_hi = [int(_ceil_to(cnt_hi[:, t].max(), 128) // 128) for t in range(nt)]
    nb_tot = [nb_lo[t] + nb_hi[t] for t in range(nt)]
    off_nb = np.cumsum([0] + nb_tot).tolist()  # per-tile block offset
    tot_nb = off_nb[-1]

    idx_arrs = []
    ddsc_arrs = []
    for p in range(ncores):
        idx_a = np.zeros((128, tot_nb * 8), np.int16)
        ddsc_a = np.zeros((128, tot_nb, 2), np.float32)
        ddsc_a[:, :, 0] = -1.0  # dd=-1 -> SEG row zero
        for t in range(nt):
            lo_gs, hi_gs, lo_dd, hi_dd = per_core[p][t]
            ob = off_nb[t]
            li = np.zeros(nb_lo[t] * 128, np.int64)
            li[: len(lo_gs)] = lo_gs
            idx_a[:, ob * 8: (ob + nb_lo[t]) * 8] = _wrap_idx(
                li.astype(np.int16))
            if nb_hi[t]:
                hi = np.zeros(nb_hi[t] * 128, np.int64)
                hi[: len(hi_gs)] = hi_gs
                idx_a[:, (ob + nb_lo[t]) * 8: (ob + nb_tot[t]) * 8] = \
                    _wrap_idx(hi.astype(np.int16))
            base = p * dpc + t * 128
            for boff, dd_list in ((0, lo_dd), (nb_lo[t], hi_dd)):
                i = np.arange(len(dd_list))
                b = boff + i // 128
                e = i % 128
                ddsc_a[e, ob + b, 0] = dd_list.astype(np.float32)
                ddsc_a[e, ob + b, 1] = seg_scale[base + dd_list]
        idx_arrs.append(idx_a)
        ddsc_arrs.append(np.ascontiguousarray(ddsc_a))
    return dict(nb_lo=nb_lo, nb_hi=nb_hi, off_nb=off_nb, tot_nb=tot_nb,
                idx=idx_arrs, ddsc=ddsc_arrs)


def host_prep(inputs, ncores=NCORES, n_bins=None):
    """Pure-numpy preprocessing: sharding, edge sorting, idx/ddsc
    construction, weight layout.  Only index arithmetic + data movement."""
    x = np.asarray(inputs["x"], np.float32)
    t_nodes, f_in = x.shape
    dim = np.asarray(inputs["W_i2"]).shape[0]
    ncls = np.asarray(inputs["b_fb"]).shape[0]
    if n_bins is None:
        if t_nodes == T and f_in == F_IN:
            n_bins = N_BINS
        else:
            n_bins = int(np.asarray(inputs["index_1"]).max()) + 1

    assert t_nodes % ncores == 0, (t_nodes, ncores)
    tpc = t_nodes // ncores
    tpad = _ceil_to(tpc, 128)
    nt = tpad // 128
    kin = _ceil_to(f_in, 128)
    assert n_bins % ncores == 0, (n_bins, ncores)
    bpc = n_bins // ncores
    bpad = _ceil_to(bpc, 128)
    bt = bpad // 128

    cfg = dict(
        t_nodes=t_nodes, f_in=f_in, dim=dim, ncls=ncls, n_bins=n_bins,
        ncores=ncores, tpc=tpc, tpad=tpad, nt=nt, kin=kin, kc=kin // 128,
        bpc=bpc, bpad=bpad, bt=bt, g_rows=ncores * tpad,
    )

    # ---- edge relations (with self-loops), degree norm
    rel = {}
    for r, key in ((1, "edge_index_1"), (2, "edge_index_2")):
        ei = np.asarray(inputs[key]).astype(np.int64)
        loop = np.arange(t_nodes, dtype=np.int64)
        s = np.concatenate([ei[0], loop])
        d = np.concatenate([ei[1], loop])
        deg = np.bincount(d, minlength=t_nodes).astype(np.float64)
        dinv = (1.0 / np.sqrt(np.maximum(deg, 1.0))).astype(np.float32)
        rel[r] = dict(
            prep=_prep_edges(s, d, tpc, tpad, tpc, tpad, ncores, dinv),
            dinv=dinv,
        )
    cfg["rel"] = rel

    # ---- readout (scatter-mean): treat (node -> bin) as edges
    ro = {}
    for i, key in ((1, "index_1"), (2, "index_2")):
        idx = np.asarray(inputs[key]).astype(np.int64)
        cnt = np.bincount(idx, minlength=n_bins).astype(np.float64)
        invc = (1.0 / np.maximum(cnt, 1.0)).astype(np.float32)
        nodes = np.arange(t_nodes, dtype=np.int64)
        ro[i] = dict(
            prep=_prep_edges(nodes, idx, bpc, bpad, tpc, tpad, ncores, invc),
        )
    cfg["ro"] = ro

    # ---- per-core x^T slices (bf16) in sub-chunked layout
    # [128, nsub, kc, SUBW]: partition = k%128, sub-chunk of SUBW node
    # columns, contiguous per (partition, sub) for a single fat DMA.
    SUBW = 256
    nsub = _ceil_to(tpad, SUBW) // SUBW
    cfg["subw"] = SUBW
    cfg["nsub"] = nsub
    kc = kin // 128
    xT = []
    for p in range(ncores):
        xs = np.zeros((kin, nsub * SUBW), np.float32)
        xs[:f_in, :tpc] = x[p * tpc: (p + 1) * tpc].T
        # [kc, 128, nsub, SUBW] -> [128, nsub, kc, SUBW]
        a = xs.reshape(kc, 128, nsub, SUBW).transpose(1, 2, 0, 3)
        xT.append(np.ascontiguousarray(a).astype(BF16))
    cfg["xT"] = xT

    # ---- dinv per-node tiles [128, nt] f32 per relation per core
    for r in (1, 2):
        dn = []
        dinv = rel[r]["dinv"]
        for p in range(ncores):
            a = np.zeros((128, nt), np.float32)
            vp = np.zeros(tpad, np.float32)
            vp[:tpc] = dinv[p * tpc: (p + 1) * tpc]
            a[:, :] = vp.reshape(nt, 128).T
            dn.append(a)
        rel[r]["dinv_n"] = dn

    # ---- weights
    w = {}
    w["wi1"] = _chunk_weight(np.asarray(inputs["W_i1"], np.float32))
    w["wi2"] = _chunk_weight(np.asarray(inputs["W_i2"], np.float32))
    for nm, src in (("wc11", "Wc11"), ("wc12", "Wc12"),
                    ("wc21", "Wc21"), ("wc22", "Wc22"),
                    ("wm1a", "W_m1a"), ("wm1b", "W_m1b"),
                    ("wm2a", "W_m2a"), ("wm2b", "W_m2b"),
                    ("wfa", "W_fa"), ("wfb", "W_fb")):
        w[nm] = _chunk_weight(np.asarray(inputs[src], np.float32))
    for nm, src in (("bi1", "b_i1"), ("bi2", "b_i2"),
                    ("bc11", "bc11"), ("bc12", "bc12"),
                    ("bc21", "bc21"), ("bc22", "bc22"),
                    ("bm1a", "b_m1a"), ("bm1b", "b_m1b"),
                    ("bm2a", "b_m2a"), ("bm2b", "b_m2b"),
                    ("bfa", "b_fa"), ("bfb", "b_fb")):
        w[nm] = _chunk_bias(np.asarray(inputs[src], np.float32))
    w["ident16"] = np.eye(128, dtype=BF16)
    w["ident32"] = np.eye(128, dtype=np.float32)
    w["iota"] = np.tile(np.arange(128, dtype=np.float32), (128, 1))
    cfg["w"] = w
    return cfg


def _nchunks(total, step, base=0):
    out = []
    o = 0
    while o < total:
        out.append((base + o, min(step, total - o)))
        o += step
    return out


def build_program(cfg):
    """Build the SPMD bass program (one program, 8 cores)."""
    import concourse.bass as bass
    import concourse.mybir as mybir
    import concourse.tile as tile
    from concourse import bacc

    dt = mybir.dt
    AF = mybir.ActivationFunctionType
    ALU = mybir.AluOpType

    nt, tpad, kc = cfg["nt"], cfg["tpad"], cfg["kc"]
    bt, bpad = cfg["bt"], cfg["bpad"]
    dim, ncls = cfg["dim"], cfg["ncls"]
    dc = dim // 128
    g_rows = cfg["g_rows"]
    ncores = cfg["ncores"]
    rel, ro = cfg["rel"], cfg["ro"]
    rg = [list(range(ncores))]
    SUBW, nsub = cfg["subw"], cfg["nsub"]

    cw = _chunk_widths(tpad)           # node-range chunk widths
    cum = np.cumsum([0] + cw).tolist()  # local row offsets
    ctiles = [range(cum[c] // 128, cum[c + 1] // 128) for c in range(NCHUNK)]

    nc = bacc.Bacc("TRN2", target_bir_lowering=False, debug=False,
                   num_devices=ncores, num_swdge_queues=4)
    qstate = [0]

    def next_q():
        q = qstate[0]
        qstate[0] = (q + 1) % 4
        return q

    # ---------------- I/O declarations ----------------
    xT = nc.dram_tensor("xT", [128, nsub, kc, SUBW], dt.bfloat16,
                        kind="ExternalInput")
    idx_in, ddsc_in, dinvn_in = {}, {}, {}
    for r in (1, 2):
        pr = rel[r]["prep"]
        idx_in[r] = nc.dram_tensor(f"idx{r}", [128, pr["tot_nb"] * 8],
                                   dt.int16, kind="ExternalInput")
        ddsc_in[r] = nc.dram_tensor(f"ddsc{r}", [128, pr["tot_nb"], 2],
                                    dt.float32, kind="ExternalInput")
        dinvn_in[r] = nc.dram_tensor(f"dinvn{r}", [128, nt], dt.float32,
                                     kind="ExternalInput")
    idxr_in, ddscr_in = {}, {}
    for i in (1, 2):
        pr = ro[i]["prep"]
        idxr_in[i] = nc.dram_tensor(f"idxr{i}", [128, pr["tot_nb"] * 8],
                                    dt.int16, kind="ExternalInput")
        ddscr_in[i] = nc.dram_tensor(f"ddscr{i}", [128, pr["tot_nb"], 2],
                                     dt.float32, kind="ExternalInput")

    wnames_bf = dict(
        wi1=[128, kc, dim], wi2=[128, dc, dim],
        wc11=[128, dc, dim], wc12=[128, dc, dim],
        wc21=[128, dc, dim], wc22=[128, dc, dim],
        wm1a=[128, 2 * dc, dim], wm1b=[128, dc, dim],
        wm2a=[128, 2 * dc, dim], wm2b=[128, dc, dim],
        wfa=[128, 2 * dc, dim], wfb=[128, dc, ncls],
        ident16=[128, 128],
    )
    wnames_f32 = dict(
        bi1=[128, dc], bi2=[128, dc],
        bc11=[128, dc], bc12=[128, dc], bc21=[128, dc], bc22=[128, dc],
        bm1a=[128, dc], bm1b=[128, dc], bm2a=[128, dc], bm2b=[128, dc],
        bfa=[128, dc], bfb=[128, 1],
        ident32=[128, 128], iota=[128, 128],
    )
    win = {}
    for nm, shp in wnames_bf.items():
        win[nm] = nc.dram_tensor(nm, shp, dt.bfloat16, kind="ExternalInput")
    for nm, shp in wnames_f32.items():
        win[nm] = nc.dram_tensor(nm, shp, dt.float32, kind="ExternalInput")

    out_dram = nc.dram_tensor("out", [bpad, ncls], dt.float32,
                              kind="ExternalOutput")

    nb_max = max(
        max(rel[r]["prep"]["nb_lo"][t] + rel[r]["prep"]["nb_hi"][t]
            for r in (1, 2) for t in range(nt)),
        max(ro[i]["prep"]["nb_lo"][t] + ro[i]["prep"]["nb_hi"][t]
            for i in (1, 2) for t in range(bt)),
    )

    with tile.TileContext(nc) as tc:
        with (
            tc.tile_pool(name="wpool", bufs=1) as wpool,
            tc.tile_pool(name="hpool", bufs=1) as hpool,
            tc.tile_pool(name="xpool", bufs=2) as xpool,
            tc.tile_pool(name="rpool", bufs=4) as rpool,
            tc.tile_pool(name="edpool", bufs=4) as edpool,
            tc.tile_pool(name="segpool", bufs=4) as segpool,
            tc.tile_pool(name="idxpool", bufs=6) as idxpool,
            tc.tile_pool(name="apool", bufs=4) as apool,
            tc.tile_pool(name="gpool", bufs=3) as gpool,
            tc.tile_pool(name="mpool", bufs=4) as mpool,
            tc.tile_pool(name="pbig", bufs=3, space="PSUM") as pbig,
            tc.tile_pool(name="pagg", bufs=2, space="PSUM") as pagg,
            tc.tile_pool(name="pcnv", bufs=3, space="PSUM") as pcnv,
            tc.tile_pool(name="dpool", bufs=1, space="DRAM") as dpool,
        ):
            # ---- resident weights
            wsb = {}
            for nm in list(wnames_bf) + list(wnames_f32):
                shp = wnames_bf.get(nm) or wnames_f32[nm]
                dtyp = dt.bfloat16 if nm in wnames_bf else dt.float32
                wt = wpool.tile(shp, dtyp, name=f"sb_{nm}", tag=f"w_{nm}")
                nc.sync.dma_start(wt[:], win[nm][:])
                wsb[nm] = wt
            dinvn_sb = {}
            for r in (1, 2):
                dv = wpool.tile([128, nt], dt.float32, name=f"sb_dinvn{r}",
                                tag=f"w_dinvn{r}")
                nc.sync.dma_start(dv[:], dinvn_in[r][:])
                dinvn_sb[r] = dv

            def a_step(h_src, t, gt_dst):
                """Transpose h tile t to node-major and scale by dinv."""
                trp = []
                for f in range(dc):
                    tp = pcnv.tile([128, 128], dt.bfloat16, name="trp",
                                   tag="cnv")
                    nc.tensor.transpose(
                        tp[:], h_src[:, f, t * 128:(t + 1) * 128],
                        wsb["ident16"][:])
                    trp.append(tp)
                for r in (1, 2):
                    for f in range(dc):
                        nc.vector.tensor_scalar_mul(
                            gt_dst[:, (r - 1) * dim + f * 128:
                                   (r - 1) * dim + (f + 1) * 128],
                            trp[f][:], dinvn_sb[r][:, t:t + 1])

            # =========== Phase 1: input MLP  h0 = relu(x@Wi1+bi1)@Wi2+bi2
            # interleaved per AllGather chunk; AG1_c fires when chunk done.
            h_cur = hpool.tile([128, dc, tpad], dt.bfloat16, name="h0T",
                               tag="hT")
            g_loc1 = dpool.tile([tpad, 2 * dim], dt.bfloat16, name="g_loc1",
                                tag="g_loc1")
            g_full1 = dpool.tile([g_rows, 2 * dim], dt.bfloat16,
                                 name="g_full1", tag="g_full1")
            for c in range(NCHUNK):
                subs = [s for s in range(nsub)
                        if cum[c] <= s * SUBW < cum[c + 1]]
                for s in subs:
                    n0 = s * SUBW
                    nw = min(SUBW, tpad - n0)
                    xt = xpool.tile([128, kc, SUBW], dt.bfloat16, name="xt",
                                    tag="xt")
                    nc.sync.dma_start(xt[:], xT[:, s])
                    ps1 = []
                    for f in range(dc):
                        p_ = pbig.tile([128, 512], dt.float32, name="ps1",
                                       tag="mlp")
                        ps1.append(p_)
                        for k in range(kc):
                            nc.tensor.matmul(
                                p_[:, :nw],
                                lhsT=wsb["wi1"][:, k, f * 128:(f + 1) * 128],
                                rhs=xt[:, k, :nw],
                                start=(k == 0), stop=(k == kc - 1))
                    a1 = []
                    for f in range(dc):
                        a_ = apool.tile([128, 512], dt.bfloat16, name="a1",
                                        tag="a1")
                        nc.scalar.activation(a_[:, :nw], ps1[f][:, :nw],
                                             AF.Relu, bias=wsb["bi1"][:, f:f + 1])
                        a1.append(a_)
                    for f2 in range(dc):
                        p2 = pbig.tile([128, 512], dt.float32, name="ps2",
                                       tag="mlp")
                        for k2 in range(dc):
                            nc.tensor.matmul(
                                p2[:, :nw],
                                lhsT=wsb["wi2"][:, k2, f2 * 128:(f2 + 1) * 128],
                                rhs=a1[k2][:, :nw],
                                start=(k2 == 0), stop=(k2 == dc - 1))
                        nc.vector.tensor_scalar(
                            h_cur[:, f2, n0:n0 + nw], p2[:, :nw],
                            wsb["bi2"][:, f2:f2 + 1], None, ALU.add)
                for t in ctiles[c]:
                    gt = gpool.tile([128, 2 * dim], dt.bfloat16, name="gt",
                                    tag="gt")
                    a_step(h_cur, t, gt)
                    nc.sync.dma_start(g_loc1[t * 128:(t + 1) * 128, :], gt[:])
                nc.gpsimd.collective_compute(
                    "AllGather", ALU.bypass, replica_groups=rg,
                    ins=[g_loc1[cum[c]:cum[c + 1], :]],
                    outs=[g_full1[ncores * cum[c]:ncores * cum[c + 1], :]])

            # =========== Phase 2: two GCN rounds
            g_fulls = {1: g_full1}
            hf_loc = dpool.tile([tpad, dim], dt.bfloat16, name="hf_loc",
                                tag="hf_loc")
            hf_full = dpool.tile([g_rows, dim], dt.bfloat16, name="hf_full",
                                 tag="hf_full")
            for rnd in (1, 2):
                g_full = g_fulls[rnd]
                # ---- a) two conv relations (gather + on-device SEG)
                houts = []
                for r in (1, 2):
                    pr = rel[r]["prep"]
                    wc = wsb[f"wc{rnd}{r}"]
                    bc = wsb[f"bc{rnd}{r}"]
                    hout = dpool.tile([128, dc, tpad], dt.bfloat16,
                                      name=f"h{r}T", tag=f"h12_{rnd}{r}")
                    for t in range(nt):
                        nbl, nbh = pr["nb_lo"][t], pr["nb_hi"][t]
                        nb = nbl + nbh
                        ob = pr["off_nb"][t]
                        idxt = idxpool.tile([128, nb_max * 8], dt.int16,
                                            name="idxt", tag="idx")
                        nc.sync.dma_start(idxt[:, :nb * 8],
                                          idx_in[r][:, ob * 8:(ob + nb) * 8])
                        ddsct = rpool.tile([128, nb_max, 2], dt.float32,
                                           name="ddsct", tag="ddsc")
                        nc.sync.dma_start(ddsct[:, :nb, :],
                                          ddsc_in[r][:, ob:ob + nb, :])
                        segt = segpool.tile([128, nb_max * 128], dt.bfloat16,
                                            name="segt", tag="seg")
                        for b in range(nb):
                            nc.vector.tensor_scalar(
                                segt[:, b * 128:(b + 1) * 128],
                                wsb["iota"][:], ddsct[:, b, 0:1],
                                ddsct[:, b, 1:2], ALU.is_equal, ALU.mult)
                        ed = edpool.tile([128, nb_max, dim], dt.bfloat16,
                                         name="ed", tag="ed")
                        nc.gpsimd.dma_gather(
                            ed[:, 0:nbl, :],
                            g_full[:, (r - 1) * dim:r * dim],
                            idxt[:, 0:nbl * 8],
                            nbl * 128, nbl * 128, dim,
                            elem_step=2 * dim, single_packet=False,
                            queue_num=next_q())
                        if nbh:
                            nc.gpsimd.dma_gather(
                                ed[:, nbl:nb, :],
                                g_full[SPLIT:g_rows, (r - 1) * dim:r * dim],
                                idxt[:, nbl * 8:nb * 8],
                                nbh * 128, nbh * 128, dim,
                                elem_step=2 * dim, single_packet=False,
                                queue_num=next_q())
                        # segment-sum: SEG stationary, gathered rows moving
                        agg = pagg.tile([128, dim], dt.float32, name="agg",
                                        tag="agg")
                        for b in range(nb):
                            nc.tensor.matmul(
                                agg[:],
                                lhsT=segt[:, b * 128:(b + 1) * 128],
                                rhs=ed[:, b, :],
                                start=(b == 0), stop=(b == nb - 1))
                        aggs = mpool.tile([128, dim], dt.bfloat16, name="aggs",
                                          tag="aggs")
                        nc.vector.tensor_copy(aggs[:], agg[:])
                        # transpose to feature-major for the conv matmul
                        aggT = mpool.tile([128, dim], dt.bfloat16,
                                          name="aggT", tag="aggT")
                        for f in range(dc):
                            tp = pcnv.tile([128, 128], dt.bfloat16,
                                           name="tpc", tag="cnv")
                            nc.tensor.transpose(
                                tp[:], aggs[:, f * 128:(f + 1) * 128],
                                wsb["ident16"][:])
                            nc.vector.tensor_copy(
                                aggT[:, f * 128:(f + 1) * 128], tp[:])
                        cps_f = [pcnv.tile([128, 128], dt.float32,
                                           name=f"cps{f}", tag="cnv")
                                 for f in range(dc)]
                        for f2 in range(dc):
                            for k in range(dc):
                                nc.tensor.matmul(
                                    cps_f[f2][:],
                                    lhsT=wc[:, k, f2 * 128:(f2 + 1) * 128],
                                    rhs=aggT[:, k * 128:(k + 1) * 128],
                                    start=(k == 0), stop=(k == dc - 1))
                        hstage = gpool.tile([128, dc, 128], dt.bfloat16,
                                            name="hstage", tag="hstage")
                        for f2 in range(dc):
                            nc.vector.tensor_scalar(
                                hstage[:, f2, :],
                                cps_f[f2][:],
                                bc[:, f2:f2 + 1], 0.0, ALU.add, ALU.max)
                        nc.sync.dma_start(hout[:, :, t * 128:(t + 1) * 128],
                                          hstage[:])
                    houts.append(hout)

                # ---- b) mlp_rnd on concat(h1, h2), interleaved with the
                # a-step + chunked AllGather for the next stage
                wma = wsb[f"wm{rnd}a"]
                wmb = wsb[f"wm{rnd}b"]
                bma = wsb[f"bm{rnd}a"]
                bmb = wsb[f"bm{rnd}b"]
                h_next = hpool.tile([128, dc, tpad], dt.bfloat16,
                                    name=f"hm{rnd}T", tag="hT")
                if rnd == 1:
                    g_loc2 = dpool.tile([tpad, 2 * dim], dt.bfloat16,
                                        name="g_loc2", tag="g_loc2")
                    g_full2 = dpool.tile([g_rows, 2 * dim], dt.bfloat16,
                                         name="g_full2", tag="g_full2")
                    g_fulls[2] = g_full2
                for c in range(NCHUNK):
                    for (n0, nw) in _nchunks(cw[c], 512, base=cum[c]):
                        ps1 = []
                        for f in range(dc):
                            p_ = pbig.tile([128, 512], dt.float32, name="psm1",
                                           tag="mlp")
                            ps1.append(p_)
                        for k in range(2 * dc):
                            rhs_src = houts[0] if k < dc else houts[1]
                            rhs_t = rpool.tile([128, 512], dt.bfloat16,
                                               name="ht", tag="ht")
                            nc.sync.dma_start(rhs_t[:, :nw],
                                              rhs_src[:, k % dc, n0:n0 + nw])
                            for f in range(dc):
                                nc.tensor.matmul(
                                    ps1[f][:, :nw],
                                    lhsT=wma[:, k, f * 128:(f + 1) * 128],
                                    rhs=rhs_t[:, :nw],
                                    start=(k == 0), stop=(k == 2 * dc - 1))
                        am = []
                        for f in range(dc):
                            a_ = apool.tile([128, 512], dt.bfloat16, name="am",
                                            tag="a1")
                            nc.scalar.activation(a_[:, :nw], ps1[f][:, :nw],
                                                 AF.Relu, bias=bma[:, f:f + 1])
                            am.append(a_)
                        for f2 in range(dc):
                            p2 = pbig.tile([128, 512], dt.float32, name="psm2",
                                           tag="mlp")
                            for k2 in range(dc):
                                nc.tensor.matmul(
                                    p2[:, :nw],
                                    lhsT=wmb[:, k2, f2 * 128:(f2 + 1) * 128],
                                    rhs=am[k2][:, :nw],
                                    start=(k2 == 0), stop=(k2 == dc - 1))
                            nc.vector.tensor_scalar(
                                h_next[:, f2, n0:n0 + nw], p2[:, :nw],
                                bmb[:, f2:f2 + 1], None, ALU.add)
                    if rnd == 1:
                        for t in ctiles[c]:
                            gt = gpool.tile([128, 2 * dim], dt.bfloat16,
                                            name="gt", tag="gt")
                            a_step(h_next, t, gt)
                            nc.sync.dma_start(
                                g_loc2[t * 128:(t + 1) * 128, :], gt[:])
                        nc.gpsimd.collective_compute(
                            "AllGather", ALU.bypass, replica_groups=rg,
                            ins=[g_loc2[cum[c]:cum[c + 1], :]],
                            outs=[g_full2[ncores * cum[c]:
                                          ncores * cum[c + 1], :]])
                    else:
                        # final h: transpose only (no dinv scaling)
                        for t in ctiles[c]:
                            gt = gpool.tile([128, 2 * dim], dt.bfloat16,
                                            name="gtf", tag="gt")
                            for f in range(dc):
                                tp = pcnv.tile([128, 128], dt.bfloat16,
                                               name="trpf", tag="cnv")
                                nc.tensor.transpose(
                                    tp[:], h_next[:, f, t * 128:(t + 1) * 128],
                                    wsb["ident16"][:])
                                nc.vector.tensor_copy(
                                    gt[:, f * 128:(f + 1) * 128], tp[:])
                            nc.sync.dma_start(
                                hf_loc[t * 128:(t + 1) * 128, :], gt[:, :dim])
                        nc.gpsimd.collective_compute(
                            "AllGather", ALU.bypass, replica_groups=rg,
                            ins=[hf_loc[cum[c]:cum[c + 1], :]],
                            outs=[hf_full[ncores * cum[c]:
                                          ncores * cum[c + 1], :]])
                h_cur = h_next

            # =========== Phase 3: readout (bin-sharded scatter-mean)
            rcat = mpool.tile([128, 2 * dc, bpad], dt.bfloat16, name="rcat",
                              tag="rcat")
            for i in (1, 2):
                pr = ro[i]["prep"]
                for t in range(bt):
                    nbl, nbh = pr["nb_lo"][t], pr["nb_hi"][t]
                    nb = nbl + nbh
                    ob = pr["off_nb"][t]
                    idxt = idxpool.tile([128, nb_max * 8], dt.int16,
                                        name="idxtr", tag="idx")
                    nc.sync.dma_start(idxt[:, :nb * 8],
                                      idxr_in[i][:, ob * 8:(ob + nb) * 8])
                    ddsct = rpool.tile([128, nb_max, 2], dt.float32,
                                       name="ddsctr", tag="ddsc")
                    nc.sync.dma_start(ddsct[:, :nb, :],
                                      ddscr_in[i][:, ob:ob + nb, :])
                    segt = segpool.tile([128, nb_max * 128], dt.bfloat16,
                                        name="segtr", tag="seg")
                    for b in range(nb):
                        nc.vector.tensor_scalar(
                            segt[:, b * 128:(b + 1) * 128],
                            wsb["iota"][:], ddsct[:, b, 0:1],
                            ddsct[:, b, 1:2], ALU.is_equal, ALU.mult)
                    ed = edpool.tile([128, nb_max, dim], dt.bfloat16,
                                     name="edr", tag="ed")
                    nc.gpsimd.dma_gather(
                        ed[:, 0:nbl, :], hf_full[:],
                        idxt[:, 0:nbl * 8],
                        nbl * 128, nbl * 128, dim,
                        single_packet=False, queue_num=next_q())
                    if nbh:
                        nc.gpsimd.dma_gather(
                            ed[:, nbl:nb, :], hf_full[SPLIT:g_rows, :],
                            idxt[:, nbl * 8:nb * 8],
                            nbh * 128, nbh * 128, dim,
                            single_packet=False, queue_num=next_q())
                    agg = pagg.tile([128, dim], dt.float32, name="aggr",
                                    tag="agg")
                    for b in range(nb):
                        nc.tensor.matmul(
                            agg[:],
                            lhsT=segt[:, b * 128:(b + 1) * 128],
                            rhs=ed[:, b, :],
                            start=(b == 0), stop=(b == nb - 1))
                    aggs = mpool.tile([128, dim], dt.bfloat16, name="aggsr",
                                      tag="aggs")
                    nc.vector.tensor_copy(aggs[:], agg[:])
                    for f in range(dc):
                        tp = pcnv.tile([128, 128], dt.bfloat16,
                                       name="tpr", tag="cnv")
                        nc.tensor.transpose(
                            tp[:], aggs[:, f * 128:(f + 1) * 128],
                            wsb["ident16"][:])
                        nc.vector.tensor_copy(
                            rcat[:, (i - 1) * dc + f, t * 128:(t + 1) * 128],
                            tp[:])

            # ---- final MLP + log_softmax
            logitsT = mpool.tile([128, bpad], dt.float32, name="logitsT",
                                 tag="logitsT")
            nc.vector.memset(logitsT[:], 0.0)
            for (n0, nw) in _nchunks(bpad, 512):
                ps1 = []
                for f in range(dc):
                    p_ = pbig.tile([128, 512], dt.float32, name="psf1",
                                   tag="mlp")
                    ps1.append(p_)
                for k in range(2 * dc):
                    for f in range(dc):
                        nc.tensor.matmul(
                            ps1[f][:, :nw],
                            lhsT=wsb["wfa"][:, k, f * 128:(f + 1) * 128],
                            rhs=rcat[:, k, n0:n0 + nw],
                            start=(k == 0), stop=(k == 2 * dc - 1))
                af = []
                for f in range(dc):
                    a_ = apool.tile([128, 512], dt.bfloat16, name="af",
                                    tag="a1")
                    nc.scalar.activation(a_[:, :nw], ps1[f][:, :nw], AF.Relu,
                                         bias=wsb["bfa"][:, f:f + 1])
                    af.append(a_)
                pl = pbig.tile([128, 512], dt.float32, name="psl", tag="mlp")
                for k2 in range(dc):
                    nc.tensor.matmul(
                        pl[:ncls, :nw],
                        lhsT=wsb["wfb"][:, k2, :ncls],
                        rhs=af[k2][:, :nw],
                        start=(k2 == 0), stop=(k2 == dc - 1))
                nc.vector.tensor_scalar(
                    logitsT[:ncls, n0:n0 + nw], pl[:ncls, :nw],
                    wsb["bfb"][:ncls, 0:1], None, ALU.add)

            for t in range(bt):
                ltp = pcnv.tile([128, 128], dt.float32, name="ltp", tag="cnv")
                nc.tensor.transpose(
                    ltp[:], logitsT[:, t * 128:(t + 1) * 128],
                    wsb["ident32"][:])
                mx = mpool.tile([128, 1], dt.float32, name="mx", tag="mx")
                nc.vector.tensor_reduce(mx[:], ltp[:, :ncls],
                                        mybir.AxisListType.X, ALU.max)
                z = mpool.tile([128, ncls], dt.float32, name="z", tag="z")
                nc.vector.tensor_scalar(z[:], ltp[:, :ncls], mx[:, 0:1], None,
                                        ALU.subtract)
                ez = mpool.tile([128, ncls], dt.float32, name="ez", tag="z")
                nc.scalar.activation(ez[:], z[:], AF.Exp)
                sm = mpool.tile([128, 1], dt.float32, name="sm", tag="mx")
                nc.vector.tensor_reduce(sm[:], ez[:], mybir.AxisListType.X,
                                        ALU.add)
                ls = mpool.tile([128, 1], dt.float32, name="ls", tag="mx")
                nc.scalar.activation(ls[:], sm[:], AF.Ln)
                o = mpool.tile([128, ncls], dt.float32, name="o", tag="z")
                nc.vector.tensor_scalar(o[:], z[:], ls[:, 0:1], None,
                                        ALU.subtract)
                nc.sync.dma_start(out_dram[t * 128:(t + 1) * 128, :], o[:])

    nc.compile()
    return nc


_CACHE = {}


def build_in_maps(cfg):
    in_maps = []
    for p in range(cfg["ncores"]):
        m = dict(
            xT=cfg["xT"][p],
            idx1=cfg["rel"][1]["prep"]["idx"][p],
            ddsc1=cfg["rel"][1]["prep"]["ddsc"][p],
            idx2=cfg["rel"][2]["prep"]["idx"][p],
            ddsc2=cfg["rel"][2]["prep"]["ddsc"][p],
            dinvn1=cfg["rel"][1]["dinv_n"][p],
            dinvn2=cfg["rel"][2]["dinv_n"][p],
            idxr1=cfg["ro"][1]["prep"]["idx"][p],
            ddscr1=cfg["ro"][1]["prep"]["ddsc"][p],
            idxr2=cfg["ro"][2]["prep"]["idx"][p],
            ddscr2=cfg["ro"][2]["prep"]["ddsc"][p],
        )
        m.update({k: v for k, v in cfg["w"].items()})
        in_maps.append(m)
    return in_maps


def kernel(**inputs) -> np.ndarray:
    cfg = host_prep(inputs)
    key = (
        cfg["t_nodes"], cfg["f_in"], cfg["dim"], cfg["ncls"], cfg["n_bins"],
        tuple(tuple(cfg["rel"][r]["prep"][k]) for r in (1, 2)
              for k in ("nb_lo", "nb_hi")),
        tuple(tuple(cfg["ro"][i]["prep"][k]) for i in (1, 2)
              for k in ("nb_lo", "nb_hi")),
    )
    if key not in _CACHE:
        _CACHE[key] = build_program(cfg)
    nc = _CACHE[key]

    from concourse.bass_utils import run_bass_kernel_spmd

    in_maps = build_in_maps(cfg)
    res = run_bass_kernel_spmd(nc, in_maps, list(range(cfg["ncores"])))
    outs = [res.results[p]["out"][: cfg["bpc"]] for p in range(cfg["ncores"])]
    return np.ascontiguousarray(np.concatenate(outs, axis=0), np.float32)


# revision 7
# speedup vs baseline: 1.1559x; 1.1559x over previous
"""Trainium2 Bass kernel for nn_Net_50620484551136 (gnn_message_passing).

Network (see problem reference):
  h  = MLP(x)                     # 4652 -> 256 -> 256
  h1 = relu(GCN(h, e1)); h2 = relu(GCN(h, e2))
  h  = MLP([h1, h2])              # 512 -> 256 -> 256
  h1 = relu(GCN(h, e1)); h2 = relu(GCN(h, e2))
  h  = MLP([h1, h2])
  r1 = scatter_mean(h, index_1, N); r2 = scatter_mean(h, index_2, N)
  out = log_softmax(MLP([r1, r2]))

Strategy (8 NeuronCores, SPMD single program):
  - Tuple nodes sharded contiguously across cores (6250/core, padded to 6272).
  - All dense matmuls run feature-major (h^T: [feat, node]) in bf16, fp32 PSUM.
  - GCN: matmul commutes with aggregation, so we aggregate g = h * dinv[src]
    (node-major, bf16) and apply the conv weight after.  Each round: write
    g1|g2 locally, AllGather (4 node-range chunks, overlapped with the MLP
    that produces them) to a full [50176, 512] chunk-major buffer, then each
    core gathers its incoming-edge rows (sorted by dst) with
    gpsimd.dma_gather and segment-sums them with PE matmuls against one-hot
    SEG blocks built ON DEVICE from compact (dd, scale) pairs via
    DVE iota==dd * scale (SEG carries dinv[dst]).
  - dma_gather indices are int16, so gathers are split into a low range
    (rows < 32768) and a high range; block counts are per-tile (max over the
    8 cores) so one static program serves all cores with minimal padding.
  - Scatter-mean readout: output bins sharded across cores (625/core, padded
    to 640); same gather+SEG machinery against the AllGathered final h, with
    1/count folded into the SEG scale.  Final MLP + log_softmax on device;
    host concatenates the 8 output shards.
"""

import numpy as np
import ml_dtypes

BF16 = ml_dtypes.bfloat16

# Problem constants (hardcoded per harness contract).
T = 50000
N_BINS = 5000
F_IN = 4652
DIM = 256
N_CLASSES = 5
NCORES = 8
SPLIT = 32768  # int16 gather index limit
NCHUNK = 4     # AllGather chunks per buffer


def _ceil_to(x, m):
    return (x + m - 1) // m * m


def _wrap_idx(v):
    """int16 index vector (len % 16 == 0) -> [128, len/16] wrapped layout."""
    assert len(v) % 16 == 0
    w = v.reshape(-1, 16).T.astype(np.int16)  # [16, len/16]
    return np.tile(w, (8, 1))  # [128, len/16]


def _chunk_weight(w, dtype=BF16):
    """[K, M] -> [128, ceil(K/128), M] (partition = k%128, block = k//128)."""
    k, m = w.shape
    kp = _ceil_to(k, 128)
    wp = np.zeros((kp, m), np.float32)
    wp[:k] = w
    return np.ascontiguousarray(
        wp.reshape(kp // 128, 128, m).transpose(1, 0, 2)
    ).astype(dtype)


def _chunk_bias(b):
    """[M] -> [128, ceil(M/128)] f32 (partition = m%128, col = m//128)."""
    m = len(b)
    mp = _ceil_to(m, 128)
    bp = np.zeros(mp, np.float32)
    bp[:m] = b
    return np.ascontiguousarray(bp.reshape(mp // 128, 128).T).astype(np.float32)


def _chunk_widths(pad):
    """Split `pad` (multiple of 128) into NCHUNK widths, each mult of 128."""
    ntile = pad // 128
    per = ntile // NCHUNK
    ws = [per * 128] * (NCHUNK - 1)
    ws.append(pad - sum(ws))
    return ws


def _cm_rows(src, spc, spad, ncores):
    """Chunk-major global row id for each source node (vectorized).

    Layout: for chunk c (widths from _chunk_widths(spad)), rows
    [ncores*cum[c], ncores*cum[c+1]) hold [rank0 rows, rank1 rows, ...].
    """
    ws = _chunk_widths(spad)
    cum = np.cumsum([0] + ws)  # [NCHUNK+1]
    p = src // spc
    l = src % spc
    c = np.minimum(np.searchsorted(cum, l, side="right") - 1, NCHUNK - 1)
    return ncores * cum[c] + p * np.array(ws)[c] + (l - cum[c])


def _prep_edges(src, dst, dpc, dpad, spc, spad, ncores, seg_scale):
    """Per-core gather indices + compact SEG inputs for one (src -> dst)
    relation.  dst space is sharded dpc-per-core (padded dpad); src rows live
    in a chunk-major AllGathered buffer (see _cm_rows).  Aggregation output
    for dst d is sum over edges e with dst==d of seg_scale[d] * g[src_e].

    Per-tile block counts are variable (max over cores).  Returns dict with
    per-core idx/ddsc arrays plus global per-tile nb_lo/nb_hi lists.
    """
    nt = dpad // 128
    order = np.argsort(dst, kind="stable")
    src = src[order]
    dst = dst[order]
    core_of = dst // dpc
    gsrc = _cm_rows(src, spc, spad, ncores)

    per_core = []  # [p][t] = (lo_gs, hi_gs, lo_dd, hi_dd)
    cnt_lo = np.zeros((ncores, nt), np.int64)
    cnt_hi = np.zeros((ncores, nt), np.int64)
    for p in range(ncores):
        sel = core_of == p
        sp = gsrc[sel]
        ld = dst[sel] - p * dpc
        tiles = []
        for t in range(nt):
            m = (ld // 128) == t
            st = sp[m]
            dd = (ld[m] - t * 128).astype(np.int64)
            lo = st < SPLIT
            tiles.append((st[lo], st[~lo] - SPLIT, dd[lo], dd[~lo]))
            cnt_lo[p, t] = lo.sum()
            cnt_hi[p, t] = (~lo).sum()
        per_core.append(tiles)

    nb_lo = [int(_ceil_to(max(cnt_lo[:, t].max(), 1), 128) // 128)
             for t in range(nt)]
    nb_hi = [int(_ceil_to(cnt_hi[:, t].max(), 128) // 128) for t in range(nt)]
    nb_tot = [nb_lo[t] + nb_hi[t] for t in range(nt)]
    off_nb = np.cumsum([0] + nb_tot).tolist()  # per-tile block offset
    tot_nb = off_nb[-1]

    idx_arrs = []
    ddsc_arrs = []
    for p in range(ncores):
        idx_a = np.zeros((128, tot_nb * 8), np.int16)
        ddsc_a = np.zeros((128, tot_nb, 2), np.float32)
        ddsc_a[:, :, 0] = -1.0  # dd=-1 -> SEG row zero
        for t in range(nt):
            lo_gs, hi_gs, lo_dd, hi_dd = per_core[p][t]
            ob = off_nb[t]
            li = np.zeros(nb_lo[t] * 128, np.int64)
            li[: len(lo_gs)] = lo_gs
            idx_a[:, ob * 8: (ob + nb_lo[t]) * 8] = _wrap_idx(
                li.astype(np.int16))
            if nb_hi[t]:
                hi = np.zeros(nb_hi[t] * 128, np.int64)
                hi[: len(hi_gs)] = hi_gs
                idx_a[:, (ob + nb_lo[t]) * 8: (ob + nb_tot[t]) * 8] = \
                    _wrap_idx(hi.astype(np.int16))
            base = p * dpc + t * 128
            for boff, dd_list in ((0, lo_dd), (nb_lo[t], hi_dd)):
                i = np.arange(len(dd_list))
                b = boff + i // 128
                e = i % 128
                ddsc_a[e, ob + b, 0] = dd_list.astype(np.float32)
                ddsc_a[e, ob + b, 1] = seg_scale[base + dd_list]
        idx_arrs.append(idx_a)
        ddsc_arrs.append(np.ascontiguousarray(ddsc_a))
    return dict(nb_lo=nb_lo, nb_hi=nb_hi, off_nb=off_nb, tot_nb=tot_nb,
                idx=idx_arrs, ddsc=ddsc_arrs)


def host_prep(inputs, ncores=NCORES, n_bins=None):
    """Pure-numpy preprocessing: sharding, edge sorting, idx/ddsc
    construction, weight layout.  Only index arithmetic + data movement."""
    x = np.asarray(inputs["x"], np.float32)
    t_nodes, f_in = x.shape
    dim = np.asarray(inputs["W_i2"]).shape[0]
    ncls = np.asarray(inputs["b_fb"]).shape[0]
    if n_bins is None:
        if t_nodes == T and f_in == F_IN:
            n_bins = N_BINS
        else:
            n_bins = int(np.asarray(inputs["index_1"]).max()) + 1

    assert t_nodes % ncores == 0, (t_nodes, ncores)
    tpc = t_nodes // ncores
    tpad = _ceil_to(tpc, 128)
    nt = tpad // 128
    kin = _ceil_to(f_in, 128)
    assert n_bins % ncores == 0, (n_bins, ncores)
    bpc = n_bins // ncores
    bpad = _ceil_to(bpc, 128)
    bt = bpad // 128

    cfg = dict(
        t_nodes=t_nodes, f_in=f_in, dim=dim, ncls=ncls, n_bins=n_bins,
        ncores=ncores, tpc=tpc, tpad=tpad, nt=nt, kin=kin, kc=kin // 128,
        bpc=bpc, bpad=bpad, bt=bt, g_rows=ncores * tpad,
    )

    # ---- edge relations (with self-loops), degree norm
    rel = {}
    for r, key in ((1, "edge_index_1"), (2, "edge_index_2")):
        ei = np.asarray(inputs[key]).astype(np.int64)
        loop = np.arange(t_nodes, dtype=np.int64)
        s = np.concatenate([ei[0], loop])
        d = np.concatenate([ei[1], loop])
        deg = np.bincount(d, minlength=t_nodes).astype(np.float64)
        dinv = (1.0 / np.sqrt(np.maximum(deg, 1.0))).astype(np.float32)
        rel[r] = dict(
            prep=_prep_edges(s, d, tpc, tpad, tpc, tpad, ncores, dinv),
            dinv=dinv,
        )
    cfg["rel"] = rel

    # ---- readout (scatter-mean): treat (node -> bin) as edges
    ro = {}
    for i, key in ((1, "index_1"), (2, "index_2")):
        idx = np.asarray(inputs[key]).astype(np.int64)
        cnt = np.bincount(idx, minlength=n_bins).astype(np.float64)
        invc = (1.0 / np.maximum(cnt, 1.0)).astype(np.float32)
        nodes = np.arange(t_nodes, dtype=np.int64)
        ro[i] = dict(
            prep=_prep_edges(nodes, idx, bpc, bpad, tpc, tpad, ncores, invc),
        )
    cfg["ro"] = ro

    # ---- per-core x^T slices (bf16) in sub-chunked layout
    # [128, nsub, kc, SUBW]: partition = k%128, sub-chunk of SUBW node
    # columns, contiguous per (partition, sub) for a single fat DMA.
    SUBW = 256
    nsub = _ceil_to(tpad, SUBW) // SUBW
    cfg["subw"] = SUBW
    cfg["nsub"] = nsub
    kc = kin // 128
    xT = []
    for p in range(ncores):
        xs = np.zeros((kin, nsub * SUBW), np.float32)
        xs[:f_in, :tpc] = x[p * tpc: (p + 1) * tpc].T
        # [kc, 128, nsub, SUBW] -> [128, nsub, kc, SUBW]
        a = xs.reshape(kc, 128, nsub, SUBW).transpose(1, 2, 0, 3)
        xT.append(np.ascontiguousarray(a).astype(BF16))
    cfg["xT"] = xT

    # ---- dinv per-node tiles [128, nt] f32 per relation per core
    for r in (1, 2):
        dn = []
        dinv = rel[r]["dinv"]
        for p in range(ncores):
            a = np.zeros((128, nt), np.float32)
            vp = np.zeros(tpad, np.float32)
            vp[:tpc] = dinv[p * tpc: (p + 1) * tpc]
            a[:, :] = vp.reshape(nt, 128).T
            dn.append(a)
        rel[r]["dinv_n"] = dn

    # ---- weights
    w = {}
    w["wi1"] = _chunk_weight(np.asarray(inputs["W_i1"], np.float32))
    w["wi2"] = _chunk_weight(np.asarray(inputs["W_i2"], np.float32))
    for nm, src in (("wc11", "Wc11"), ("wc12", "Wc12"),
                    ("wc21", "Wc21"), ("wc22", "Wc22"),
                    ("wm1a", "W_m1a"), ("wm1b", "W_m1b"),
                    ("wm2a", "W_m2a"), ("wm2b", "W_m2b"),
                    ("wfa", "W_fa"), ("wfb", "W_fb")):
        w[nm] = _chunk_weight(np.asarray(inputs[src], np.float32))
    for nm, src in (("bi1", "b_i1"), ("bi2", "b_i2"),
                    ("bc11", "bc11"), ("bc12", "bc12"),
                    ("bc21", "bc21"), ("bc22", "bc22"),
                    ("bm1a", "b_m1a"), ("bm1b", "b_m1b"),
                    ("bm2a", "b_m2a"), ("bm2b", "b_m2b"),
                    ("bfa", "b_fa"), ("bfb", "b_fb")):
        w[nm] = _chunk_bias(np.asarray(inputs[src], np.float32))
    w["ident16"] = np.eye(128, dtype=BF16)
    w["ident32"] = np.eye(128, dtype=np.float32)
    w["iota"] = np.tile(np.arange(128, dtype=np.float32), (128, 1))
    cfg["w"] = w
    return cfg


def _nchunks(total, step, base=0):
    out = []
    o = 0
    while o < total:
        out.append((base + o, min(step, total - o)))
        o += step
    return out


def build_program(cfg):
    """Build the SPMD bass program (one program, 8 cores)."""
    import concourse.bass as bass
    import concourse.mybir as mybir
    import concourse.tile as tile
    from concourse import bacc

    dt = mybir.dt
    AF = mybir.ActivationFunctionType
    ALU = mybir.AluOpType

    nt, tpad, kc = cfg["nt"], cfg["tpad"], cfg["kc"]
    bt, bpad = cfg["bt"], cfg["bpad"]
    dim, ncls = cfg["dim"], cfg["ncls"]
    dc = dim // 128
    g_rows = cfg["g_rows"]
    ncores = cfg["ncores"]
    rel, ro = cfg["rel"], cfg["ro"]
    rg = [list(range(ncores))]
    SUBW, nsub = cfg["subw"], cfg["nsub"]

    cw = _chunk_widths(tpad)           # node-range chunk widths
    cum = np.cumsum([0] + cw).tolist()  # local row offsets
    ctiles = [range(cum[c] // 128, cum[c + 1] // 128) for c in range(NCHUNK)]

    nc = bacc.Bacc("TRN2", target_bir_lowering=False, debug=False,
                   num_devices=ncores, num_swdge_queues=4)
    qstate = [0]

    def next_q():
        q = qstate[0]
        qstate[0] = (q + 1) % 4
        return q

    # ---------------- I/O declarations ----------------
    xT = nc.dram_tensor("xT", [128, nsub, kc, SUBW], dt.bfloat16,
                        kind="ExternalInput")
    idx_in, ddsc_in, dinvn_in = {}, {}, {}
    for r in (1, 2):
        pr = rel[r]["prep"]
        idx_in[r] = nc.dram_tensor(f"idx{r}", [128, pr["tot_nb"] * 8],
                                   dt.int16, kind="ExternalInput")
        ddsc_in[r] = nc.dram_tensor(f"ddsc{r}", [128, pr["tot_nb"], 2],
                                    dt.float32, kind="ExternalInput")
        dinvn_in[r] = nc.dram_tensor(f"dinvn{r}", [128, nt], dt.float32,
                                     kind="ExternalInput")
    idxr_in, ddscr_in = {}, {}
    for i in (1, 2):
        pr = ro[i]["prep"]
        idxr_in[i] = nc.dram_tensor(f"idxr{i}", [128, pr["tot_nb"] * 8],
                                    dt.int16, kind="ExternalInput")
        ddscr_in[i] = nc.dram_tensor(f"ddscr{i}", [128, pr["tot_nb"], 2],
                                     dt.float32, kind="ExternalInput")

    wnames_bf = dict(
        wi1=[128, kc, dim], wi2=[128, dc, dim],
        wc11=[128, dc, dim], wc12=[128, dc, dim],
        wc21=[128, dc, dim], wc22=[128, dc, dim],
        wm1a=[128, 2 * dc, dim], wm1b=[128, dc, dim],
        wm2a=[128, 2 * dc, dim], wm2b=[128, dc, dim],
        wfa=[128, 2 * dc, dim], wfb=[128, dc, ncls],
        ident16=[128, 128],
    )
    wnames_f32 = dict(
        bi1=[128, dc], bi2=[128, dc],
        bc11=[128, dc], bc12=[128, dc], bc21=[128, dc], bc22=[128, dc],
        bm1a=[128, dc], bm1b=[128, dc], bm2a=[128, dc], bm2b=[128, dc],
        bfa=[128, dc], bfb=[128, 1],
        ident32=[128, 128], iota=[128, 128],
    )
    win = {}
    for nm, shp in wnames_bf.items():
        win[nm] = nc.dram_tensor(nm, shp, dt.bfloat16, kind="ExternalInput")
    for nm, shp in wnames_f32.items():
        win[nm] = nc.dram_tensor(nm, shp, dt.float32, kind="ExternalInput")

    out_dram = nc.dram_tensor("out", [bpad, ncls], dt.float32,
                              kind="ExternalOutput")

    nb_max = max(
        max(rel[r]["prep"]["nb_lo"][t] + rel[r]["prep"]["nb_hi"][t]
            for r in (1, 2) for t in range(nt)),
        max(ro[i]["prep"]["nb_lo"][t] + ro[i]["prep"]["nb_hi"][t]
            for i in (1, 2) for t in range(bt)),
    )

    with tile.TileContext(nc) as tc:
        with (
            tc.tile_pool(name="wpool", bufs=1) as wpool,
            tc.tile_pool(name="hpool", bufs=1) as hpool,
            tc.tile_pool(name="xpool", bufs=2) as xpool,
            tc.tile_pool(name="rpool", bufs=4) as rpool,
            tc.tile_pool(name="edpool", bufs=4) as edpool,
            tc.tile_pool(name="segpool", bufs=4) as segpool,
            tc.tile_pool(name="idxpool", bufs=6) as idxpool,
            tc.tile_pool(name="apool", bufs=4) as apool,
            tc.tile_pool(name="gpool", bufs=3) as gpool,
            tc.tile_pool(name="mpool", bufs=4) as mpool,
            tc.tile_pool(name="pbig", bufs=3, space="PSUM") as pbig,
            tc.tile_pool(name="pagg", bufs=2, space="PSUM") as pagg,
            tc.tile_pool(name="pcnv", bufs=3, space="PSUM") as pcnv,
            tc.tile_pool(name="dpool", bufs=1, space="DRAM") as dpool,
        ):
            # ---- resident weights
            wsb = {}
            for nm in list(wnames_bf) + list(wnames_f32):
                shp = wnames_bf.get(nm) or wnames_f32[nm]
                dtyp = dt.bfloat16 if nm in wnames_bf else dt.float32
                wt = wpool.tile(shp, dtyp, name=f"sb_{nm}", tag=f"w_{nm}")
                nc.sync.dma_start(wt[:], win[nm][:])
                wsb[nm] = wt
            dinvn_sb = {}
            for r in (1, 2):
                dv = wpool.tile([128, nt], dt.float32, name=f"sb_dinvn{r}",
                                tag=f"w_dinvn{r}")
                nc.sync.dma_start(dv[:], dinvn_in[r][:])
                dinvn_sb[r] = dv

            def a_step(h_src, t, gt_dst):
                """Transpose h tile t to node-major and scale by dinv."""
                trp = []
                for f in range(dc):
                    tp = pcnv.tile([128, 128], dt.bfloat16, name="trp",
                                   tag="cnv")
                    nc.tensor.transpose(
                        tp[:], h_src[:, f, t * 128:(t + 1) * 128],
                        wsb["ident16"][:])
                    trp.append(tp)
                for r in (1, 2):
                    for f in range(dc):
                        nc.vector.tensor_scalar_mul(
                            gt_dst[:, (r - 1) * dim + f * 128:
                                   (r - 1) * dim + (f + 1) * 128],
                            trp[f][:], dinvn_sb[r][:, t:t + 1])

            def conv_tile(pr, g_full, wc, bc, hout, r, t):
                """One (relation, dst-tile) conv step: gather + SEG + W."""
                nbl, nbh = pr["nb_lo"][t], pr["nb_hi"][t]
                nb = nbl + nbh
                ob = pr["off_nb"][t]
                idxt = idxpool.tile([128, nb_max * 8], dt.int16,
                                    name="idxt", tag="idx")
                nc.sync.dma_start(idxt[:, :nb * 8],
                                  idx_in[r][:, ob * 8:(ob + nb) * 8])
                ddsct = rpool.tile([128, nb_max, 2], dt.float32,
                                   name="ddsct", tag="ddsc")
                nc.sync.dma_start(ddsct[:, :nb, :],
                                  ddsc_in[r][:, ob:ob + nb, :])
                segt = segpool.tile([128, nb_max * 128], dt.float8e4,
                                    name="segt", tag="seg")
                for b in range(nb):
                    nc.vector.tensor_scalar(
                        segt[:, b * 128:(b + 1) * 128],
                        wsb["iota"][:], ddsct[:, b, 0:1],
                        ddsct[:, b, 1:2], ALU.is_equal, ALU.mult)
                ed = edpool.tile([128, nb_max, dim], dt.float8e4,
                                 name="ed", tag="ed")
                nc.gpsimd.dma_gather(
                    ed[:, 0:nbl, :],
                    g_full[:, (r - 1) * dim:r * dim],
                    idxt[:, 0:nbl * 8],
                    nbl * 128, nbl * 128, dim,
                    elem_step=2 * dim, single_packet=False,
                    queue_num=next_q())
                if nbh:
                    nc.gpsimd.dma_gather(
                        ed[:, nbl:nb, :],
                        g_full[SPLIT:g_rows, (r - 1) * dim:r * dim],
                        idxt[:, nbl * 8:nb * 8],
                        nbh * 128, nbh * 128, dim,
                        elem_step=2 * dim, single_packet=False,
                        queue_num=next_q())
                agg = pagg.tile([128, dim], dt.float32, name="agg", tag="agg")
                for b in range(nb):
                    nc.tensor.matmul(
                        agg[:],
                        lhsT=segt[:, b * 128:(b + 1) * 128],
                        rhs=ed[:, b, :],
                        start=(b == 0), stop=(b == nb - 1))
                aggs = mpool.tile([128, dim], dt.bfloat16, name="aggs",
                                  tag="aggs")
                nc.vector.tensor_copy(aggs[:], agg[:])
                aggT = mpool.tile([128, dim], dt.bfloat16, name="aggT",
                                  tag="aggT")
                for f in range(dc):
                    tp = pcnv.tile([128, 128], dt.bfloat16, name="tpc",
                                   tag="cnv")
                    nc.tensor.transpose(
                        tp[:], aggs[:, f * 128:(f + 1) * 128],
                        wsb["ident16"][:])
                    nc.vector.tensor_copy(
                        aggT[:, f * 128:(f + 1) * 128], tp[:])
                cps_f = [pcnv.tile([128, 128], dt.float32,
                                   name=f"cps{f}", tag="cnv")
                         for f in range(dc)]
                for f2 in range(dc):
                    for k in range(dc):
                        nc.tensor.matmul(
                            cps_f[f2][:],
                            lhsT=wc[:, k, f2 * 128:(f2 + 1) * 128],
                            rhs=aggT[:, k * 128:(k + 1) * 128],
                            start=(k == 0), stop=(k == dc - 1))
                hstage = gpool.tile([128, dc, 128], dt.bfloat16,
                                    name="hstage", tag="hstage")
                for f2 in range(dc):
                    nc.vector.tensor_scalar(
                        hstage[:, f2, :], cps_f[f2][:],
                        bc[:, f2:f2 + 1], 0.0, ALU.add, ALU.max)
                nc.sync.dma_start(hout[:, :, t * 128:(t + 1) * 128],
                                  hstage[:])

            # =========== Phase 1: input MLP  h0 = relu(x@Wi1+bi1)@Wi2+bi2
            # interleaved per AllGather chunk; AG1_c fires when chunk done.
            h_cur = hpool.tile([128, dc, tpad], dt.bfloat16, name="h0T",
                               tag="hT")
            g_loc1 = dpool.tile([tpad, 2 * dim], dt.float8e4, name="g_loc1",
                                tag="g_loc1")
            g_full1 = dpool.tile([g_rows, 2 * dim], dt.float8e4,
                                 name="g_full1", tag="g_full1")
            for c in range(NCHUNK):
                subs = [s for s in range(nsub)
                        if cum[c] <= s * SUBW < cum[c + 1]]
                for s in subs:
                    n0 = s * SUBW
                    nw = min(SUBW, tpad - n0)
                    xt = xpool.tile([128, kc, SUBW], dt.bfloat16, name="xt",
                                    tag="xt")
                    nc.sync.dma_start(xt[:], xT[:, s])
                    ps1 = []
                    for f in range(dc):
                        p_ = pbig.tile([128, 512], dt.float32, name="ps1",
                                       tag="mlp")
                        ps1.append(p_)
                        for k in range(kc):
                            nc.tensor.matmul(
                                p_[:, :nw],
                                lhsT=wsb["wi1"][:, k, f * 128:(f + 1) * 128],
                                rhs=xt[:, k, :nw],
                                start=(k == 0), stop=(k == kc - 1))
                    a1 = []
                    for f in range(dc):
                        a_ = apool.tile([128, 512], dt.bfloat16, name="a1",
                                        tag="a1")
                        nc.scalar.activation(a_[:, :nw], ps1[f][:, :nw],
                                             AF.Relu, bias=wsb["bi1"][:, f:f + 1])
                        a1.append(a_)
                    for f2 in range(dc):
                        p2 = pbig.tile([128, 512], dt.float32, name="ps2",
                                       tag="mlp")
                        for k2 in range(dc):
                            nc.tensor.matmul(
                                p2[:, :nw],
                                lhsT=wsb["wi2"][:, k2, f2 * 128:(f2 + 1) * 128],
                                rhs=a1[k2][:, :nw],
                                start=(k2 == 0), stop=(k2 == dc - 1))
                        nc.vector.tensor_scalar(
                            h_cur[:, f2, n0:n0 + nw], p2[:, :nw],
                            wsb["bi2"][:, f2:f2 + 1], None, ALU.add)
                for t in ctiles[c]:
                    gt = gpool.tile([128, 2 * dim], dt.float8e4, name="gt",
                                    tag="gt")
                    a_step(h_cur, t, gt)
                    nc.sync.dma_start(g_loc1[t * 128:(t + 1) * 128, :], gt[:])
                nc.gpsimd.collective_compute(
                    "AllGather", ALU.bypass, replica_groups=rg,
                    ins=[g_loc1[cum[c]:cum[c + 1], :]],
                    outs=[g_full1[ncores * cum[c]:ncores * cum[c + 1], :]])

            # =========== Phase 2: two GCN rounds
            g_fulls = {1: g_full1}
            hf_loc = dpool.tile([tpad, dim], dt.bfloat16, name="hf_loc",
                                tag="hf_loc")
            hf_full = dpool.tile([g_rows, dim], dt.bfloat16, name="hf_full",
                                 tag="hf_full")
            for rnd in (1, 2):
                g_full = g_fulls[rnd]
                wma = wsb[f"wm{rnd}a"]
                wmb = wsb[f"wm{rnd}b"]
                bma = wsb[f"bm{rnd}a"]
                bmb = wsb[f"bm{rnd}b"]
                houts = [dpool.tile([128, dc, tpad], dt.bfloat16,
                                    name=f"h{r}T", tag=f"h12_{rnd}{r}")
                         for r in (1, 2)]
                h_next = hpool.tile([128, dc, tpad], dt.bfloat16,
                                    name=f"hm{rnd}T", tag="hT")
                if rnd == 1:
                    g_loc2 = dpool.tile([tpad, 2 * dim], dt.float8e4,
                                        name="g_loc2", tag="g_loc2")
                    g_full2 = dpool.tile([g_rows, 2 * dim], dt.float8e4,
                                         name="g_full2", tag="g_full2")
                    g_fulls[2] = g_full2
                # software pipeline: chunk c's conv tiles, then chunk c's
                # MLP + a-step + AllGather (overlaps chunk c+1's conv)
                for c in range(NCHUNK):
                    for t in ctiles[c]:
                        for r in (1, 2):
                            conv_tile(rel[r]["prep"], g_full,
                                      wsb[f"wc{rnd}{r}"], wsb[f"bc{rnd}{r}"],
                                      houts[r - 1], r, t)
                    for (n0, nw) in _nchunks(cw[c], 512, base=cum[c]):
                        ps1 = []
                        for f in range(dc):
                            p_ = pbig.tile([128, 512], dt.float32, name="psm1",
                                           tag="mlp")
                            ps1.append(p_)
                        for k in range(2 * dc):
                            rhs_src = houts[0] if k < dc else houts[1]
                            rhs_t = rpool.tile([128, 512], dt.bfloat16,
                                               name="ht", tag="ht")
                            nc.sync.dma_start(rhs_t[:, :nw],
                                              rhs_src[:, k % dc, n0:n0 + nw])
                            for f in range(dc):
                                nc.tensor.matmul(
                                    ps1[f][:, :nw],
                                    lhsT=wma[:, k, f * 128:(f + 1) * 128],
                                    rhs=rhs_t[:, :nw],
                                    start=(k == 0), stop=(k == 2 * dc - 1))
                        am = []
                        for f in range(dc):
                            a_ = apool.tile([128, 512], dt.bfloat16, name="am",
                                            tag="a1")
                            nc.scalar.activation(a_[:, :nw], ps1[f][:, :nw],
                                                 AF.Relu, bias=bma[:, f:f + 1])
                            am.append(a_)
                        for f2 in range(dc):
                            p2 = pbig.tile([128, 512], dt.float32, name="psm2",
                                           tag="mlp")
                            for k2 in range(dc):
                                nc.tensor.matmul(
                                    p2[:, :nw],
                                    lhsT=wmb[:, k2, f2 * 128:(f2 + 1) * 128],
                                    rhs=am[k2][:, :nw],
                                    start=(k2 == 0), stop=(k2 == dc - 1))
                            nc.vector.tensor_scalar(
                                h_next[:, f2, n0:n0 + nw], p2[:, :nw],
                                bmb[:, f2:f2 + 1], None, ALU.add)
                    if rnd == 1:
                        for t in ctiles[c]:
                            gt = gpool.tile([128, 2 * dim], dt.float8e4,
                                            name="gt", tag="gt")
                            a_step(h_next, t, gt)
                            nc.sync.dma_start(
                                g_loc2[t * 128:(t + 1) * 128, :], gt[:])
                        nc.gpsimd.collective_compute(
                            "AllGather", ALU.bypass, replica_groups=rg,
                            ins=[g_loc2[cum[c]:cum[c + 1], :]],
                            outs=[g_full2[ncores * cum[c]:
                                          ncores * cum[c + 1], :]])
                    else:
                        # final h: transpose only (no dinv scaling)
                        for t in ctiles[c]:
                            gt = gpool.tile([128, 2 * dim], dt.bfloat16,
                                            name="gtf", tag="gt")
                            for f in range(dc):
                                tp = pcnv.tile([128, 128], dt.bfloat16,
                                               name="trpf", tag="cnv")
                                nc.tensor.transpose(
                                    tp[:], h_next[:, f, t * 128:(t + 1) * 128],
                                    wsb["ident16"][:])
                                nc.vector.tensor_copy(
                                    gt[:, f * 128:(f + 1) * 128], tp[:])
                            nc.sync.dma_start(
                                hf_loc[t * 128:(t + 1) * 128, :], gt[:, :dim])
                        nc.gpsimd.collective_compute(
                            "AllGather", ALU.bypass, replica_groups=rg,
                            ins=[hf_loc[cum[c]:cum[c + 1], :]],
                            outs=[hf_full[ncores * cum[c]:
                                          ncores * cum[c + 1], :]])
                h_cur = h_next

            # =========== Phase 3: readout (bin-sharded scatter-mean)
            rcat = mpool.tile([128, 2 * dc, bpad], dt.bfloat16, name="rcat",
                              tag="rcat")
            for i in (1, 2):
                pr = ro[i]["prep"]
                for t in range(bt):
                    nbl, nbh = pr["nb_lo"][t], pr["nb_hi"][t]
                    nb = nbl + nbh
                    ob = pr["off_nb"][t]
                    idxt = idxpool.tile([128, nb_max * 8], dt.int16,
                                        name="idxtr", tag="idx")
                    nc.sync.dma_start(idxt[:, :nb * 8],
                                      idxr_in[i][:, ob * 8:(ob + nb) * 8])
                    ddsct = rpool.tile([128, nb_max, 2], dt.float32,
                                       name="ddsctr", tag="ddsc")
                    nc.sync.dma_start(ddsct[:, :nb, :],
                                      ddscr_in[i][:, ob:ob + nb, :])
                    segt = segpool.tile([128, nb_max * 128], dt.bfloat16,
                                        name="segtr", tag="seg")
                    for b in range(nb):
                        nc.vector.tensor_scalar(
                            segt[:, b * 128:(b + 1) * 128],
                            wsb["iota"][:], ddsct[:, b, 0:1],
                            ddsct[:, b, 1:2], ALU.is_equal, ALU.mult)
                    ed = edpool.tile([128, nb_max, dim], dt.bfloat16,
                                     name="edr", tag="ed")
                    nc.gpsimd.dma_gather(
                        ed[:, 0:nbl, :], hf_full[:],
                        idxt[:, 0:nbl * 8],
                        nbl * 128, nbl * 128, dim,
                        single_packet=False, queue_num=next_q())
                    if nbh:
                        nc.gpsimd.dma_gather(
                            ed[:, nbl:nb, :], hf_full[SPLIT:g_rows, :],
                            idxt[:, nbl * 8:nb * 8],
                            nbh * 128, nbh * 128, dim,
                            single_packet=False, queue_num=next_q())
                    agg = pagg.tile([128, dim], dt.float32, name="aggr",
                                    tag="agg")
                    for b in range(nb):
                        nc.tensor.matmul(
                            agg[:],
                            lhsT=segt[:, b * 128:(b + 1) * 128],
                            rhs=ed[:, b, :],
                            start=(b == 0), stop=(b == nb - 1))
                    aggs = mpool.tile([128, dim], dt.bfloat16, name="aggsr",
                                      tag="aggs")
                    nc.vector.tensor_copy(aggs[:], agg[:])
                    for f in range(dc):
                        tp = pcnv.tile([128, 128], dt.bfloat16,
                                       name="tpr", tag="cnv")
                        nc.tensor.transpose(
                            tp[:], aggs[:, f * 128:(f + 1) * 128],
                            wsb["ident16"][:])
                        nc.vector.tensor_copy(
                            rcat[:, (i - 1) * dc + f, t * 128:(t + 1) * 128],
                            tp[:])

            # ---- final MLP + log_softmax
            logitsT = mpool.tile([128, bpad], dt.float32, name="logitsT",
                                 tag="logitsT")
            nc.vector.memset(logitsT[:], 0.0)
            for (n0, nw) in _nchunks(bpad, 512):
                ps1 = []
                for f in range(dc):
                    p_ = pbig.tile([128, 512], dt.float32, name="psf1",
                                   tag="mlp")
                    ps1.append(p_)
                for k in range(2 * dc):
                    for f in range(dc):
                        nc.tensor.matmul(
                            ps1[f][:, :nw],
                            lhsT=wsb["wfa"][:, k, f * 128:(f + 1) * 128],
                            rhs=rcat[:, k, n0:n0 + nw],
                            start=(k == 0), stop=(k == 2 * dc - 1))
                af = []
                for f in range(dc):
                    a_ = apool.tile([128, 512], dt.bfloat16, name="af",
                                    tag="a1")
                    nc.scalar.activation(a_[:, :nw], ps1[f][:, :nw], AF.Relu,
                                         bias=wsb["bfa"][:, f:f + 1])
                    af.append(a_)
                pl = pbig.tile([128, 512], dt.float32, name="psl", tag="mlp")
                for k2 in range(dc):
                    nc.tensor.matmul(
                        pl[:ncls, :nw],
                        lhsT=wsb["wfb"][:, k2, :ncls],
                        rhs=af[k2][:, :nw],
                        start=(k2 == 0), stop=(k2 == dc - 1))
                nc.vector.tensor_scalar(
                    logitsT[:ncls, n0:n0 + nw], pl[:ncls, :nw],
                    wsb["bfb"][:ncls, 0:1], None, ALU.add)

            for t in range(bt):
                ltp = pcnv.tile([128, 128], dt.float32, name="ltp", tag="cnv")
                nc.tensor.transpose(
                    ltp[:], logitsT[:, t * 128:(t + 1) * 128],
                    wsb["ident32"][:])
                mx = mpool.tile([128, 1], dt.float32, name="mx", tag="mx")
                nc.vector.tensor_reduce(mx[:], ltp[:, :ncls],
                                        mybir.AxisListType.X, ALU.max)
                z = mpool.tile([128, ncls], dt.float32, name="z", tag="z")
                nc.vector.tensor_scalar(z[:], ltp[:, :ncls], mx[:, 0:1], None,
                                        ALU.subtract)
                ez = mpool.tile([128, ncls], dt.float32, name="ez", tag="z")
                nc.scalar.activation(ez[:], z[:], AF.Exp)
                sm = mpool.tile([128, 1], dt.float32, name="sm", tag="mx")
                nc.vector.tensor_reduce(sm[:], ez[:], mybir.AxisListType.X,
                                        ALU.add)
                ls = mpool.tile([128, 1], dt.float32, name="ls", tag="mx")
                nc.scalar.activation(ls[:], sm[:], AF.Ln)
                o = mpool.tile([128, ncls], dt.float32, name="o", tag="z")
                nc.vector.tensor_scalar(o[:], z[:], ls[:, 0:1], None,
                                        ALU.subtract)
                nc.sync.dma_start(out_dram[t * 128:(t + 1) * 128, :], o[:])

    nc.compile()
    return nc


_CACHE = {}


def build_in_maps(cfg):
    in_maps = []
    for p in range(cfg["ncores"]):
        m = dict(
            xT=cfg["xT"][p],
            idx1=cfg["rel"][1]["prep"]["idx"][p],
            ddsc1=cfg["rel"][1]["prep"]["ddsc"][p],
            idx2=cfg["rel"][2]["prep"]["idx"][p],
            ddsc2=cfg["rel"][2]["prep"]["ddsc"][p],
            dinvn1=cfg["rel"][1]["dinv_n"][p],
            dinvn2=cfg["rel"][2]["dinv_n"][p],
            idxr1=cfg["ro"][1]["prep"]["idx"][p],
            ddscr1=cfg["ro"][1]["prep"]["ddsc"][p],
            idxr2=cfg["ro"][2]["prep"]["idx"][p],
            ddscr2=cfg["ro"][2]["prep"]["ddsc"][p],
        )
        m.update({k: v for k, v in cfg["w"].items()})
        in_maps.append(m)
    return in_maps


def kernel(**inputs) -> np.ndarray:
    cfg = host_prep(inputs)
    key = (
        cfg["t_nodes"], cfg["f_in"], cfg["dim"], cfg["ncls"], cfg["n_bins"],
        tuple(tuple(cfg["rel"][r]["prep"][k]) for r in (1, 2)
              for k in ("nb_lo", "nb_hi")),
        tuple(tuple(cfg["ro"][i]["prep"][k]) for i in (1, 2)
              for k in ("nb_lo", "nb_hi")),
    )
    if key not in _CACHE:
        _CACHE[key] = build_program(cfg)
    nc = _CACHE[key]

    from concourse.bass_utils import run_bass_kernel_spmd

    in_maps = build_in_maps(cfg)
    res = run_bass_kernel_spmd(nc, in_maps, list(range(cfg["ncores"])))
    outs = [res.results[p]["out"][: cfg["bpc"]] for p in range(cfg["ncores"])]
    return np.ascontiguousarray(np.concatenate(outs, axis=0), np.float32)


# revision 24
# speedup vs baseline: 1.6931x; 1.4648x over previous
"""Trainium2 Bass kernel for nn_Net_50620484551136 (gnn_message_passing).

Network (see problem reference):
  h  = MLP(x)                     # 4652 -> 256 -> 256
  h1 = relu(GCN(h, e1)); h2 = relu(GCN(h, e2))
  h  = MLP([h1, h2])              # 512 -> 256 -> 256
  h1 = relu(GCN(h, e1)); h2 = relu(GCN(h, e2))
  h  = MLP([h1, h2])
  r1 = scatter_mean(h, index_1, N); r2 = scatter_mean(h, index_2, N)
  out = log_softmax(MLP([r1, r2]))

Strategy (8 NeuronCores, SPMD single program):
  - Tuple nodes sharded contiguously across cores (6250/core, padded to 6272).
  - All dense matmuls run feature-major (h^T: [feat, node]) in bf16, fp32 PSUM.
  - GCN: matmul commutes with aggregation, so we aggregate g = h * dinv[src]
    (node-major, bf16) and apply the conv weight after.  Each round: write
    g1|g2 locally, AllGather (4 node-range chunks, overlapped with the MLP
    that produces them) to a full [50176, 512] chunk-major buffer, then each
    core gathers its incoming-edge rows (sorted by dst) with
    gpsimd.dma_gather and segment-sums them with PE matmuls against one-hot
    SEG blocks built ON DEVICE from compact (dd, scale) pairs via
    DVE iota==dd * scale (SEG carries dinv[dst]).
  - dma_gather indices are int16, so gathers are split into a low range
    (rows < 32768) and a high range; block counts are per-tile (max over the
    8 cores) so one static program serves all cores with minimal padding.
  - Scatter-mean readout: output bins sharded across cores (625/core, padded
    to 640); same gather+SEG machinery against the AllGathered final h, with
    1/count folded into the SEG scale.  Final MLP + log_softmax on device;
    host concatenates the 8 output shards.
"""

import numpy as np
import ml_dtypes

BF16 = ml_dtypes.bfloat16
F8 = ml_dtypes.float8_e4m3

# Problem constants (hardcoded per harness contract).
T = 50000
N_BINS = 5000
F_IN = 4652
DIM = 256
N_CLASSES = 5
NCORES = 8
SPLIT = 32768  # int16 gather index limit
NCHUNK = 4     # AllGather chunks per buffer


def _ceil_to(x, m):
    return (x + m - 1) // m * m


def _wrap_idx(v):
    """int16 index vector (len % 16 == 0) -> [128, len/16] wrapped layout."""
    assert len(v) % 16 == 0
    w = v.reshape(-1, 16).T.astype(np.int16)  # [16, len/16]
    return np.tile(w, (8, 1))  # [128, len/16]


def _chunk_weight(w, dtype=BF16):
    """[K, M] -> [128, ceil(K/128), M] (partition = k%128, block = k//128)."""
    k, m = w.shape
    kp = _ceil_to(k, 128)
    wp = np.zeros((kp, m), np.float32)
    wp[:k] = w
    return np.ascontiguousarray(
        wp.reshape(kp // 128, 128, m).transpose(1, 0, 2)
    ).astype(dtype)


def _chunk_bias(b):
    """[M] -> [128, ceil(M/128)] f32 (partition = m%128, col = m//128)."""
    m = len(b)
    mp = _ceil_to(m, 128)
    bp = np.zeros(mp, np.float32)
    bp[:m] = b
    return np.ascontiguousarray(bp.reshape(mp // 128, 128).T).astype(np.float32)


def _chunk_widths(pad):
    """Split `pad` (multiple of 128) into NCHUNK widths, each mult of 128."""
    ntile = pad // 128
    per = ntile // NCHUNK
    ws = [per * 128] * (NCHUNK - 1)
    ws.append(pad - sum(ws))
    return ws


def _cm_rows(src, spc, spad, ncores):
    """Chunk-major global row id for each source node (vectorized).

    Layout: for chunk c (widths from _chunk_widths(spad)), rows
    [ncores*cum[c], ncores*cum[c+1]) hold [rank0 rows, rank1 rows, ...].
    """
    ws = _chunk_widths(spad)
    cum = np.cumsum([0] + ws)  # [NCHUNK+1]
    p = src // spc
    l = src % spc
    c = np.minimum(np.searchsorted(cum, l, side="right") - 1, NCHUNK - 1)
    return ncores * cum[c] + p * np.array(ws)[c] + (l - cum[c])


def _prep_edges(src, dst, dpc, dpad, spc, spad, ncores, seg_scale,
                seg_dtype):
    """Per-core gather indices + host-built one-hot SEG blocks for one
    (src -> dst) relation.  dst space is sharded dpc-per-core (padded dpad);
    src rows live in a chunk-major AllGathered buffer (see _cm_rows).
    Aggregation output for dst d is sum over edges e with dst==d of
    seg_scale[d] * g[src_e].

    Per-tile block counts are variable (max over cores).  Returns dict with
    per-core idx/seg arrays plus global per-tile nb_lo/nb_hi lists.
    """
    nt = dpad // 128
    order = np.argsort(dst, kind="stable")
    src = src[order]
    dst = dst[order]
    core_of = dst // dpc
    gsrc = _cm_rows(src, spc, spad, ncores)

    per_core = []  # [p][t] = (lo_gs, hi_gs, lo_dd, hi_dd)
    cnt_lo = np.zeros((ncores, nt), np.int64)
    cnt_hi = np.zeros((ncores, nt), np.int64)
    for p in range(ncores):
        sel = core_of == p
        sp = gsrc[sel]
        ld = dst[sel] - p * dpc
        tiles = []
        for t in range(nt):
            m = (ld // 128) == t
            st = sp[m]
            dd = (ld[m] - t * 128).astype(np.int64)
            lo = st < SPLIT
            tiles.append((st[lo], st[~lo] - SPLIT, dd[lo], dd[~lo]))
            cnt_lo[p, t] = lo.sum()
            cnt_hi[p, t] = (~lo).sum()
        per_core.append(tiles)

    nb_lo = [int(_ceil_to(max(cnt_lo[:, t].max(), 1), 128) // 128)
             for t in range(nt)]
    nb_hi = [int(_ceil_to(cnt_hi[:, t].max(), 128) // 128) for t in range(nt)]
    nb_tot = [nb_lo[t] + nb_hi[t] for t in range(nt)]
    off_nb = np.cumsum([0] + nb_tot).tolist()  # per-tile block offset
    tot_nb = off_nb[-1]

    idx_arrs = []
    seg_arrs = []
    for p in range(ncores):
        idx_a = np.zeros((128, tot_nb * 8), np.int16)
        seg_f = np.zeros((128, tot_nb * 128), np.float32)
        for t in range(nt):
            lo_gs, hi_gs, lo_dd, hi_dd = per_core[p][t]
            ob = off_nb[t]
            li = np.zeros(nb_lo[t] * 128, np.int64)
            li[: len(lo_gs)] = lo_gs
            idx_a[:, ob * 8: (ob + nb_lo[t]) * 8] = _wrap_idx(
                li.astype(np.int16))
            if nb_hi[t]:
                hi = np.zeros(nb_hi[t] * 128, np.int64)
                hi[: len(hi_gs)] = hi_gs
                idx_a[:, (ob + nb_lo[t]) * 8: (ob + nb_tot[t]) * 8] = \
                    _wrap_idx(hi.astype(np.int16))
            base = p * dpc + t * 128
            for boff, dd_list in ((0, lo_dd), (nb_lo[t], hi_dd)):
                i = np.arange(len(dd_list))
                b = ob + boff + i // 128
                e = i % 128
                seg_f[e, b * 128 + dd_list] = seg_scale[base + dd_list]
        idx_arrs.append(idx_a)
        seg_arrs.append(np.ascontiguousarray(seg_f.astype(seg_dtype)))
    return dict(nb_lo=nb_lo, nb_hi=nb_hi, off_nb=off_nb, tot_nb=tot_nb,
                idx=idx_arrs, seg=seg_arrs)


def host_prep(inputs, ncores=NCORES, n_bins=None):
    """Pure-numpy preprocessing: sharding, edge sorting, idx/ddsc
    construction, weight layout.  Only index arithmetic + data movement."""
    x = np.asarray(inputs["x"], np.float32)
    t_nodes, f_in = x.shape
    dim = np.asarray(inputs["W_i2"]).shape[0]
    ncls = np.asarray(inputs["b_fb"]).shape[0]
    if n_bins is None:
        if t_nodes == T and f_in == F_IN:
            n_bins = N_BINS
        else:
            n_bins = int(np.asarray(inputs["index_1"]).max()) + 1

    assert t_nodes % ncores == 0, (t_nodes, ncores)
    tpc = t_nodes // ncores
    tpad = _ceil_to(tpc, 128)
    nt = tpad // 128
    kin = _ceil_to(f_in, 128)
    assert n_bins % ncores == 0, (n_bins, ncores)
    bpc = n_bins // ncores
    bpad = _ceil_to(bpc, 128)
    bt = bpad // 128

    cfg = dict(
        t_nodes=t_nodes, f_in=f_in, dim=dim, ncls=ncls, n_bins=n_bins,
        ncores=ncores, tpc=tpc, tpad=tpad, nt=nt, kin=kin, kc=kin // 128,
        bpc=bpc, bpad=bpad, bt=bt, g_rows=ncores * tpad,
    )

    # ---- edge relations (with self-loops), degree norm
    rel = {}
    for r, key in ((1, "edge_index_1"), (2, "edge_index_2")):
        ei = np.asarray(inputs[key]).astype(np.int64)
        loop = np.arange(t_nodes, dtype=np.int64)
        s = np.concatenate([ei[0], loop])
        d = np.concatenate([ei[1], loop])
        deg = np.bincount(d, minlength=t_nodes).astype(np.float64)
        dinv = (1.0 / np.sqrt(np.maximum(deg, 1.0))).astype(np.float32)
        rel[r] = dict(
            prep=_prep_edges(s, d, tpc, tpad, tpc, tpad, ncores, dinv, F8),
            dinv=dinv,
        )
    cfg["rel"] = rel

    # ---- readout (scatter-mean): treat (node -> bin) as edges
    ro = {}
    for i, key in ((1, "index_1"), (2, "index_2")):
        idx = np.asarray(inputs[key]).astype(np.int64)
        cnt = np.bincount(idx, minlength=n_bins).astype(np.float64)
        invc = (1.0 / np.maximum(cnt, 1.0)).astype(np.float32)
        nodes = np.arange(t_nodes, dtype=np.int64)
        ro[i] = dict(
            prep=_prep_edges(nodes, idx, bpc, bpad, tpc, tpad, ncores, invc,
                             BF16),
        )
    cfg["ro"] = ro

    # ---- per-core x^T slices (bf16) in sub-chunked layout
    # [128, nsub, kc, SUBW]: partition = k%128, sub-chunk of SUBW node
    # columns, contiguous per (partition, sub) for a single fat DMA.
    SUBW = 256
    nsub = _ceil_to(tpad, SUBW) // SUBW
    cfg["subw"] = SUBW
    cfg["nsub"] = nsub
    kc = kin // 128
    xT = []
    for p in range(ncores):
        xs = np.zeros((kin, nsub * SUBW), np.float32)
        xs[:f_in, :tpc] = x[p * tpc: (p + 1) * tpc].T
        # [kc, 128, nsub, SUBW] -> [128, nsub, kc, SUBW]
        a = xs.reshape(kc, 128, nsub, SUBW).transpose(1, 2, 0, 3)
        xT.append(np.ascontiguousarray(a).astype(BF16))
    cfg["xT"] = xT

    # ---- dinv per-node tiles [128, nt] f32 per relation per core
    for r in (1, 2):
        dn = []
        dinv = rel[r]["dinv"]
        for p in range(ncores):
            a = np.zeros((128, nt), np.float32)
            vp = np.zeros(tpad, np.float32)
            vp[:tpc] = dinv[p * tpc: (p + 1) * tpc]
            a[:, :] = vp.reshape(nt, 128).T
            dn.append(a)
        rel[r]["dinv_n"] = dn

    # ---- weights
    w = {}
    w["wi1"] = _chunk_weight(np.asarray(inputs["W_i1"], np.float32))
    w["wi2"] = _chunk_weight(np.asarray(inputs["W_i2"], np.float32))
    for nm, src in (("wc11", "Wc11"), ("wc12", "Wc12"),
                    ("wc21", "Wc21"), ("wc22", "Wc22"),
                    ("wm1a", "W_m1a"), ("wm1b", "W_m1b"),
                    ("wm2a", "W_m2a"), ("wm2b", "W_m2b"),
                    ("wfa", "W_fa"), ("wfb", "W_fb")):
        w[nm] = _chunk_weight(np.asarray(inputs[src], np.float32))
    for nm, src in (("bi1", "b_i1"), ("bi2", "b_i2"),
                    ("bc11", "bc11"), ("bc12", "bc12"),
                    ("bc21", "bc21"), ("bc22", "bc22"),
                    ("bm1a", "b_m1a"), ("bm1b", "b_m1b"),
                    ("bm2a", "b_m2a"), ("bm2b", "b_m2b"),
                    ("bfa", "b_fa"), ("bfb", "b_fb")):
        w[nm] = _chunk_bias(np.asarray(inputs[src], np.float32))
    w["ident16"] = np.eye(128, dtype=BF16)
    w["ident32"] = np.eye(128, dtype=np.float32)
    cfg["w"] = w
    return cfg


def _nchunks(total, step, base=0):
    out = []
    o = 0
    while o < total:
        out.append((base + o, min(step, total - o)))
        o += step
    return out


def build_program(cfg):
    """Build the SPMD bass program (one program, 8 cores)."""
    import concourse.bass as bass
    import concourse.mybir as mybir
    import concourse.tile as tile
    from concourse import bacc

    dt = mybir.dt
    AF = mybir.ActivationFunctionType
    ALU = mybir.AluOpType

    nt, tpad, kc = cfg["nt"], cfg["tpad"], cfg["kc"]
    bt, bpad = cfg["bt"], cfg["bpad"]
    dim, ncls = cfg["dim"], cfg["ncls"]
    dc = dim // 128
    g_rows = cfg["g_rows"]
    ncores = cfg["ncores"]
    rel, ro = cfg["rel"], cfg["ro"]
    rg = [list(range(ncores))]
    SUBW, nsub = cfg["subw"], cfg["nsub"]

    cw = _chunk_widths(tpad)           # node-range chunk widths
    cum = np.cumsum([0] + cw).tolist()  # local row offsets
    ctiles = [range(cum[c] // 128, cum[c + 1] // 128) for c in range(NCHUNK)]

    nc = bacc.Bacc("TRN2", target_bir_lowering=False, debug=False,
                   num_devices=ncores, num_swdge_queues=4)
    qstate = [0]

    def next_q():
        q = qstate[0]
        qstate[0] = (q + 1) % 4
        return q

    # ---------------- I/O declarations ----------------
    xT = nc.dram_tensor("xT", [128, nsub, kc, SUBW], dt.bfloat16,
                        kind="ExternalInput")
    idx_in, seg_in, dinvn_in = {}, {}, {}
    for r in (1, 2):
        pr = rel[r]["prep"]
        idx_in[r] = nc.dram_tensor(f"idx{r}", [128, pr["tot_nb"] * 8],
                                   dt.int16, kind="ExternalInput")
        seg_in[r] = nc.dram_tensor(f"seg{r}", [128, pr["tot_nb"] * 128],
                                   dt.float8e4, kind="ExternalInput")
        dinvn_in[r] = nc.dram_tensor(f"dinvn{r}", [128, nt], dt.float32,
                                     kind="ExternalInput")
    idxr_in, segr_in = {}, {}
    for i in (1, 2):
        pr = ro[i]["prep"]
        idxr_in[i] = nc.dram_tensor(f"idxr{i}", [128, pr["tot_nb"] * 8],
                                    dt.int16, kind="ExternalInput")
        segr_in[i] = nc.dram_tensor(f"segr{i}", [128, pr["tot_nb"] * 128],
                                    dt.bfloat16, kind="ExternalInput")

    wnames_bf = dict(
        wi1=[128, kc, dim], wi2=[128, dc, dim],
        wc11=[128, dc, dim], wc12=[128, dc, dim],
        wc21=[128, dc, dim], wc22=[128, dc, dim],
        wm1a=[128, 2 * dc, dim], wm1b=[128, dc, dim],
        wm2a=[128, 2 * dc, dim], wm2b=[128, dc, dim],
        wfa=[128, 2 * dc, dim], wfb=[128, dc, ncls],
        ident16=[128, 128],
    )
    wnames_f32 = dict(
        bi1=[128, dc], bi2=[128, dc],
        bc11=[128, dc], bc12=[128, dc], bc21=[128, dc], bc22=[128, dc],
        bm1a=[128, dc], bm1b=[128, dc], bm2a=[128, dc], bm2b=[128, dc],
        bfa=[128, dc], bfb=[128, 1],
        ident32=[128, 128],
    )
    win = {}
    for nm, shp in wnames_bf.items():
        win[nm] = nc.dram_tensor(nm, shp, dt.bfloat16, kind="ExternalInput")
    for nm, shp in wnames_f32.items():
        win[nm] = nc.dram_tensor(nm, shp, dt.float32, kind="ExternalInput")

    out_dram = nc.dram_tensor("out", [bpad, ncls], dt.float32,
                              kind="ExternalOutput")

    nb_max = max(rel[r]["prep"]["nb_lo"][t] + rel[r]["prep"]["nb_hi"][t]
                 for r in (1, 2) for t in range(nt))
    nb_ro_max = max(ro[i]["prep"]["nb_lo"][t] + ro[i]["prep"]["nb_hi"][t]
                    for i in (1, 2) for t in range(bt))
    nb_max = max(nb_max, nb_ro_max)

    with tile.TileContext(nc) as tc:
        with (
            tc.tile_pool(name="wpool", bufs=1) as wpool,
            tc.tile_pool(name="hpool", bufs=1) as hpool,
            tc.tile_pool(name="xpool", bufs=2) as xpool,
            tc.tile_pool(name="rpool", bufs=4) as rpool,
            tc.tile_pool(name="edpool", bufs=4) as edpool,
            tc.tile_pool(name="segpool", bufs=4) as segpool,
            tc.tile_pool(name="idxpool", bufs=6) as idxpool,
            tc.tile_pool(name="apool", bufs=4) as apool,
            tc.tile_pool(name="gpool", bufs=3) as gpool,
            tc.tile_pool(name="mpool", bufs=4) as mpool,
            tc.tile_pool(name="spool", bufs=1) as spool,
            tc.tile_pool(name="pbig", bufs=3, space="PSUM") as pbig,
            tc.tile_pool(name="pagg", bufs=2, space="PSUM") as pagg,
            tc.tile_pool(name="pcnv", bufs=3, space="PSUM") as pcnv,
            tc.tile_pool(name="dpool", bufs=1, space="DRAM") as dpool,
        ):
            # ---- resident weights
            wsb = {}
            for nm in list(wnames_bf) + list(wnames_f32):
                shp = wnames_bf.get(nm) or wnames_f32[nm]
                dtyp = dt.bfloat16 if nm in wnames_bf else dt.float32
                wt = wpool.tile(shp, dtyp, name=f"sb_{nm}", tag=f"w_{nm}")
                nc.sync.dma_start(wt[:], win[nm][:])
                wsb[nm] = wt
            dinvn_sb = {}
            for r in (1, 2):
                dv = wpool.tile([128, nt], dt.float32, name=f"sb_dinvn{r}",
                                tag=f"w_dinvn{r}")
                nc.sync.dma_start(dv[:], dinvn_in[r][:])
                dinvn_sb[r] = dv

            def a_step(h_src, t, gt_dst):
                """Transpose h tile t to node-major and scale by dinv."""
                trp = []
                for f in range(dc):
                    tp = pcnv.tile([128, 128], dt.bfloat16, name="trp",
                                   tag="cnv")
                    nc.tensor.transpose(
                        tp[:], h_src[:, f, t * 128:(t + 1) * 128],
                        wsb["ident16"][:])
                    trp.append(tp)
                for r in (1, 2):
                    for f in range(dc):
                        nc.vector.tensor_scalar_mul(
                            gt_dst[:, (r - 1) * dim + f * 128:
                                   (r - 1) * dim + (f + 1) * 128],
                            trp[f][:], dinvn_sb[r][:, t:t + 1])

            def conv_tile(pr, g_full, wc, bc, hout, r, t):
                """One (relation, dst-tile) conv step: gather + SEG + W."""
                nbl, nbh = pr["nb_lo"][t], pr["nb_hi"][t]
                nb = nbl + nbh
                ob = pr["off_nb"][t]
                idxt = idxpool.tile([128, nb_max * 8], dt.int16,
                                    name="idxt", tag="idx")
                nc.sync.dma_start(idxt[:, :nb * 8],
                                  idx_in[r][:, ob * 8:(ob + nb) * 8])
                segt = segpool.tile([128, nb_max * 128], dt.float8e4,
                                    name="segt", tag="seg")
                nc.sync.dma_start(segt[:, :nb * 128],
                                  seg_in[r][:, ob * 128:(ob + nb) * 128])
                ed = edpool.tile([128, nb_max, dim], dt.float8e4,
                                 name="ed", tag="ed")
                nc.gpsimd.dma_gather(
                    ed[:, 0:nbl, :],
                    g_full[:, (r - 1) * dim:r * dim],
                    idxt[:, 0:nbl * 8],
                    nbl * 128, nbl * 128, dim,
                    elem_step=2 * dim, single_packet=False,
                    queue_num=next_q())
                if nbh:
                    nc.gpsimd.dma_gather(
                        ed[:, nbl:nb, :],
                        g_full[SPLIT:g_rows, (r - 1) * dim:r * dim],
                        idxt[:, nbl * 8:nb * 8],
                        nbh * 128, nbh * 128, dim,
                        elem_step=2 * dim, single_packet=False,
                        queue_num=next_q())
                agg = pagg.tile([128, dim], dt.float32, name="agg", tag="agg")
                for b in range(nb):
                    nc.tensor.matmul(
                        agg[:],
                        lhsT=segt[:, b * 128:(b + 1) * 128],
                        rhs=ed[:, b, :],
                        start=(b == 0), stop=(b == nb - 1))
                aggs = mpool.tile([128, dim], dt.bfloat16, name="aggs",
                                  tag="aggs")
                nc.vector.tensor_copy(aggs[:], agg[:])
                aggT = mpool.tile([128, dim], dt.bfloat16, name="aggT",
                                  tag="aggT")
                for f in range(dc):
                    tp = pcnv.tile([128, 128], dt.bfloat16, name="tpc",
                                   tag="cnv")
                    nc.tensor.transpose(
                        tp[:], aggs[:, f * 128:(f + 1) * 128],
                        wsb["ident16"][:])
                    nc.vector.tensor_copy(
                        aggT[:, f * 128:(f + 1) * 128], tp[:])
                cps_f = [pcnv.tile([128, 128], dt.float32,
                                   name=f"cps{f}", tag="cnv")
                         for f in range(dc)]
                for f2 in range(dc):
                    for k in range(dc):
                        nc.tensor.matmul(
                            cps_f[f2][:],
                            lhsT=wc[:, k, f2 * 128:(f2 + 1) * 128],
                            rhs=aggT[:, k * 128:(k + 1) * 128],
                            start=(k == 0), stop=(k == dc - 1))
                hstage = gpool.tile([128, dc, 128], dt.bfloat16,
                                    name="hstage", tag="hstage")
                for f2 in range(dc):
                    nc.scalar.activation(hstage[:, f2, :], cps_f[f2][:],
                                         AF.Relu, bias=bc[:, f2:f2 + 1])
                nc.sync.dma_start(hout[:, :, t * 128:(t + 1) * 128],
                                  hstage[:])

            # =========== Phase 1: input MLP  h0 = relu(x@Wi1+bi1)@Wi2+bi2
            # interleaved per AllGather chunk; AG1_c fires when chunk done.
            h_cur = hpool.tile([128, dc, tpad], dt.bfloat16, name="h0T",
                               tag="hT")
            g_loc1 = dpool.tile([tpad, 2 * dim], dt.float8e4, name="g_loc1",
                                tag="g_loc1")
            g_full1 = dpool.tile([g_rows, 2 * dim], dt.float8e4,
                                 name="g_full1", tag="g_full1")
            for c in range(NCHUNK):
                subs = [s for s in range(nsub)
                        if cum[c] <= s * SUBW < cum[c + 1]]
                for s in subs:
                    n0 = s * SUBW
                    nw = min(SUBW, tpad - n0)
                    xt = xpool.tile([128, kc, SUBW], dt.bfloat16, name="xt",
                                    tag="xt")
                    nc.sync.dma_start(xt[:], xT[:, s])
                    ps1 = []
                    for f in range(dc):
                        p_ = pbig.tile([128, 512], dt.float32, name="ps1",
                                       tag="mlp")
                        ps1.append(p_)
                        for k in range(kc):
                            nc.tensor.matmul(
                                p_[:, :nw],
                                lhsT=wsb["wi1"][:, k, f * 128:(f + 1) * 128],
                                rhs=xt[:, k, :nw],
                                start=(k == 0), stop=(k == kc - 1))
                    a1 = []
                    for f in range(dc):
                        a_ = apool.tile([128, 512], dt.bfloat16, name="a1",
                                        tag="a1")
                        nc.scalar.activation(a_[:, :nw], ps1[f][:, :nw],
                                             AF.Relu, bias=wsb["bi1"][:, f:f + 1])
                        a1.append(a_)
                    for f2 in range(dc):
                        p2 = pbig.tile([128, 512], dt.float32, name="ps2",
                                       tag="mlp")
                        for k2 in range(dc):
                            nc.tensor.matmul(
                                p2[:, :nw],
                                lhsT=wsb["wi2"][:, k2, f2 * 128:(f2 + 1) * 128],
                                rhs=a1[k2][:, :nw],
                                start=(k2 == 0), stop=(k2 == dc - 1))
                        nc.vector.tensor_scalar(
                            h_cur[:, f2, n0:n0 + nw], p2[:, :nw],
                            wsb["bi2"][:, f2:f2 + 1], None, ALU.add)
                for t in ctiles[c]:
                    gt = gpool.tile([128, 2 * dim], dt.float8e4, name="gt",
                                    tag="gt")
                    a_step(h_cur, t, gt)
                    nc.sync.dma_start(g_loc1[t * 128:(t + 1) * 128, :], gt[:])
                nc.gpsimd.collective_compute(
                    "AllGather", ALU.bypass, replica_groups=rg,
                    ins=[g_loc1[cum[c]:cum[c + 1], :]],
                    outs=[g_full1[ncores * cum[c]:ncores * cum[c + 1], :]])

            # =========== Phase 2: two GCN rounds
            g_fulls = {1: g_full1}
            hf_loc = dpool.tile([tpad, dim], dt.bfloat16, name="hf_loc",
                                tag="hf_loc")
            hf_full = dpool.tile([g_rows, dim], dt.bfloat16, name="hf_full",
                                 tag="hf_full")
            for rnd in (1, 2):
                g_full = g_fulls[rnd]
                wma = wsb[f"wm{rnd}a"]
                wmb = wsb[f"wm{rnd}b"]
                bma = wsb[f"bm{rnd}a"]
                bmb = wsb[f"bm{rnd}b"]
                houts = [dpool.tile([128, dc, tpad], dt.bfloat16,
                                    name=f"h{r}T", tag=f"h12_{rnd}{r}")
                         for r in (1, 2)]
                h_next = hpool.tile([128, dc, tpad], dt.bfloat16,
                                    name=f"hm{rnd}T", tag="hT")
                if rnd == 1:
                    g_loc2 = dpool.tile([tpad, 2 * dim], dt.float8e4,
                                        name="g_loc2", tag="g_loc2")
                    g_full2 = dpool.tile([g_rows, 2 * dim], dt.float8e4,
                                         name="g_full2", tag="g_full2")
                    g_fulls[2] = g_full2
                # software pipeline: chunk c's conv tiles, then chunk c's
                # MLP + a-step + AllGather (overlaps chunk c+1's conv)
                for c in range(NCHUNK):
                    for t in ctiles[c]:
                        for r in (1, 2):
                            conv_tile(rel[r]["prep"], g_full,
                                      wsb[f"wc{rnd}{r}"], wsb[f"bc{rnd}{r}"],
                                      houts[r - 1], r, t)
                    for (n0, nw) in _nchunks(cw[c], 512, base=cum[c]):
                        ps1 = []
                        for f in range(dc):
                            p_ = pbig.tile([128, 512], dt.float32, name="psm1",
                                           tag="mlp")
                            ps1.append(p_)
                        for k in range(2 * dc):
                            rhs_src = houts[0] if k < dc else houts[1]
                            rhs_t = rpool.tile([128, 512], dt.bfloat16,
                                               name="ht", tag="ht")
                            nc.sync.dma_start(rhs_t[:, :nw],
                                              rhs_src[:, k % dc, n0:n0 + nw])
                            for f in range(dc):
                                nc.tensor.matmul(
                                    ps1[f][:, :nw],
                                    lhsT=wma[:, k, f * 128:(f + 1) * 128],
                                    rhs=rhs_t[:, :nw],
                                    start=(k == 0), stop=(k == 2 * dc - 1))
                        am = []
                        for f in range(dc):
                            a_ = apool.tile([128, 512], dt.bfloat16, name="am",
                                            tag="a1")
                            nc.scalar.activation(a_[:, :nw], ps1[f][:, :nw],
                                                 AF.Relu, bias=bma[:, f:f + 1])
                            am.append(a_)
                        for f2 in range(dc):
                            p2 = pbig.tile([128, 512], dt.float32, name="psm2",
                                           tag="mlp")
                            for k2 in range(dc):
                                nc.tensor.matmul(
                                    p2[:, :nw],
                                    lhsT=wmb[:, k2, f2 * 128:(f2 + 1) * 128],
                                    rhs=am[k2][:, :nw],
                                    start=(k2 == 0), stop=(k2 == dc - 1))
                            nc.vector.tensor_scalar(
                                h_next[:, f2, n0:n0 + nw], p2[:, :nw],
                                bmb[:, f2:f2 + 1], None, ALU.add)
                    if rnd == 1:
                        for t in ctiles[c]:
                            gt = gpool.tile([128, 2 * dim], dt.float8e4,
                                            name="gt", tag="gt")
                            a_step(h_next, t, gt)
                            nc.sync.dma_start(
                                g_loc2[t * 128:(t + 1) * 128, :], gt[:])
                        nc.gpsimd.collective_compute(
                            "AllGather", ALU.bypass, replica_groups=rg,
                            ins=[g_loc2[cum[c]:cum[c + 1], :]],
                            outs=[g_full2[ncores * cum[c]:
                                          ncores * cum[c + 1], :]])
                    else:
                        # final h: transpose only (no dinv scaling)
                        for t in ctiles[c]:
                            gt = gpool.tile([128, 2 * dim], dt.bfloat16,
                                            name="gtf", tag="gt")
                            for f in range(dc):
                                tp = pcnv.tile([128, 128], dt.bfloat16,
                                               name="trpf", tag="cnv")
                                nc.tensor.transpose(
                                    tp[:], h_next[:, f, t * 128:(t + 1) * 128],
                                    wsb["ident16"][:])
                                nc.vector.tensor_copy(
                                    gt[:, f * 128:(f + 1) * 128], tp[:])
                            nc.sync.dma_start(
                                hf_loc[t * 128:(t + 1) * 128, :], gt[:, :dim])
                        nc.gpsimd.collective_compute(
                            "AllGather", ALU.bypass, replica_groups=rg,
                            ins=[hf_loc[cum[c]:cum[c + 1], :]],
                            outs=[hf_full[ncores * cum[c]:
                                          ncores * cum[c + 1], :]])
                h_cur = h_next

            # =========== Phase 3: readout (bin-sharded scatter-mean)
            rcat = spool.tile([128, 2 * dc, bpad], dt.bfloat16, name="rcat",
                              tag="rcat")
            for i in (1, 2):
                pr = ro[i]["prep"]
                for t in range(bt):
                    nbl, nbh = pr["nb_lo"][t], pr["nb_hi"][t]
                    nb = nbl + nbh
                    ob = pr["off_nb"][t]
                    idxt = idxpool.tile([128, nb_max * 8], dt.int16,
                                        name="idxtr", tag="idx")
                    nc.sync.dma_start(idxt[:, :nb * 8],
                                      idxr_in[i][:, ob * 8:(ob + nb) * 8])
                    segt = segpool.tile([128, nb_ro_max * 128], dt.bfloat16,
                                        name="segtr", tag="segr")
                    nc.sync.dma_start(segt[:, :nb * 128],
                                      segr_in[i][:, ob * 128:(ob + nb) * 128])
                    ed = edpool.tile([128, nb_max, dim], dt.bfloat16,
                                     name="edr", tag="ed")
                    nc.gpsimd.dma_gather(
                        ed[:, 0:nbl, :], hf_full[:],
                        idxt[:, 0:nbl * 8],
                        nbl * 128, nbl * 128, dim,
                        single_packet=False, queue_num=next_q())
                    if nbh:
                        nc.gpsimd.dma_gather(
                            ed[:, nbl:nb, :], hf_full[SPLIT:g_rows, :],
                            idxt[:, nbl * 8:nb * 8],
                            nbh * 128, nbh * 128, dim,
                            single_packet=False, queue_num=next_q())
                    agg = pagg.tile([128, dim], dt.float32, name="aggr",
                                    tag="agg")
                    for b in range(nb):
                        nc.tensor.matmul(
                            agg[:],
                            lhsT=segt[:, b * 128:(b + 1) * 128],
                            rhs=ed[:, b, :],
                            start=(b == 0), stop=(b == nb - 1))
                    aggs = mpool.tile([128, dim], dt.bfloat16, name="aggsr",
                                      tag="aggs")
                    nc.vector.tensor_copy(aggs[:], agg[:])
                    for f in range(dc):
                        tp = pcnv.tile([128, 128], dt.bfloat16,
                                       name="tpr", tag="cnv")
                        nc.tensor.transpose(
                            tp[:], aggs[:, f * 128:(f + 1) * 128],
                            wsb["ident16"][:])
                        nc.vector.tensor_copy(
                            rcat[:, (i - 1) * dc + f, t * 128:(t + 1) * 128],
                            tp[:])

            # ---- final MLP + log_softmax
            logitsT = spool.tile([128, bpad], dt.float32, name="logitsT",
                                 tag="logitsT")
            nc.vector.memset(logitsT[:], 0.0)
            for (n0, nw) in _nchunks(bpad, 512):
                ps1 = []
                for f in range(dc):
                    p_ = pbig.tile([128, 512], dt.float32, name="psf1",
                                   tag="mlp")
                    ps1.append(p_)
                for k in range(2 * dc):
                    for f in range(dc):
                        nc.tensor.matmul(
                            ps1[f][:, :nw],
                            lhsT=wsb["wfa"][:, k, f * 128:(f + 1) * 128],
                            rhs=rcat[:, k, n0:n0 + nw],
                            start=(k == 0), stop=(k == 2 * dc - 1))
                af = []
                for f in range(dc):
                    a_ = apool.tile([128, 512], dt.bfloat16, name="af",
                                    tag="a1")
                    nc.scalar.activation(a_[:, :nw], ps1[f][:, :nw], AF.Relu,
                                         bias=wsb["bfa"][:, f:f + 1])
                    af.append(a_)
                pl = pbig.tile([128, 512], dt.float32, name="psl", tag="mlp")
                for k2 in range(dc):
                    nc.tensor.matmul(
                        pl[:ncls, :nw],
                        lhsT=wsb["wfb"][:, k2, :ncls],
                        rhs=af[k2][:, :nw],
                        start=(k2 == 0), stop=(k2 == dc - 1))
                nc.vector.tensor_scalar(
                    logitsT[:ncls, n0:n0 + nw], pl[:ncls, :nw],
                    wsb["bfb"][:ncls, 0:1], None, ALU.add)

            for t in range(bt):
                ltp = pcnv.tile([128, 128], dt.float32, name="ltp", tag="cnv")
                nc.tensor.transpose(
                    ltp[:], logitsT[:, t * 128:(t + 1) * 128],
                    wsb["ident32"][:])
                mx = mpool.tile([128, 1], dt.float32, name="mx", tag="mx")
                nc.vector.tensor_reduce(mx[:], ltp[:, :ncls],
                                        mybir.AxisListType.X, ALU.max)
                z = mpool.tile([128, ncls], dt.float32, name="z", tag="z")
                nc.vector.tensor_scalar(z[:], ltp[:, :ncls], mx[:, 0:1], None,
                                        ALU.subtract)
                ez = mpool.tile([128, ncls], dt.float32, name="ez", tag="z")
                nc.scalar.activation(ez[:], z[:], AF.Exp)
                sm = mpool.tile([128, 1], dt.float32, name="sm", tag="mx")
                nc.vector.tensor_reduce(sm[:], ez[:], mybir.AxisListType.X,
                                        ALU.add)
                ls = mpool.tile([128, 1], dt.float32, name="ls", tag="mx")
                nc.scalar.activation(ls[:], sm[:], AF.Ln)
                o = mpool.tile([128, ncls], dt.float32, name="o", tag="z")
                nc.vector.tensor_scalar(o[:], z[:], ls[:, 0:1], None,
                                        ALU.subtract)
                nc.sync.dma_start(out_dram[t * 128:(t + 1) * 128, :], o[:])

    nc.compile()
    return nc


_CACHE = {}


def build_in_maps(cfg):
    in_maps = []
    for p in range(cfg["ncores"]):
        m = dict(
            xT=cfg["xT"][p],
            idx1=cfg["rel"][1]["prep"]["idx"][p],
            seg1=cfg["rel"][1]["prep"]["seg"][p],
            idx2=cfg["rel"][2]["prep"]["idx"][p],
            seg2=cfg["rel"][2]["prep"]["seg"][p],
            dinvn1=cfg["rel"][1]["dinv_n"][p],
            dinvn2=cfg["rel"][2]["dinv_n"][p],
            idxr1=cfg["ro"][1]["prep"]["idx"][p],
            segr1=cfg["ro"][1]["prep"]["seg"][p],
            idxr2=cfg["ro"][2]["prep"]["idx"][p],
            segr2=cfg["ro"][2]["prep"]["seg"][p],
        )
        m.update({k: v for k, v in cfg["w"].items()})
        in_maps.append(m)
    return in_maps


def kernel(**inputs) -> np.ndarray:
    cfg = host_prep(inputs)
    key = (
        cfg["t_nodes"], cfg["f_in"], cfg["dim"], cfg["ncls"], cfg["n_bins"],
        tuple(tuple(cfg["rel"][r]["prep"][k]) for r in (1, 2)
              for k in ("nb_lo", "nb_hi")),
        tuple(tuple(cfg["ro"][i]["prep"][k]) for i in (1, 2)
              for k in ("nb_lo", "nb_hi")),
    )
    if key not in _CACHE:
        _CACHE[key] = build_program(cfg)
    nc = _CACHE[key]

    from concourse.bass_utils import run_bass_kernel_spmd

    in_maps = build_in_maps(cfg)
    res = run_bass_kernel_spmd(nc, in_maps, list(range(cfg["ncores"])))
    outs = [res.results[p]["out"][: cfg["bpc"]] for p in range(cfg["ncores"])]
    return np.ascontiguousarray(np.concatenate(outs, axis=0), np.float32)


# revision 34
# speedup vs baseline: 1.8374x; 1.0853x over previous
"""Trainium2 Bass kernel for nn_Net_50620484551136 (gnn_message_passing).

Network (see problem reference):
  h  = MLP(x)                     # 4652 -> 256 -> 256
  h1 = relu(GCN(h, e1)); h2 = relu(GCN(h, e2))
  h  = MLP([h1, h2])              # 512 -> 256 -> 256
  h1 = relu(GCN(h, e1)); h2 = relu(GCN(h, e2))
  h  = MLP([h1, h2])
  r1 = scatter_mean(h, index_1, N); r2 = scatter_mean(h, index_2, N)
  out = log_softmax(MLP([r1, r2]))

Strategy (8 NeuronCores, SPMD single program):
  - Tuple nodes sharded contiguously across cores (6250/core, padded to 6272).
  - All dense matmuls run feature-major (h^T: [feat, node]) in bf16, fp32 PSUM.
  - GCN: matmul commutes with aggregation, so we aggregate g = h * dinv[src]
    (node-major, bf16) and apply the conv weight after.  Each round: write
    g1|g2 locally, AllGather (4 node-range chunks, overlapped with the MLP
    that produces them) to a full [50176, 512] chunk-major buffer, then each
    core gathers its incoming-edge rows (sorted by dst) with
    gpsimd.dma_gather and segment-sums them with PE matmuls against one-hot
    SEG blocks built ON DEVICE from compact (dd, scale) pairs via
    DVE iota==dd * scale (SEG carries dinv[dst]).
  - dma_gather indices are int16, so gathers are split into a low range
    (rows < 32768) and a high range; block counts are per-tile (max over the
    8 cores) so one static program serves all cores with minimal padding.
  - Scatter-mean readout: output bins sharded across cores (625/core, padded
    to 640); same gather+SEG machinery against the AllGathered final h, with
    1/count folded into the SEG scale.  Final MLP + log_softmax on device;
    host concatenates the 8 output shards.
"""

import numpy as np
import ml_dtypes

BF16 = ml_dtypes.bfloat16
F8 = ml_dtypes.float8_e4m3

# Problem constants (hardcoded per harness contract).
T = 50000
N_BINS = 5000
F_IN = 4652
DIM = 256
N_CLASSES = 5
NCORES = 8
SPLIT = 32768  # int16 gather index limit
NCHUNK = 4     # AllGather chunks per buffer


def _ceil_to(x, m):
    return (x + m - 1) // m * m


def _wrap_idx(v):
    """int16 index vector (len % 16 == 0) -> [128, len/16] wrapped layout."""
    assert len(v) % 16 == 0
    w = v.reshape(-1, 16).T.astype(np.int16)  # [16, len/16]
    return np.tile(w, (8, 1))  # [128, len/16]


def _chunk_weight(w, dtype=BF16):
    """[K, M] -> [128, ceil(K/128), M] (partition = k%128, block = k//128)."""
    k, m = w.shape
    kp = _ceil_to(k, 128)
    wp = np.zeros((kp, m), np.float32)
    wp[:k] = w
    return np.ascontiguousarray(
        wp.reshape(kp // 128, 128, m).transpose(1, 0, 2)
    ).astype(dtype)


def _chunk_bias(b):
    """[M] -> [128, ceil(M/128)] f32 (partition = m%128, col = m//128)."""
    m = len(b)
    mp = _ceil_to(m, 128)
    bp = np.zeros(mp, np.float32)
    bp[:m] = b
    return np.ascontiguousarray(bp.reshape(mp // 128, 128).T).astype(np.float32)


def _chunk_widths(pad):
    """Split `pad` (multiple of 128) into NCHUNK widths, each mult of 128."""
    ntile = pad // 128
    per = ntile // NCHUNK
    ws = [per * 128] * (NCHUNK - 1)
    ws.append(pad - sum(ws))
    return ws


def _cm_rows(src, spc, spad, ncores):
    """Chunk-major global row id for each source node (vectorized).

    Layout: for chunk c (widths from _chunk_widths(spad)), rows
    [ncores*cum[c], ncores*cum[c+1]) hold [rank0 rows, rank1 rows, ...].
    """
    ws = _chunk_widths(spad)
    cum = np.cumsum([0] + ws)  # [NCHUNK+1]
    p = src // spc
    l = src % spc
    c = np.minimum(np.searchsorted(cum, l, side="right") - 1, NCHUNK - 1)
    return ncores * cum[c] + p * np.array(ws)[c] + (l - cum[c])


def _prep_edges(src, dst, dpc, dpad, spc, spad, ncores, seg_scale,
                seg_dtype):
    """Per-core gather indices + host-built one-hot SEG blocks for one
    (src -> dst) relation.  dst space is sharded dpc-per-core (padded dpad);
    src rows live in a chunk-major AllGathered buffer (see _cm_rows).
    Aggregation output for dst d is sum over edges e with dst==d of
    seg_scale[d] * g[src_e].

    Per-tile block counts are variable (max over cores).  Returns dict with
    per-core idx/seg arrays plus global per-tile nb_lo/nb_hi lists.
    """
    nt = dpad // 128
    order = np.argsort(dst, kind="stable")
    src = src[order]
    dst = dst[order]
    core_of = dst // dpc
    gsrc = _cm_rows(src, spc, spad, ncores)

    per_core = []  # [p][t] = (lo_gs, hi_gs, lo_dd, hi_dd)
    cnt_lo = np.zeros((ncores, nt), np.int64)
    cnt_hi = np.zeros((ncores, nt), np.int64)
    for p in range(ncores):
        sel = core_of == p
        sp = gsrc[sel]
        ld = dst[sel] - p * dpc
        tiles = []
        for t in range(nt):
            m = (ld // 128) == t
            st = sp[m]
            dd = (ld[m] - t * 128).astype(np.int64)
            lo = st < SPLIT
            tiles.append((st[lo], st[~lo] - SPLIT, dd[lo], dd[~lo]))
            cnt_lo[p, t] = lo.sum()
            cnt_hi[p, t] = (~lo).sum()
        per_core.append(tiles)

    nb_lo = [int(_ceil_to(max(cnt_lo[:, t].max(), 1), 128) // 128)
             for t in range(nt)]
    nb_hi = [int(_ceil_to(cnt_hi[:, t].max(), 128) // 128) for t in range(nt)]
    nb_tot = [nb_lo[t] + nb_hi[t] for t in range(nt)]
    off_nb = np.cumsum([0] + nb_tot).tolist()  # per-tile block offset
    tot_nb = off_nb[-1]

    idx_arrs = []
    seg_arrs = []
    for p in range(ncores):
        idx_a = np.zeros((128, tot_nb * 8), np.int16)
        seg_f = np.zeros((128, tot_nb * 128), np.float32)
        for t in range(nt):
            lo_gs, hi_gs, lo_dd, hi_dd = per_core[p][t]
            ob = off_nb[t]
            li = np.zeros(nb_lo[t] * 128, np.int64)
            li[: len(lo_gs)] = lo_gs
            idx_a[:, ob * 8: (ob + nb_lo[t]) * 8] = _wrap_idx(
                li.astype(np.int16))
            if nb_hi[t]:
                hi = np.zeros(nb_hi[t] * 128, np.int64)
                hi[: len(hi_gs)] = hi_gs
                idx_a[:, (ob + nb_lo[t]) * 8: (ob + nb_tot[t]) * 8] = \
                    _wrap_idx(hi.astype(np.int16))
            base = p * dpc + t * 128
            for boff, dd_list in ((0, lo_dd), (nb_lo[t], hi_dd)):
                i = np.arange(len(dd_list))
                b = ob + boff + i // 128
                e = i % 128
                seg_f[e, b * 128 + dd_list] = seg_scale[base + dd_list]
        idx_arrs.append(idx_a)
        seg_arrs.append(np.ascontiguousarray(seg_f.astype(seg_dtype)))
    return dict(nb_lo=nb_lo, nb_hi=nb_hi, off_nb=off_nb, tot_nb=tot_nb,
                idx=idx_arrs, seg=seg_arrs)


def host_prep(inputs, ncores=NCORES, n_bins=None):
    """Pure-numpy preprocessing: sharding, edge sorting, idx/ddsc
    construction, weight layout.  Only index arithmetic + data movement."""
    x = np.asarray(inputs["x"], np.float32)
    t_nodes, f_in = x.shape
    dim = np.asarray(inputs["W_i2"]).shape[0]
    ncls = np.asarray(inputs["b_fb"]).shape[0]
    if n_bins is None:
        if t_nodes == T and f_in == F_IN:
            n_bins = N_BINS
        else:
            n_bins = int(np.asarray(inputs["index_1"]).max()) + 1

    assert t_nodes % ncores == 0, (t_nodes, ncores)
    tpc = t_nodes // ncores
    tpad = _ceil_to(tpc, 128)
    nt = tpad // 128
    kin = _ceil_to(f_in, 128)
    assert n_bins % ncores == 0, (n_bins, ncores)
    bpc = n_bins // ncores
    bpad = _ceil_to(bpc, 128)
    bt = bpad // 128

    cfg = dict(
        t_nodes=t_nodes, f_in=f_in, dim=dim, ncls=ncls, n_bins=n_bins,
        ncores=ncores, tpc=tpc, tpad=tpad, nt=nt, kin=kin, kc=kin // 128,
        bpc=bpc, bpad=bpad, bt=bt, g_rows=ncores * tpad,
    )

    # ---- edge relations (with self-loops), degree norm
    rel = {}
    for r, key in ((1, "edge_index_1"), (2, "edge_index_2")):
        ei = np.asarray(inputs[key]).astype(np.int64)
        loop = np.arange(t_nodes, dtype=np.int64)
        s = np.concatenate([ei[0], loop])
        d = np.concatenate([ei[1], loop])
        deg = np.bincount(d, minlength=t_nodes).astype(np.float64)
        dinv = (1.0 / np.sqrt(np.maximum(deg, 1.0))).astype(np.float32)
        rel[r] = dict(
            prep=_prep_edges(s, d, tpc, tpad, tpc, tpad, ncores, dinv, F8),
            dinv=dinv,
        )
    cfg["rel"] = rel

    # ---- readout (scatter-mean): treat (node -> bin) as edges
    ro = {}
    for i, key in ((1, "index_1"), (2, "index_2")):
        idx = np.asarray(inputs[key]).astype(np.int64)
        cnt = np.bincount(idx, minlength=n_bins).astype(np.float64)
        invc = (1.0 / np.maximum(cnt, 1.0)).astype(np.float32)
        nodes = np.arange(t_nodes, dtype=np.int64)
        ro[i] = dict(
            prep=_prep_edges(nodes, idx, bpc, bpad, tpc, tpad, ncores, invc,
                             BF16),
        )
    cfg["ro"] = ro

    # ---- per-core x^T slices (bf16) in sub-chunked layout
    # [128, nsub, kc, SUBW]: partition = k%128, sub-chunk of SUBW node
    # columns, contiguous per (partition, sub) for a single fat DMA.
    SUBW = 256
    nsub = _ceil_to(tpad, SUBW) // SUBW
    cfg["subw"] = SUBW
    cfg["nsub"] = nsub
    kc = kin // 128
    xT = []
    for p in range(ncores):
        xs = np.zeros((kin, nsub * SUBW), np.float32)
        xs[:f_in, :tpc] = x[p * tpc: (p + 1) * tpc].T
        # [kc, 128, nsub, SUBW] -> [128, nsub, kc, SUBW]
        a = xs.reshape(kc, 128, nsub, SUBW).transpose(1, 2, 0, 3)
        xT.append(np.ascontiguousarray(a).astype(F8))
    cfg["xT"] = xT

    # ---- dinv per-node tiles [128, nt] f32 per relation per core
    for r in (1, 2):
        dn = []
        dinv = rel[r]["dinv"]
        for p in range(ncores):
            a = np.zeros((128, nt), np.float32)
            vp = np.zeros(tpad, np.float32)
            vp[:tpc] = dinv[p * tpc: (p + 1) * tpc]
            a[:, :] = vp.reshape(nt, 128).T
            dn.append(a)
        rel[r]["dinv_n"] = dn

    # ---- weights
    w = {}
    w["wi1"] = _chunk_weight(np.asarray(inputs["W_i1"], np.float32), F8)
    w["wi2"] = _chunk_weight(np.asarray(inputs["W_i2"], np.float32))
    for nm, src in (("wc11", "Wc11"), ("wc12", "Wc12"),
                    ("wc21", "Wc21"), ("wc22", "Wc22"),
                    ("wm1a", "W_m1a"), ("wm1b", "W_m1b"),
                    ("wm2a", "W_m2a"), ("wm2b", "W_m2b"),
                    ("wfa", "W_fa"), ("wfb", "W_fb")):
        w[nm] = _chunk_weight(np.asarray(inputs[src], np.float32))
    for nm, src in (("bi1", "b_i1"), ("bi2", "b_i2"),
                    ("bc11", "bc11"), ("bc12", "bc12"),
                    ("bc21", "bc21"), ("bc22", "bc22"),
                    ("bm1a", "b_m1a"), ("bm1b", "b_m1b"),
                    ("bm2a", "b_m2a"), ("bm2b", "b_m2b"),
                    ("bfa", "b_fa"), ("bfb", "b_fb")):
        w[nm] = _chunk_bias(np.asarray(inputs[src], np.float32))
    w["ident16"] = np.eye(128, dtype=BF16)
    w["ident32"] = np.eye(128, dtype=np.float32)
    cfg["w"] = w
    return cfg


def _nchunks(total, step, base=0):
    out = []
    o = 0
    while o < total:
        out.append((base + o, min(step, total - o)))
        o += step
    return out


def build_program(cfg):
    """Build the SPMD bass program (one program, 8 cores)."""
    import concourse.bass as bass
    import concourse.mybir as mybir
    import concourse.tile as tile
    from concourse import bacc

    dt = mybir.dt
    AF = mybir.ActivationFunctionType
    ALU = mybir.AluOpType

    nt, tpad, kc = cfg["nt"], cfg["tpad"], cfg["kc"]
    bt, bpad = cfg["bt"], cfg["bpad"]
    dim, ncls = cfg["dim"], cfg["ncls"]
    dc = dim // 128
    g_rows = cfg["g_rows"]
    ncores = cfg["ncores"]
    rel, ro = cfg["rel"], cfg["ro"]
    rg = [list(range(ncores))]
    SUBW, nsub = cfg["subw"], cfg["nsub"]

    cw = _chunk_widths(tpad)           # node-range chunk widths
    cum = np.cumsum([0] + cw).tolist()  # local row offsets
    ctiles = [range(cum[c] // 128, cum[c + 1] // 128) for c in range(NCHUNK)]

    nc = bacc.Bacc("TRN2", target_bir_lowering=False, debug=False,
                   num_devices=ncores, num_swdge_queues=4)
    qstate = [0]

    def next_q():
        q = qstate[0]
        qstate[0] = (q + 1) % 4
        return q

    # ---------------- I/O declarations ----------------
    xT = nc.dram_tensor("xT", [128, nsub, kc, SUBW], dt.float8e4,
                        kind="ExternalInput")
    idx_in, seg_in, dinvn_in = {}, {}, {}
    for r in (1, 2):
        pr = rel[r]["prep"]
        idx_in[r] = nc.dram_tensor(f"idx{r}", [128, pr["tot_nb"] * 8],
                                   dt.int16, kind="ExternalInput")
        seg_in[r] = nc.dram_tensor(f"seg{r}", [128, pr["tot_nb"] * 128],
                                   dt.float8e4, kind="ExternalInput")
        dinvn_in[r] = nc.dram_tensor(f"dinvn{r}", [128, nt], dt.float32,
                                     kind="ExternalInput")
    idxr_in, segr_in = {}, {}
    for i in (1, 2):
        pr = ro[i]["prep"]
        idxr_in[i] = nc.dram_tensor(f"idxr{i}", [128, pr["tot_nb"] * 8],
                                    dt.int16, kind="ExternalInput")
        segr_in[i] = nc.dram_tensor(f"segr{i}", [128, pr["tot_nb"] * 128],
                                    dt.bfloat16, kind="ExternalInput")

    wnames_f8 = dict(wi1=[128, kc, dim])
    wnames_bf = dict(
        wi2=[128, dc, dim],
        wc11=[128, dc, dim], wc12=[128, dc, dim],
        wc21=[128, dc, dim], wc22=[128, dc, dim],
        wm1a=[128, 2 * dc, dim], wm1b=[128, dc, dim],
        wm2a=[128, 2 * dc, dim], wm2b=[128, dc, dim],
        wfa=[128, 2 * dc, dim], wfb=[128, dc, ncls],
        ident16=[128, 128],
    )
    wnames_f32 = dict(
        bi1=[128, dc], bi2=[128, dc],
        bc11=[128, dc], bc12=[128, dc], bc21=[128, dc], bc22=[128, dc],
        bm1a=[128, dc], bm1b=[128, dc], bm2a=[128, dc], bm2b=[128, dc],
        bfa=[128, dc], bfb=[128, 1],
        ident32=[128, 128],
    )
    win = {}
    for nm, shp in wnames_f8.items():
        win[nm] = nc.dram_tensor(nm, shp, dt.float8e4, kind="ExternalInput")
    for nm, shp in wnames_bf.items():
        win[nm] = nc.dram_tensor(nm, shp, dt.bfloat16, kind="ExternalInput")
    for nm, shp in wnames_f32.items():
        win[nm] = nc.dram_tensor(nm, shp, dt.float32, kind="ExternalInput")

    out_dram = nc.dram_tensor("out", [bpad, ncls], dt.float32,
                              kind="ExternalOutput")

    nb_max = max(rel[r]["prep"]["nb_lo"][t] + rel[r]["prep"]["nb_hi"][t]
                 for r in (1, 2) for t in range(nt))
    nb_ro_max = max(ro[i]["prep"]["nb_lo"][t] + ro[i]["prep"]["nb_hi"][t]
                    for i in (1, 2) for t in range(bt))
    nb_max = max(nb_max, nb_ro_max)

    with tile.TileContext(nc) as tc:
        with (
            tc.tile_pool(name="wpool", bufs=1) as wpool,
            tc.tile_pool(name="hpool", bufs=1) as hpool,
            tc.tile_pool(name="xpool", bufs=2) as xpool,
            tc.tile_pool(name="rpool", bufs=4) as rpool,
            tc.tile_pool(name="edpool", bufs=6) as edpool,
            tc.tile_pool(name="segpool", bufs=4) as segpool,
            tc.tile_pool(name="idxpool", bufs=6) as idxpool,
            tc.tile_pool(name="apool", bufs=4) as apool,
            tc.tile_pool(name="gpool", bufs=3) as gpool,
            tc.tile_pool(name="mpool", bufs=4) as mpool,
            tc.tile_pool(name="spool", bufs=1) as spool,
            tc.tile_pool(name="pbig", bufs=3, space="PSUM") as pbig,
            tc.tile_pool(name="pagg", bufs=2, space="PSUM") as pagg,
            tc.tile_pool(name="pcnv", bufs=3, space="PSUM") as pcnv,
            tc.tile_pool(name="dpool", bufs=1, space="DRAM") as dpool,
        ):
            # ---- resident weights
            wsb = {}
            for nm in list(wnames_f8) + list(wnames_bf) + list(wnames_f32):
                shp = (wnames_f8.get(nm) or wnames_bf.get(nm)
                       or wnames_f32[nm])
                dtyp = (dt.float8e4 if nm in wnames_f8 else
                        dt.bfloat16 if nm in wnames_bf else dt.float32)
                wt = wpool.tile(shp, dtyp, name=f"sb_{nm}", tag=f"w_{nm}")
                nc.sync.dma_start(wt[:], win[nm][:])
                wsb[nm] = wt
            dinvn_sb = {}
            for r in (1, 2):
                dv = wpool.tile([128, nt], dt.float32, name=f"sb_dinvn{r}",
                                tag=f"w_dinvn{r}")
                nc.sync.dma_start(dv[:], dinvn_in[r][:])
                dinvn_sb[r] = dv

            def a_step(h_src, t, gt_dst):
                """Transpose h tile t to node-major and scale by dinv."""
                trp = []
                for f in range(dc):
                    tp = pcnv.tile([128, 128], dt.bfloat16, name="trp",
                                   tag="cnv")
                    nc.tensor.transpose(
                        tp[:], h_src[:, f, t * 128:(t + 1) * 128],
                        wsb["ident16"][:])
                    trp.append(tp)
                for r in (1, 2):
                    for f in range(dc):
                        nc.vector.tensor_scalar_mul(
                            gt_dst[:, (r - 1) * dim + f * 128:
                                   (r - 1) * dim + (f + 1) * 128],
                            trp[f][:], dinvn_sb[r][:, t:t + 1])

            def conv_tile(pr, g_full, wc, bc, hout, r, t):
                """One (relation, dst-tile) conv step: gather + SEG + W."""
                nbl, nbh = pr["nb_lo"][t], pr["nb_hi"][t]
                nb = nbl + nbh
                ob = pr["off_nb"][t]
                idxt = idxpool.tile([128, nb_max * 8], dt.int16,
                                    name="idxt", tag="idx")
                nc.sync.dma_start(idxt[:, :nb * 8],
                                  idx_in[r][:, ob * 8:(ob + nb) * 8])
                segt = segpool.tile([128, nb_max, 128], dt.float8e4,
                                    name="segt", tag="seg")
                nc.sync.dma_start(segt[:, :nb, :],
                                  seg_in[r][:, ob * 128:(ob + nb) * 128])
                ed = edpool.tile([128, nb_max, dim], dt.float8e4,
                                 name="ed", tag="ed")
                nc.gpsimd.dma_gather(
                    ed[:, 0:nbl, :],
                    g_full[:, (r - 1) * dim:r * dim],
                    idxt[:, 0:nbl * 8],
                    nbl * 128, nbl * 128, dim,
                    elem_step=2 * dim, single_packet=False,
                    queue_num=next_q())
                if nbh:
                    nc.gpsimd.dma_gather(
                        ed[:, nbl:nb, :],
                        g_full[SPLIT:g_rows, (r - 1) * dim:r * dim],
                        idxt[:, nbl * 8:nb * 8],
                        nbh * 128, nbh * 128, dim,
                        elem_step=2 * dim, single_packet=False,
                        queue_num=next_q())
                agg = pagg.tile([128, dim], dt.float32, name="agg", tag="agg")
                npair = nb // 2
                for p_ in range(npair):
                    b = 2 * p_
                    nc.tensor.matmul(
                        agg[:],
                        lhsT=segt[:, b:b + 2, :],
                        rhs=ed[:, b:b + 2, :],
                        start=(b == 0), stop=(b + 2 == nb),
                        perf_mode=mybir.MatmulPerfMode.DoubleRow)
                if nb % 2:
                    nc.tensor.matmul(
                        agg[:],
                        lhsT=segt[:, nb - 1, :],
                        rhs=ed[:, nb - 1, :],
                        start=(nb == 1), stop=True)
                aggs = mpool.tile([128, dim], dt.bfloat16, name="aggs",
                                  tag="aggs")
                nc.vector.tensor_copy(aggs[:], agg[:])
                aggT = mpool.tile([128, dim], dt.bfloat16, name="aggT",
                                  tag="aggT")
                for f in range(dc):
                    tp = pcnv.tile([128, 128], dt.bfloat16, name="tpc",
                                   tag="cnv")
                    nc.tensor.transpose(
                        tp[:], aggs[:, f * 128:(f + 1) * 128],
                        wsb["ident16"][:])
                    nc.vector.tensor_copy(
                        aggT[:, f * 128:(f + 1) * 128], tp[:])
                cps_f = [pcnv.tile([128, 128], dt.float32,
                                   name=f"cps{f}", tag="cnv")
                         for f in range(dc)]
                for f2 in range(dc):
                    for k in range(dc):
                        nc.tensor.matmul(
                            cps_f[f2][:],
                            lhsT=wc[:, k, f2 * 128:(f2 + 1) * 128],
                            rhs=aggT[:, k * 128:(k + 1) * 128],
                            start=(k == 0), stop=(k == dc - 1))
                hstage = gpool.tile([128, dc, 128], dt.bfloat16,
                                    name="hstage", tag="hstage")
                for f2 in range(dc):
                    nc.scalar.activation(hstage[:, f2, :], cps_f[f2][:],
                                         AF.Relu, bias=bc[:, f2:f2 + 1])
                nc.sync.dma_start(hout[:, :, t * 128:(t + 1) * 128],
                                  hstage[:])

            # =========== Phase 1: input MLP  h0 = relu(x@Wi1+bi1)@Wi2+bi2
            # interleaved per AllGather chunk; AG1_c fires when chunk done.
            h_cur = hpool.tile([128, dc, tpad], dt.bfloat16, name="h0T",
                               tag="hT")
            g_loc1 = dpool.tile([tpad, 2 * dim], dt.float8e4, name="g_loc1",
                                tag="g_loc1")
            g_full1 = dpool.tile([g_rows, 2 * dim], dt.float8e4,
                                 name="g_full1", tag="g_full1")
            for c in range(NCHUNK):
                subs = [s for s in range(nsub)
                        if cum[c] <= s * SUBW < cum[c + 1]]
                for s in subs:
                    n0 = s * SUBW
                    nw = min(SUBW, tpad - n0)
                    xt = xpool.tile([128, kc, SUBW], dt.float8e4, name="xt",
                                    tag="xt")
                    nc.sync.dma_start(xt[:], xT[:, s])
                    ps1 = []
                    for f in range(dc):
                        p_ = pbig.tile([128, 512], dt.float32, name="ps1",
                                       tag="mlp")
                        ps1.append(p_)
                        for k in range(0, kc - 1, 2):
                            nc.tensor.matmul(
                                p_[:, :nw],
                                lhsT=wsb["wi1"][:, k:k + 2,
                                                f * 128:(f + 1) * 128],
                                rhs=xt[:, k:k + 2, :nw],
                                start=(k == 0), stop=(k + 2 == kc),
                                perf_mode=mybir.MatmulPerfMode.DoubleRow)
                        if kc % 2:
                            nc.tensor.matmul(
                                p_[:, :nw],
                                lhsT=wsb["wi1"][:, kc - 1,
                                                f * 128:(f + 1) * 128],
                                rhs=xt[:, kc - 1, :nw],
                                start=(kc == 1), stop=True)
                    a1 = []
                    for f in range(dc):
                        a_ = apool.tile([128, 512], dt.bfloat16, name="a1",
                                        tag="a1")
                        nc.scalar.activation(a_[:, :nw], ps1[f][:, :nw],
                                             AF.Relu, bias=wsb["bi1"][:, f:f + 1])
                        a1.append(a_)
                    for f2 in range(dc):
                        p2 = pbig.tile([128, 512], dt.float32, name="ps2",
                                       tag="mlp")
                        for k2 in range(dc):
                            nc.tensor.matmul(
                                p2[:, :nw],
                                lhsT=wsb["wi2"][:, k2, f2 * 128:(f2 + 1) * 128],
                                rhs=a1[k2][:, :nw],
                                start=(k2 == 0), stop=(k2 == dc - 1))
                        nc.vector.tensor_scalar(
                            h_cur[:, f2, n0:n0 + nw], p2[:, :nw],
                            wsb["bi2"][:, f2:f2 + 1], None, ALU.add)
                for t in ctiles[c]:
                    gt = gpool.tile([128, 2 * dim], dt.float8e4, name="gt",
                                    tag="gt")
                    a_step(h_cur, t, gt)
                    nc.sync.dma_start(g_loc1[t * 128:(t + 1) * 128, :], gt[:])
                nc.gpsimd.collective_compute(
                    "AllGather", ALU.bypass, replica_groups=rg,
                    ins=[g_loc1[cum[c]:cum[c + 1], :]],
                    outs=[g_full1[ncores * cum[c]:ncores * cum[c + 1], :]])

            # =========== Phase 2: two GCN rounds
            g_fulls = {1: g_full1}
            hf_loc = dpool.tile([tpad, dim], dt.bfloat16, name="hf_loc",
                                tag="hf_loc")
            hf_full = dpool.tile([g_rows, dim], dt.bfloat16, name="hf_full",
                                 tag="hf_full")
            for rnd in (1, 2):
                g_full = g_fulls[rnd]
                wma = wsb[f"wm{rnd}a"]
                wmb = wsb[f"wm{rnd}b"]
                bma = wsb[f"bm{rnd}a"]
                bmb = wsb[f"bm{rnd}b"]
                houts = [dpool.tile([128, dc, tpad], dt.bfloat16,
                                    name=f"h{r}T", tag=f"h12_{rnd}{r}")
                         for r in (1, 2)]
                h_next = hpool.tile([128, dc, tpad], dt.bfloat16,
                                    name=f"hm{rnd}T", tag="hT")
                if rnd == 1:
                    g_loc2 = dpool.tile([tpad, 2 * dim], dt.float8e4,
                                        name="g_loc2", tag="g_loc2")
                    g_full2 = dpool.tile([g_rows, 2 * dim], dt.float8e4,
                                         name="g_full2", tag="g_full2")
                    g_fulls[2] = g_full2
                # software pipeline: chunk c's conv tiles, then chunk c's
                # MLP + a-step + AllGather (overlaps chunk c+1's conv)
                for c in range(NCHUNK):
                    for t in ctiles[c]:
                        for r in (1, 2):
                            conv_tile(rel[r]["prep"], g_full,
                                      wsb[f"wc{rnd}{r}"], wsb[f"bc{rnd}{r}"],
                                      houts[r - 1], r, t)
                    for (n0, nw) in _nchunks(cw[c], 512, base=cum[c]):
                        ps1 = []
                        for f in range(dc):
                            p_ = pbig.tile([128, 512], dt.float32, name="psm1",
                                           tag="mlp")
                            ps1.append(p_)
                        for k in range(2 * dc):
                            rhs_src = houts[0] if k < dc else houts[1]
                            rhs_t = rpool.tile([128, 512], dt.bfloat16,
                                               name="ht", tag="ht")
                            nc.sync.dma_start(rhs_t[:, :nw],
                                              rhs_src[:, k % dc, n0:n0 + nw])
                            for f in range(dc):
                                nc.tensor.matmul(
                                    ps1[f][:, :nw],
                                    lhsT=wma[:, k, f * 128:(f + 1) * 128],
                                    rhs=rhs_t[:, :nw],
                                    start=(k == 0), stop=(k == 2 * dc - 1))
                        am = []
                        for f in range(dc):
                            a_ = apool.tile([128, 512], dt.bfloat16, name="am",
                                            tag="a1")
                            nc.scalar.activation(a_[:, :nw], ps1[f][:, :nw],
                                                 AF.Relu, bias=bma[:, f:f + 1])
                            am.append(a_)
                        for f2 in range(dc):
                            p2 = pbig.tile([128, 512], dt.float32, name="psm2",
                                           tag="mlp")
                            for k2 in range(dc):
                                nc.tensor.matmul(
                                    p2[:, :nw],
                                    lhsT=wmb[:, k2, f2 * 128:(f2 + 1) * 128],
                                    rhs=am[k2][:, :nw],
                                    start=(k2 == 0), stop=(k2 == dc - 1))
                            nc.vector.tensor_scalar(
                                h_next[:, f2, n0:n0 + nw], p2[:, :nw],
                                bmb[:, f2:f2 + 1], None, ALU.add)
                    if rnd == 1:
                        for t in ctiles[c]:
                            gt = gpool.tile([128, 2 * dim], dt.float8e4,
                                            name="gt", tag="gt")
                            a_step(h_next, t, gt)
                            nc.sync.dma_start(
                                g_loc2[t * 128:(t + 1) * 128, :], gt[:])
                        nc.gpsimd.collective_compute(
                            "AllGather", ALU.bypass, replica_groups=rg,
                            ins=[g_loc2[cum[c]:cum[c + 1], :]],
                            outs=[g_full2[ncores * cum[c]:
                                          ncores * cum[c + 1], :]])
                    else:
                        # final h: transpose only (no dinv scaling)
                        for t in ctiles[c]:
                            gt = gpool.tile([128, 2 * dim], dt.bfloat16,
                                            name="gtf", tag="gt")
                            for f in range(dc):
                                tp = pcnv.tile([128, 128], dt.bfloat16,
                                               name="trpf", tag="cnv")
                                nc.tensor.transpose(
                                    tp[:], h_next[:, f, t * 128:(t + 1) * 128],
                                    wsb["ident16"][:])
                                nc.vector.tensor_copy(
                                    gt[:, f * 128:(f + 1) * 128], tp[:])
                            nc.sync.dma_start(
                                hf_loc[t * 128:(t + 1) * 128, :], gt[:, :dim])
                        nc.gpsimd.collective_compute(
                            "AllGather", ALU.bypass, replica_groups=rg,
                            ins=[hf_loc[cum[c]:cum[c + 1], :]],
                            outs=[hf_full[ncores * cum[c]:
                                          ncores * cum[c + 1], :]])
                h_cur = h_next

            # =========== Phase 3: readout (bin-sharded scatter-mean)
            rcat = spool.tile([128, 2 * dc, bpad], dt.bfloat16, name="rcat",
                              tag="rcat")
            for i in (1, 2):
                pr = ro[i]["prep"]
                for t in range(bt):
                    nbl, nbh = pr["nb_lo"][t], pr["nb_hi"][t]
                    nb = nbl + nbh
                    ob = pr["off_nb"][t]
                    idxt = idxpool.tile([128, nb_max * 8], dt.int16,
                                        name="idxtr", tag="idx")
                    nc.sync.dma_start(idxt[:, :nb * 8],
                                      idxr_in[i][:, ob * 8:(ob + nb) * 8])
                    segt = segpool.tile([128, nb_ro_max * 128], dt.bfloat16,
                                        name="segtr", tag="segr")
                    nc.sync.dma_start(segt[:, :nb * 128],
                                      segr_in[i][:, ob * 128:(ob + nb) * 128])
                    ed = edpool.tile([128, nb_max, dim], dt.bfloat16,
                                     name="edr", tag="ed")
                    nc.gpsimd.dma_gather(
                        ed[:, 0:nbl, :], hf_full[:],
                        idxt[:, 0:nbl * 8],
                        nbl * 128, nbl * 128, dim,
                        single_packet=False, queue_num=next_q())
                    if nbh:
                        nc.gpsimd.dma_gather(
                            ed[:, nbl:nb, :], hf_full[SPLIT:g_rows, :],
                            idxt[:, nbl * 8:nb * 8],
                            nbh * 128, nbh * 128, dim,
                            single_packet=False, queue_num=next_q())
                    agg = pagg.tile([128, dim], dt.float32, name="aggr",
                                    tag="agg")
                    for b in range(nb):
                        nc.tensor.matmul(
                            agg[:],
                            lhsT=segt[:, b * 128:(b + 1) * 128],
                            rhs=ed[:, b, :],
                            start=(b == 0), stop=(b == nb - 1))
                    aggs = mpool.tile([128, dim], dt.bfloat16, name="aggsr",
                                      tag="aggs")
                    nc.vector.tensor_copy(aggs[:], agg[:])
                    for f in range(dc):
                        tp = pcnv.tile([128, 128], dt.bfloat16,
                                       name="tpr", tag="cnv")
                        nc.tensor.transpose(
                            tp[:], aggs[:, f * 128:(f + 1) * 128],
                            wsb["ident16"][:])
                        nc.vector.tensor_copy(
                            rcat[:, (i - 1) * dc + f, t * 128:(t + 1) * 128],
                            tp[:])

            # ---- final MLP + log_softmax
            logitsT = spool.tile([128, bpad], dt.float32, name="logitsT",
                                 tag="logitsT")
            nc.vector.memset(logitsT[:], 0.0)
            for (n0, nw) in _nchunks(bpad, 512):
                ps1 = []
                for f in range(dc):
                    p_ = pbig.tile([128, 512], dt.float32, name="psf1",
                                   tag="mlp")
                    ps1.append(p_)
                for k in range(2 * dc):
                    for f in range(dc):
                        nc.tensor.matmul(
                            ps1[f][:, :nw],
                            lhsT=wsb["wfa"][:, k, f * 128:(f + 1) * 128],
                            rhs=rcat[:, k, n0:n0 + nw],
                            start=(k == 0), stop=(k == 2 * dc - 1))
                af = []
                for f in range(dc):
                    a_ = apool.tile([128, 512], dt.bfloat16, name="af",
                                    tag="a1")
                    nc.scalar.activation(a_[:, :nw], ps1[f][:, :nw], AF.Relu,
                                         bias=wsb["bfa"][:, f:f + 1])
                    af.append(a_)
                pl = pbig.tile([128, 512], dt.float32, name="psl", tag="mlp")
                for k2 in range(dc):
                    nc.tensor.matmul(
                        pl[:ncls, :nw],
                        lhsT=wsb["wfb"][:, k2, :ncls],
                        rhs=af[k2][:, :nw],
                        start=(k2 == 0), stop=(k2 == dc - 1))
                nc.vector.tensor_scalar(
                    logitsT[:ncls, n0:n0 + nw], pl[:ncls, :nw],
                    wsb["bfb"][:ncls, 0:1], None, ALU.add)

            for t in range(bt):
                ltp = pcnv.tile([128, 128], dt.float32, name="ltp", tag="cnv")
                nc.tensor.transpose(
                    ltp[:], logitsT[:, t * 128:(t + 1) * 128],
                    wsb["ident32"][:])
                mx = mpool.tile([128, 1], dt.float32, name="mx", tag="mx")
                nc.vector.tensor_reduce(mx[:], ltp[:, :ncls],
                                        mybir.AxisListType.X, ALU.max)
                z = mpool.tile([128, ncls], dt.float32, name="z", tag="z")
                nc.vector.tensor_scalar(z[:], ltp[:, :ncls], mx[:, 0:1], None,
                                        ALU.subtract)
                ez = mpool.tile([128, ncls], dt.float32, name="ez", tag="z")
                nc.scalar.activation(ez[:], z[:], AF.Exp)
                sm = mpool.tile([128, 1], dt.float32, name="sm", tag="mx")
                nc.vector.tensor_reduce(sm[:], ez[:], mybir.AxisListType.X,
                                        ALU.add)
                ls = mpool.tile([128, 1], dt.float32, name="ls", tag="mx")
                nc.scalar.activation(ls[:], sm[:], AF.Ln)
                o = mpool.tile([128, ncls], dt.float32, name="o", tag="z")
                nc.vector.tensor_scalar(o[:], z[:], ls[:, 0:1], None,
                                        ALU.subtract)
                nc.sync.dma_start(out_dram[t * 128:(t + 1) * 128, :], o[:])

    nc.compile()
    return nc


_CACHE = {}


def build_in_maps(cfg):
    in_maps = []
    for p in range(cfg["ncores"]):
        m = dict(
            xT=cfg["xT"][p],
            idx1=cfg["rel"][1]["prep"]["idx"][p],
            seg1=cfg["rel"][1]["prep"]["seg"][p],
            idx2=cfg["rel"][2]["prep"]["idx"][p],
            seg2=cfg["rel"][2]["prep"]["seg"][p],
            dinvn1=cfg["rel"][1]["dinv_n"][p],
            dinvn2=cfg["rel"][2]["dinv_n"][p],
            idxr1=cfg["ro"][1]["prep"]["idx"][p],
            segr1=cfg["ro"][1]["prep"]["seg"][p],
            idxr2=cfg["ro"][2]["prep"]["idx"][p],
            segr2=cfg["ro"][2]["prep"]["seg"][p],
        )
        m.update({k: v for k, v in cfg["w"].items()})
        in_maps.append(m)
    return in_maps


def kernel(**inputs) -> np.ndarray:
    cfg = host_prep(inputs)
    key = (
        cfg["t_nodes"], cfg["f_in"], cfg["dim"], cfg["ncls"], cfg["n_bins"],
        tuple(tuple(cfg["rel"][r]["prep"][k]) for r in (1, 2)
              for k in ("nb_lo", "nb_hi")),
        tuple(tuple(cfg["ro"][i]["prep"][k]) for i in (1, 2)
              for k in ("nb_lo", "nb_hi")),
    )
    if key not in _CACHE:
        _CACHE[key] = build_program(cfg)
    nc = _CACHE[key]

    from concourse.bass_utils import run_bass_kernel_spmd

    in_maps = build_in_maps(cfg)
    res = run_bass_kernel_spmd(nc, in_maps, list(range(cfg["ncores"])))
    outs = [res.results[p]["out"][: cfg["bpc"]] for p in range(cfg["ncores"])]
    return np.ascontiguousarray(np.concatenate(outs, axis=0), np.float32)


# revision 40
# speedup vs baseline: 1.8539x; 1.0090x over previous
"""Trainium2 Bass kernel for nn_Net_50620484551136 (gnn_message_passing).

Network (see problem reference):
  h  = MLP(x)                     # 4652 -> 256 -> 256
  h1 = relu(GCN(h, e1)); h2 = relu(GCN(h, e2))
  h  = MLP([h1, h2])              # 512 -> 256 -> 256
  h1 = relu(GCN(h, e1)); h2 = relu(GCN(h, e2))
  h  = MLP([h1, h2])
  r1 = scatter_mean(h, index_1, N); r2 = scatter_mean(h, index_2, N)
  out = log_softmax(MLP([r1, r2]))

Strategy (8 NeuronCores, SPMD single program):
  - Tuple nodes sharded contiguously across cores (6250/core, padded to 6272).
  - All dense matmuls run feature-major (h^T: [feat, node]) in bf16, fp32 PSUM.
  - GCN: matmul commutes with aggregation, so we aggregate g = h * dinv[src]
    (node-major, bf16) and apply the conv weight after.  Each round: write
    g1|g2 locally, AllGather (4 node-range chunks, overlapped with the MLP
    that produces them) to a full [50176, 512] chunk-major buffer, then each
    core gathers its incoming-edge rows (sorted by dst) with
    gpsimd.dma_gather and segment-sums them with PE matmuls against one-hot
    SEG blocks built ON DEVICE from compact (dd, scale) pairs via
    DVE iota==dd * scale (SEG carries dinv[dst]).
  - dma_gather indices are int16, so gathers are split into a low range
    (rows < 32768) and a high range; block counts are per-tile (max over the
    8 cores) so one static program serves all cores with minimal padding.
  - Scatter-mean readout: output bins sharded across cores (625/core, padded
    to 640); same gather+SEG machinery against the AllGathered final h, with
    1/count folded into the SEG scale.  Final MLP + log_softmax on device;
    host concatenates the 8 output shards.
"""

import numpy as np
import ml_dtypes

BF16 = ml_dtypes.bfloat16
F8 = ml_dtypes.float8_e4m3

# Problem constants (hardcoded per harness contract).
T = 50000
N_BINS = 5000
F_IN = 4652
DIM = 256
N_CLASSES = 5
NCORES = 8
SPLIT = 32768  # int16 gather index limit
NCHUNK = 4     # AllGather chunks per buffer


def _ceil_to(x, m):
    return (x + m - 1) // m * m


def _wrap_idx(v):
    """int16 index vector (len % 16 == 0) -> [128, len/16] wrapped layout."""
    assert len(v) % 16 == 0
    w = v.reshape(-1, 16).T.astype(np.int16)  # [16, len/16]
    return np.tile(w, (8, 1))  # [128, len/16]


def _chunk_weight(w, dtype=BF16):
    """[K, M] -> [128, ceil(K/128), M] (partition = k%128, block = k//128)."""
    k, m = w.shape
    kp = _ceil_to(k, 128)
    wp = np.zeros((kp, m), np.float32)
    wp[:k] = w
    return np.ascontiguousarray(
        wp.reshape(kp // 128, 128, m).transpose(1, 0, 2)
    ).astype(dtype)


def _chunk_bias(b):
    """[M] -> [128, ceil(M/128)] f32 (partition = m%128, col = m//128)."""
    m = len(b)
    mp = _ceil_to(m, 128)
    bp = np.zeros(mp, np.float32)
    bp[:m] = b
    return np.ascontiguousarray(bp.reshape(mp // 128, 128).T).astype(np.float32)


def _chunk_widths(pad):
    """Split `pad` (multiple of 128) into NCHUNK widths, each mult of 128."""
    ntile = pad // 128
    per = ntile // NCHUNK
    ws = [per * 128] * (NCHUNK - 1)
    ws.append(pad - sum(ws))
    return ws


def _cm_rows(src, spc, spad, ncores):
    """Chunk-major global row id for each source node (vectorized).

    Layout: for chunk c (widths from _chunk_widths(spad)), rows
    [ncores*cum[c], ncores*cum[c+1]) hold [rank0 rows, rank1 rows, ...].
    """
    ws = _chunk_widths(spad)
    cum = np.cumsum([0] + ws)  # [NCHUNK+1]
    p = src // spc
    l = src % spc
    c = np.minimum(np.searchsorted(cum, l, side="right") - 1, NCHUNK - 1)
    return ncores * cum[c] + p * np.array(ws)[c] + (l - cum[c])


def _prep_edges(src, dst, dpc, dpad, spc, spad, ncores, seg_scale,
                seg_dtype):
    """Per-core gather indices + host-built one-hot SEG blocks for one
    (src -> dst) relation.  dst space is sharded dpc-per-core (padded dpad);
    src rows live in a chunk-major AllGathered buffer (see _cm_rows).
    Aggregation output for dst d is sum over edges e with dst==d of
    seg_scale[d] * g[src_e].

    Per-tile block counts are variable (max over cores).  Returns dict with
    per-core idx/seg arrays plus global per-tile nb_lo/nb_hi lists.
    """
    nt = dpad // 128
    order = np.argsort(dst, kind="stable")
    src = src[order]
    dst = dst[order]
    core_of = dst // dpc
    gsrc = _cm_rows(src, spc, spad, ncores)

    per_core = []  # [p][t] = (lo_gs, hi_gs, lo_dd, hi_dd)
    cnt_lo = np.zeros((ncores, nt), np.int64)
    cnt_hi = np.zeros((ncores, nt), np.int64)
    for p in range(ncores):
        sel = core_of == p
        sp = gsrc[sel]
        ld = dst[sel] - p * dpc
        tiles = []
        for t in range(nt):
            m = (ld // 128) == t
            st = sp[m]
            dd = (ld[m] - t * 128).astype(np.int64)
            lo = st < SPLIT
            tiles.append((st[lo], st[~lo] - SPLIT, dd[lo], dd[~lo]))
            cnt_lo[p, t] = lo.sum()
            cnt_hi[p, t] = (~lo).sum()
        per_core.append(tiles)

    nb_lo = [int(_ceil_to(max(cnt_lo[:, t].max(), 1), 128) // 128)
             for t in range(nt)]
    nb_hi = [int(_ceil_to(cnt_hi[:, t].max(), 128) // 128) for t in range(nt)]
    nb_tot = [nb_lo[t] + nb_hi[t] for t in range(nt)]
    off_nb = np.cumsum([0] + nb_tot).tolist()  # per-tile block offset
    tot_nb = off_nb[-1]

    idx_arrs = []
    seg_arrs = []
    for p in range(ncores):
        idx_a = np.zeros((128, tot_nb * 8), np.int16)
        seg_f = np.zeros((128, tot_nb * 128), np.float32)
        for t in range(nt):
            lo_gs, hi_gs, lo_dd, hi_dd = per_core[p][t]
            ob = off_nb[t]
            li = np.zeros(nb_lo[t] * 128, np.int64)
            li[: len(lo_gs)] = lo_gs
            idx_a[:, ob * 8: (ob + nb_lo[t]) * 8] = _wrap_idx(
                li.astype(np.int16))
            if nb_hi[t]:
                hi = np.zeros(nb_hi[t] * 128, np.int64)
                hi[: len(hi_gs)] = hi_gs
                idx_a[:, (ob + nb_lo[t]) * 8: (ob + nb_tot[t]) * 8] = \
                    _wrap_idx(hi.astype(np.int16))
            base = p * dpc + t * 128
            for boff, dd_list in ((0, lo_dd), (nb_lo[t], hi_dd)):
                i = np.arange(len(dd_list))
                b = ob + boff + i // 128
                e = i % 128
                seg_f[e, b * 128 + dd_list] = seg_scale[base + dd_list]
        idx_arrs.append(idx_a)
        seg_arrs.append(np.ascontiguousarray(seg_f.astype(seg_dtype)))
    return dict(nb_lo=nb_lo, nb_hi=nb_hi, off_nb=off_nb, tot_nb=tot_nb,
                idx=idx_arrs, seg=seg_arrs)


def host_prep(inputs, ncores=NCORES, n_bins=None):
    """Pure-numpy preprocessing: sharding, edge sorting, idx/ddsc
    construction, weight layout.  Only index arithmetic + data movement."""
    x = np.asarray(inputs["x"], np.float32)
    t_nodes, f_in = x.shape
    dim = np.asarray(inputs["W_i2"]).shape[0]
    ncls = np.asarray(inputs["b_fb"]).shape[0]
    if n_bins is None:
        if t_nodes == T and f_in == F_IN:
            n_bins = N_BINS
        else:
            n_bins = int(np.asarray(inputs["index_1"]).max()) + 1

    assert t_nodes % ncores == 0, (t_nodes, ncores)
    tpc = t_nodes // ncores
    tpad = _ceil_to(tpc, 128)
    nt = tpad // 128
    kin = _ceil_to(f_in, 128)
    assert n_bins % ncores == 0, (n_bins, ncores)
    bpc = n_bins // ncores
    bpad = _ceil_to(bpc, 128)
    bt = bpad // 128

    cfg = dict(
        t_nodes=t_nodes, f_in=f_in, dim=dim, ncls=ncls, n_bins=n_bins,
        ncores=ncores, tpc=tpc, tpad=tpad, nt=nt, kin=kin, kc=kin // 128,
        bpc=bpc, bpad=bpad, bt=bt, g_rows=ncores * tpad,
    )

    # ---- edge relations, degree norm.  Self-loops count toward the degree
    # but are excluded from the gather lists: their contribution
    # dinv[d]^2 * h[d] is added on-device from the local g shard.
    rel = {}
    for r, key in ((1, "edge_index_1"), (2, "edge_index_2")):
        ei = np.asarray(inputs[key]).astype(np.int64)
        loop = np.arange(t_nodes, dtype=np.int64)
        d_all = np.concatenate([ei[1], loop])
        deg = np.bincount(d_all, minlength=t_nodes).astype(np.float64)
        dinv = (1.0 / np.sqrt(np.maximum(deg, 1.0))).astype(np.float32)
        rel[r] = dict(
            prep=_prep_edges(ei[0], ei[1], tpc, tpad, tpc, tpad, ncores,
                             dinv, F8),
            dinv=dinv,
        )
    cfg["rel"] = rel

    # ---- readout (scatter-mean): treat (node -> bin) as edges
    ro = {}
    for i, key in ((1, "index_1"), (2, "index_2")):
        idx = np.asarray(inputs[key]).astype(np.int64)
        cnt = np.bincount(idx, minlength=n_bins).astype(np.float64)
        invc = (1.0 / np.maximum(cnt, 1.0)).astype(np.float32)
        nodes = np.arange(t_nodes, dtype=np.int64)
        ro[i] = dict(
            prep=_prep_edges(nodes, idx, bpc, bpad, tpc, tpad, ncores, invc,
                             BF16),
        )
    cfg["ro"] = ro

    # ---- per-core x^T slices (bf16) in sub-chunked layout
    # [128, nsub, kc, SUBW]: partition = k%128, sub-chunk of SUBW node
    # columns, contiguous per (partition, sub) for a single fat DMA.
    SUBW = 256
    nsub = _ceil_to(tpad, SUBW) // SUBW
    cfg["subw"] = SUBW
    cfg["nsub"] = nsub
    kc = kin // 128
    xT = []
    for p in range(ncores):
        xs = np.zeros((kin, nsub * SUBW), np.float32)
        xs[:f_in, :tpc] = x[p * tpc: (p + 1) * tpc].T
        # [kc, 128, nsub, SUBW] -> [128, nsub, kc, SUBW]
        a = xs.reshape(kc, 128, nsub, SUBW).transpose(1, 2, 0, 3)
        xT.append(np.ascontiguousarray(a).astype(F8))
    cfg["xT"] = xT

    # ---- dinv per-node tiles [128, nt] f32 per relation per core
    for r in (1, 2):
        dn = []
        dinv = rel[r]["dinv"]
        for p in range(ncores):
            a = np.zeros((128, nt), np.float32)
            vp = np.zeros(tpad, np.float32)
            vp[:tpc] = dinv[p * tpc: (p + 1) * tpc]
            a[:, :] = vp.reshape(nt, 128).T
            dn.append(a)
        rel[r]["dinv_n"] = dn

    # ---- weights
    w = {}
    w["wi1"] = _chunk_weight(np.asarray(inputs["W_i1"], np.float32), F8)
    w["wi2"] = _chunk_weight(np.asarray(inputs["W_i2"], np.float32))
    for nm, src in (("wc11", "Wc11"), ("wc12", "Wc12"),
                    ("wc21", "Wc21"), ("wc22", "Wc22"),
                    ("wm1a", "W_m1a"), ("wm1b", "W_m1b"),
                    ("wm2a", "W_m2a"), ("wm2b", "W_m2b"),
                    ("wfa", "W_fa"), ("wfb", "W_fb")):
        w[nm] = _chunk_weight(np.asarray(inputs[src], np.float32))
    for nm, src in (("bi1", "b_i1"), ("bi2", "b_i2"),
                    ("bc11", "bc11"), ("bc12", "bc12"),
                    ("bc21", "bc21"), ("bc22", "bc22"),
                    ("bm1a", "b_m1a"), ("bm1b", "b_m1b"),
                    ("bm2a", "b_m2a"), ("bm2b", "b_m2b"),
                    ("bfa", "b_fa"), ("bfb", "b_fb")):
        w[nm] = _chunk_bias(np.asarray(inputs[src], np.float32))
    w["ident16"] = np.eye(128, dtype=BF16)
    w["ident32"] = np.eye(128, dtype=np.float32)
    cfg["w"] = w
    return cfg


def _nchunks(total, step, base=0):
    out = []
    o = 0
    while o < total:
        out.append((base + o, min(step, total - o)))
        o += step
    return out


def build_program(cfg):
    """Build the SPMD bass program (one program, 8 cores)."""
    import concourse.bass as bass
    import concourse.mybir as mybir
    import concourse.tile as tile
    from concourse import bacc

    dt = mybir.dt
    AF = mybir.ActivationFunctionType
    ALU = mybir.AluOpType

    nt, tpad, kc = cfg["nt"], cfg["tpad"], cfg["kc"]
    bt, bpad = cfg["bt"], cfg["bpad"]
    dim, ncls = cfg["dim"], cfg["ncls"]
    dc = dim // 128
    g_rows = cfg["g_rows"]
    ncores = cfg["ncores"]
    rel, ro = cfg["rel"], cfg["ro"]
    rg = [list(range(ncores))]
    SUBW, nsub = cfg["subw"], cfg["nsub"]

    cw = _chunk_widths(tpad)           # node-range chunk widths
    cum = np.cumsum([0] + cw).tolist()  # local row offsets
    ctiles = [range(cum[c] // 128, cum[c + 1] // 128) for c in range(NCHUNK)]

    nc = bacc.Bacc("TRN2", target_bir_lowering=False, debug=False,
                   num_devices=ncores, num_swdge_queues=4)
    qstate = [0]

    def next_q():
        q = qstate[0]
        qstate[0] = (q + 1) % 4
        return q

    # ---------------- I/O declarations ----------------
    xT = nc.dram_tensor("xT", [128, nsub, kc, SUBW], dt.float8e4,
                        kind="ExternalInput")
    idx_in, seg_in, dinvn_in = {}, {}, {}
    for r in (1, 2):
        pr = rel[r]["prep"]
        idx_in[r] = nc.dram_tensor(f"idx{r}", [128, pr["tot_nb"] * 8],
                                   dt.int16, kind="ExternalInput")
        seg_in[r] = nc.dram_tensor(f"seg{r}", [128, pr["tot_nb"] * 128],
                                   dt.float8e4, kind="ExternalInput")
        dinvn_in[r] = nc.dram_tensor(f"dinvn{r}", [128, nt], dt.float32,
                                     kind="ExternalInput")
    idxr_in, segr_in = {}, {}
    for i in (1, 2):
        pr = ro[i]["prep"]
        idxr_in[i] = nc.dram_tensor(f"idxr{i}", [128, pr["tot_nb"] * 8],
                                    dt.int16, kind="ExternalInput")
        segr_in[i] = nc.dram_tensor(f"segr{i}", [128, pr["tot_nb"] * 128],
                                    dt.bfloat16, kind="ExternalInput")

    wnames_f8 = dict(wi1=[128, kc, dim])
    wnames_bf = dict(
        wi2=[128, dc, dim],
        wc11=[128, dc, dim], wc12=[128, dc, dim],
        wc21=[128, dc, dim], wc22=[128, dc, dim],
        wm1a=[128, 2 * dc, dim], wm1b=[128, dc, dim],
        wm2a=[128, 2 * dc, dim], wm2b=[128, dc, dim],
        wfa=[128, 2 * dc, dim], wfb=[128, dc, ncls],
        ident16=[128, 128],
    )
    wnames_f32 = dict(
        bi1=[128, dc], bi2=[128, dc],
        bc11=[128, dc], bc12=[128, dc], bc21=[128, dc], bc22=[128, dc],
        bm1a=[128, dc], bm1b=[128, dc], bm2a=[128, dc], bm2b=[128, dc],
        bfa=[128, dc], bfb=[128, 1],
        ident32=[128, 128],
    )
    win = {}
    for nm, shp in wnames_f8.items():
        win[nm] = nc.dram_tensor(nm, shp, dt.float8e4, kind="ExternalInput")
    for nm, shp in wnames_bf.items():
        win[nm] = nc.dram_tensor(nm, shp, dt.bfloat16, kind="ExternalInput")
    for nm, shp in wnames_f32.items():
        win[nm] = nc.dram_tensor(nm, shp, dt.float32, kind="ExternalInput")

    out_dram = nc.dram_tensor("out", [bpad, ncls], dt.float32,
                              kind="ExternalOutput")

    nb_max = max(rel[r]["prep"]["nb_lo"][t] + rel[r]["prep"]["nb_hi"][t]
                 for r in (1, 2) for t in range(nt))
    nb_ro_max = max(ro[i]["prep"]["nb_lo"][t] + ro[i]["prep"]["nb_hi"][t]
                    for i in (1, 2) for t in range(bt))
    nb_max = max(nb_max, nb_ro_max)

    with tile.TileContext(nc) as tc:
        with (
            tc.tile_pool(name="wpool", bufs=1) as wpool,
            tc.tile_pool(name="hpool", bufs=1) as hpool,
            tc.tile_pool(name="xpool", bufs=2) as xpool,
            tc.tile_pool(name="rpool", bufs=4) as rpool,
            tc.tile_pool(name="edpool", bufs=8) as edpool,
            tc.tile_pool(name="segpool", bufs=6) as segpool,
            tc.tile_pool(name="idxpool", bufs=8) as idxpool,
            tc.tile_pool(name="apool", bufs=4) as apool,
            tc.tile_pool(name="gpool", bufs=3) as gpool,
            tc.tile_pool(name="mpool", bufs=4) as mpool,
            tc.tile_pool(name="spool", bufs=1) as spool,
            tc.tile_pool(name="pbig", bufs=3, space="PSUM") as pbig,
            tc.tile_pool(name="pagg", bufs=2, space="PSUM") as pagg,
            tc.tile_pool(name="pcnv", bufs=3, space="PSUM") as pcnv,
            tc.tile_pool(name="dpool", bufs=1, space="DRAM") as dpool,
        ):
            # ---- resident weights
            wsb = {}
            for nm in list(wnames_f8) + list(wnames_bf) + list(wnames_f32):
                shp = (wnames_f8.get(nm) or wnames_bf.get(nm)
                       or wnames_f32[nm])
                dtyp = (dt.float8e4 if nm in wnames_f8 else
                        dt.bfloat16 if nm in wnames_bf else dt.float32)
                wt = wpool.tile(shp, dtyp, name=f"sb_{nm}", tag=f"w_{nm}")
                nc.sync.dma_start(wt[:], win[nm][:])
                wsb[nm] = wt
            dinvn_sb = {}
            for r in (1, 2):
                dv = wpool.tile([128, nt], dt.float32, name=f"sb_dinvn{r}",
                                tag=f"w_dinvn{r}")
                nc.sync.dma_start(dv[:], dinvn_in[r][:])
                dinvn_sb[r] = dv

            def a_step(h_src, t, gt_dst):
                """Transpose h tile t to node-major and scale by dinv."""
                trp = []
                for f in range(dc):
                    tp = pcnv.tile([128, 128], dt.bfloat16, name="trp",
                                   tag="cnv")
                    nc.tensor.transpose(
                        tp[:], h_src[:, f, t * 128:(t + 1) * 128],
                        wsb["ident16"][:])
                    trp.append(tp)
                for r in (1, 2):
                    for f in range(dc):
                        nc.vector.tensor_scalar_mul(
                            gt_dst[:, (r - 1) * dim + f * 128:
                                   (r - 1) * dim + (f + 1) * 128],
                            trp[f][:], dinvn_sb[r][:, t:t + 1])

            def conv_tile(pr, g_full, wc, bc, hout, r, t, gl):
                """One (relation, dst-tile) conv step: gather + SEG + W."""
                nbl, nbh = pr["nb_lo"][t], pr["nb_hi"][t]
                nb = nbl + nbh
                ob = pr["off_nb"][t]
                idxt = idxpool.tile([128, nb_max * 8], dt.int16,
                                    name="idxt", tag="idx")
                nc.sync.dma_start(idxt[:, :nb * 8],
                                  idx_in[r][:, ob * 8:(ob + nb) * 8])
                segt = segpool.tile([128, nb_max, 128], dt.float8e4,
                                    name="segt", tag="seg")
                nc.scalar.dma_start(segt[:, :nb, :],
                                    seg_in[r][:, ob * 128:(ob + nb) * 128])
                ed = edpool.tile([128, nb_max, dim], dt.float8e4,
                                 name="ed", tag="ed")
                nc.gpsimd.dma_gather(
                    ed[:, 0:nbl, :],
                    g_full[:, (r - 1) * dim:r * dim],
                    idxt[:, 0:nbl * 8],
                    nbl * 128, nbl * 128, dim,
                    elem_step=2 * dim, single_packet=False,
                    queue_num=next_q())
                if nbh:
                    nc.gpsimd.dma_gather(
                        ed[:, nbl:nb, :],
                        g_full[SPLIT:g_rows, (r - 1) * dim:r * dim],
                        idxt[:, nbl * 8:nb * 8],
                        nbh * 128, nbh * 128, dim,
                        elem_step=2 * dim, single_packet=False,
                        queue_num=next_q())
                agg = pagg.tile([128, dim], dt.float32, name="agg", tag="agg")
                npair = nb // 2
                for p_ in range(npair):
                    b = 2 * p_
                    nc.tensor.matmul(
                        agg[:],
                        lhsT=segt[:, b:b + 2, :],
                        rhs=ed[:, b:b + 2, :],
                        start=(b == 0), stop=(b + 2 == nb),
                        perf_mode=mybir.MatmulPerfMode.DoubleRow)
                if nb % 2:
                    nc.tensor.matmul(
                        agg[:],
                        lhsT=segt[:, nb - 1, :],
                        rhs=ed[:, nb - 1, :],
                        start=(nb == 1), stop=True)
                # fold in the self-loop term dinv[d]^2 h[d] = dinv[d] g[d]
                aggs = mpool.tile([128, dim], dt.bfloat16, name="aggs",
                                  tag="aggs")
                nc.vector.scalar_tensor_tensor(
                    aggs[:], gl[:, (r - 1) * dim:r * dim],
                    dinvn_sb[r][:, t:t + 1], agg[:], ALU.mult, ALU.add)
                aggT = mpool.tile([128, dim], dt.bfloat16, name="aggT",
                                  tag="aggT")
                for f in range(dc):
                    tp = pcnv.tile([128, 128], dt.bfloat16, name="tpc",
                                   tag="cnv")
                    nc.tensor.transpose(
                        tp[:], aggs[:, f * 128:(f + 1) * 128],
                        wsb["ident16"][:])
                    nc.vector.tensor_copy(
                        aggT[:, f * 128:(f + 1) * 128], tp[:])
                cps_f = [pcnv.tile([128, 128], dt.float32,
                                   name=f"cps{f}", tag="cnv")
                         for f in range(dc)]
                for f2 in range(dc):
                    for k in range(dc):
                        nc.tensor.matmul(
                            cps_f[f2][:],
                            lhsT=wc[:, k, f2 * 128:(f2 + 1) * 128],
                            rhs=aggT[:, k * 128:(k + 1) * 128],
                            start=(k == 0), stop=(k == dc - 1))
                hstage = gpool.tile([128, dc, 128], dt.bfloat16,
                                    name="hstage", tag="hstage")
                for f2 in range(dc):
                    nc.scalar.activation(hstage[:, f2, :], cps_f[f2][:],
                                         AF.Relu, bias=bc[:, f2:f2 + 1])
                nc.scalar.dma_start(hout[:, :, t * 128:(t + 1) * 128],
                                    hstage[:])

            # =========== Phase 1: input MLP  h0 = relu(x@Wi1+bi1)@Wi2+bi2
            # interleaved per AllGather chunk; AG1_c fires when chunk done.
            h_cur = hpool.tile([128, dc, tpad], dt.bfloat16, name="h0T",
                               tag="hT")
            g_loc1 = dpool.tile([tpad, 2 * dim], dt.float8e4, name="g_loc1",
                                tag="g_loc1")
            g_full1 = dpool.tile([g_rows, 2 * dim], dt.float8e4,
                                 name="g_full1", tag="g_full1")
            for c in range(NCHUNK):
                subs = [s for s in range(nsub)
                        if cum[c] <= s * SUBW < cum[c + 1]]
                for s in subs:
                    n0 = s * SUBW
                    nw = min(SUBW, tpad - n0)
                    xt = xpool.tile([128, kc, SUBW], dt.float8e4, name="xt",
                                    tag="xt")
                    nc.sync.dma_start(xt[:], xT[:, s])
                    ps1 = []
                    for f in range(dc):
                        p_ = pbig.tile([128, 512], dt.float32, name="ps1",
                                       tag="mlp")
                        ps1.append(p_)
                        for k in range(0, kc - 1, 2):
                            nc.tensor.matmul(
                                p_[:, :nw],
                                lhsT=wsb["wi1"][:, k:k + 2,
                                                f * 128:(f + 1) * 128],
                                rhs=xt[:, k:k + 2, :nw],
                                start=(k == 0), stop=(k + 2 == kc),
                                perf_mode=mybir.MatmulPerfMode.DoubleRow)
                        if kc % 2:
                            nc.tensor.matmul(
                                p_[:, :nw],
                                lhsT=wsb["wi1"][:, kc - 1,
                                                f * 128:(f + 1) * 128],
                                rhs=xt[:, kc - 1, :nw],
                                start=(kc == 1), stop=True)
                    a1 = []
                    for f in range(dc):
                        a_ = apool.tile([128, 512], dt.bfloat16, name="a1",
                                        tag="a1")
                        nc.scalar.activation(a_[:, :nw], ps1[f][:, :nw],
                                             AF.Relu, bias=wsb["bi1"][:, f:f + 1])
                        a1.append(a_)
                    for f2 in range(dc):
                        p2 = pbig.tile([128, 512], dt.float32, name="ps2",
                                       tag="mlp")
                        for k2 in range(dc):
                            nc.tensor.matmul(
                                p2[:, :nw],
                                lhsT=wsb["wi2"][:, k2, f2 * 128:(f2 + 1) * 128],
                                rhs=a1[k2][:, :nw],
                                start=(k2 == 0), stop=(k2 == dc - 1))
                        nc.vector.tensor_scalar(
                            h_cur[:, f2, n0:n0 + nw], p2[:, :nw],
                            wsb["bi2"][:, f2:f2 + 1], None, ALU.add)
                for t in ctiles[c]:
                    gt = gpool.tile([128, 2 * dim], dt.float8e4, name="gt",
                                    tag="gt")
                    a_step(h_cur, t, gt)
                    nc.sync.dma_start(g_loc1[t * 128:(t + 1) * 128, :], gt[:])
                nc.gpsimd.collective_compute(
                    "AllGather", ALU.bypass, replica_groups=rg,
                    ins=[g_loc1[cum[c]:cum[c + 1], :]],
                    outs=[g_full1[ncores * cum[c]:ncores * cum[c + 1], :]])

            # =========== Phase 2: two GCN rounds
            g_fulls = {1: g_full1}
            hf_loc = dpool.tile([tpad, dim], dt.bfloat16, name="hf_loc",
                                tag="hf_loc")
            hf_full = dpool.tile([g_rows, dim], dt.bfloat16, name="hf_full",
                                 tag="hf_full")
            for rnd in (1, 2):
                g_full = g_fulls[rnd]
                wma = wsb[f"wm{rnd}a"]
                wmb = wsb[f"wm{rnd}b"]
                bma = wsb[f"bm{rnd}a"]
                bmb = wsb[f"bm{rnd}b"]
                houts = [dpool.tile([128, dc, tpad], dt.bfloat16,
                                    name=f"h{r}T", tag=f"h12_{rnd}{r}")
                         for r in (1, 2)]
                h_next = hpool.tile([128, dc, tpad], dt.bfloat16,
                                    name=f"hm{rnd}T", tag="hT")
                if rnd == 1:
                    g_loc2 = dpool.tile([tpad, 2 * dim], dt.float8e4,
                                        name="g_loc2", tag="g_loc2")
                    g_full2 = dpool.tile([g_rows, 2 * dim], dt.float8e4,
                                         name="g_full2", tag="g_full2")
                    g_fulls[2] = g_full2
                # software pipeline: chunk c's conv tiles, then chunk c's
                # MLP + a-step + AllGather (overlaps chunk c+1's conv)
                g_loc_cur = g_loc1 if rnd == 1 else g_loc2
                for c in range(NCHUNK):
                    for t in ctiles[c]:
                        gl = rpool.tile([128, 2 * dim], dt.float8e4,
                                        name="gl", tag="gl")
                        nc.sync.dma_start(gl[:],
                                          g_loc_cur[t * 128:(t + 1) * 128, :])
                        for r in (1, 2):
                            conv_tile(rel[r]["prep"], g_full,
                                      wsb[f"wc{rnd}{r}"], wsb[f"bc{rnd}{r}"],
                                      houts[r - 1], r, t, gl)
                    for (n0, nw) in _nchunks(cw[c], 512, base=cum[c]):
                        ps1 = []
                        for f in range(dc):
                            p_ = pbig.tile([128, 512], dt.float32, name="psm1",
                                           tag="mlp")
                            ps1.append(p_)
                        for k in range(2 * dc):
                            rhs_src = houts[0] if k < dc else houts[1]
                            rhs_t = rpool.tile([128, 512], dt.bfloat16,
                                               name="ht", tag="ht")
                            nc.sync.dma_start(rhs_t[:, :nw],
                                              rhs_src[:, k % dc, n0:n0 + nw])
                            for f in range(dc):
                                nc.tensor.matmul(
                                    ps1[f][:, :nw],
                                    lhsT=wma[:, k, f * 128:(f + 1) * 128],
                                    rhs=rhs_t[:, :nw],
                                    start=(k == 0), stop=(k == 2 * dc - 1))
                        am = []
                        for f in range(dc):
                            a_ = apool.tile([128, 512], dt.bfloat16, name="am",
                                            tag="a1")
                            nc.scalar.activation(a_[:, :nw], ps1[f][:, :nw],
                                                 AF.Relu, bias=bma[:, f:f + 1])
                            am.append(a_)
                        for f2 in range(dc):
                            p2 = pbig.tile([128, 512], dt.float32, name="psm2",
                                           tag="mlp")
                            for k2 in range(dc):
                                nc.tensor.matmul(
                                    p2[:, :nw],
                                    lhsT=wmb[:, k2, f2 * 128:(f2 + 1) * 128],
                                    rhs=am[k2][:, :nw],
                                    start=(k2 == 0), stop=(k2 == dc - 1))
                            nc.vector.tensor_scalar(
                                h_next[:, f2, n0:n0 + nw], p2[:, :nw],
                                bmb[:, f2:f2 + 1], None, ALU.add)
                    if rnd == 1:
                        for t in ctiles[c]:
                            gt = gpool.tile([128, 2 * dim], dt.float8e4,
                                            name="gt", tag="gt")
                            a_step(h_next, t, gt)
                            nc.sync.dma_start(
                                g_loc2[t * 128:(t + 1) * 128, :], gt[:])
                        nc.gpsimd.collective_compute(
                            "AllGather", ALU.bypass, replica_groups=rg,
                            ins=[g_loc2[cum[c]:cum[c + 1], :]],
                            outs=[g_full2[ncores * cum[c]:
                                          ncores * cum[c + 1], :]])
                    else:
                        # final h: transpose only (no dinv scaling)
                        for t in ctiles[c]:
                            gt = gpool.tile([128, 2 * dim], dt.bfloat16,
                                            name="gtf", tag="gt")
                            for f in range(dc):
                                tp = pcnv.tile([128, 128], dt.bfloat16,
                                               name="trpf", tag="cnv")
                                nc.tensor.transpose(
                                    tp[:], h_next[:, f, t * 128:(t + 1) * 128],
                                    wsb["ident16"][:])
                                nc.vector.tensor_copy(
                                    gt[:, f * 128:(f + 1) * 128], tp[:])
                            nc.sync.dma_start(
                                hf_loc[t * 128:(t + 1) * 128, :], gt[:, :dim])
                        nc.gpsimd.collective_compute(
                            "AllGather", ALU.bypass, replica_groups=rg,
                            ins=[hf_loc[cum[c]:cum[c + 1], :]],
                            outs=[hf_full[ncores * cum[c]:
                                          ncores * cum[c + 1], :]])
                h_cur = h_next

            # =========== Phase 3: readout (bin-sharded scatter-mean)
            rcat = spool.tile([128, 2 * dc, bpad], dt.bfloat16, name="rcat",
                              tag="rcat")
            for i in (1, 2):
                pr = ro[i]["prep"]
                for t in range(bt):
                    nbl, nbh = pr["nb_lo"][t], pr["nb_hi"][t]
                    nb = nbl + nbh
                    ob = pr["off_nb"][t]
                    idxt = idxpool.tile([128, nb_max * 8], dt.int16,
                                        name="idxtr", tag="idx")
                    nc.sync.dma_start(idxt[:, :nb * 8],
                                      idxr_in[i][:, ob * 8:(ob + nb) * 8])
                    segt = segpool.tile([128, nb_ro_max * 128], dt.bfloat16,
                                        name="segtr", tag="segr")
                    nc.sync.dma_start(segt[:, :nb * 128],
                                      segr_in[i][:, ob * 128:(ob + nb) * 128])
                    ed = edpool.tile([128, nb_max, dim], dt.bfloat16,
                                     name="edr", tag="ed")
                    nc.gpsimd.dma_gather(
                        ed[:, 0:nbl, :], hf_full[:],
                        idxt[:, 0:nbl * 8],
                        nbl * 128, nbl * 128, dim,
                        single_packet=False, queue_num=next_q())
                    if nbh:
                        nc.gpsimd.dma_gather(
                            ed[:, nbl:nb, :], hf_full[SPLIT:g_rows, :],
                            idxt[:, nbl * 8:nb * 8],
                            nbh * 128, nbh * 128, dim,
                            single_packet=False, queue_num=next_q())
                    agg = pagg.tile([128, dim], dt.float32, name="aggr",
                                    tag="agg")
                    for b in range(nb):
                        nc.tensor.matmul(
                            agg[:],
                            lhsT=segt[:, b * 128:(b + 1) * 128],
                            rhs=ed[:, b, :],
                            start=(b == 0), stop=(b == nb - 1))
                    aggs = mpool.tile([128, dim], dt.bfloat16, name="aggsr",
                                      tag="aggs")
                    nc.vector.tensor_copy(aggs[:], agg[:])
                    for f in range(dc):
                        tp = pcnv.tile([128, 128], dt.bfloat16,
                                       name="tpr", tag="cnv")
                        nc.tensor.transpose(
                            tp[:], aggs[:, f * 128:(f + 1) * 128],
                            wsb["ident16"][:])
                        nc.vector.tensor_copy(
                            rcat[:, (i - 1) * dc + f, t * 128:(t + 1) * 128],
                            tp[:])

            # ---- final MLP + log_softmax
            logitsT = spool.tile([128, bpad], dt.float32, name="logitsT",
                                 tag="logitsT")
            nc.vector.memset(logitsT[:], 0.0)
            for (n0, nw) in _nchunks(bpad, 512):
                ps1 = []
                for f in range(dc):
                    p_ = pbig.tile([128, 512], dt.float32, name="psf1",
                                   tag="mlp")
                    ps1.append(p_)
                for k in range(2 * dc):
                    for f in range(dc):
                        nc.tensor.matmul(
                            ps1[f][:, :nw],
                            lhsT=wsb["wfa"][:, k, f * 128:(f + 1) * 128],
                            rhs=rcat[:, k, n0:n0 + nw],
                            start=(k == 0), stop=(k == 2 * dc - 1))
                af = []
                for f in range(dc):
                    a_ = apool.tile([128, 512], dt.bfloat16, name="af",
                                    tag="a1")
                    nc.scalar.activation(a_[:, :nw], ps1[f][:, :nw], AF.Relu,
                                         bias=wsb["bfa"][:, f:f + 1])
                    af.append(a_)
                pl = pbig.tile([128, 512], dt.float32, name="psl", tag="mlp")
                for k2 in range(dc):
                    nc.tensor.matmul(
                        pl[:ncls, :nw],
                        lhsT=wsb["wfb"][:, k2, :ncls],
                        rhs=af[k2][:, :nw],
                        start=(k2 == 0), stop=(k2 == dc - 1))
                nc.vector.tensor_scalar(
                    logitsT[:ncls, n0:n0 + nw], pl[:ncls, :nw],
                    wsb["bfb"][:ncls, 0:1], None, ALU.add)

            for t in range(bt):
                ltp = pcnv.tile([128, 128], dt.float32, name="ltp", tag="cnv")
                nc.tensor.transpose(
                    ltp[:], logitsT[:, t * 128:(t + 1) * 128],
                    wsb["ident32"][:])
                mx = mpool.tile([128, 1], dt.float32, name="mx", tag="mx")
                nc.vector.tensor_reduce(mx[:], ltp[:, :ncls],
                                        mybir.AxisListType.X, ALU.max)
                z = mpool.tile([128, ncls], dt.float32, name="z", tag="z")
                nc.vector.tensor_scalar(z[:], ltp[:, :ncls], mx[:, 0:1], None,
                                        ALU.subtract)
                ez = mpool.tile([128, ncls], dt.float32, name="ez", tag="z")
                nc.scalar.activation(ez[:], z[:], AF.Exp)
                sm = mpool.tile([128, 1], dt.float32, name="sm", tag="mx")
                nc.vector.tensor_reduce(sm[:], ez[:], mybir.AxisListType.X,
                                        ALU.add)
                ls = mpool.tile([128, 1], dt.float32, name="ls", tag="mx")
                nc.scalar.activation(ls[:], sm[:], AF.Ln)
                o = mpool.tile([128, ncls], dt.float32, name="o", tag="z")
                nc.vector.tensor_scalar(o[:], z[:], ls[:, 0:1], None,
                                        ALU.subtract)
                nc.sync.dma_start(out_dram[t * 128:(t + 1) * 128, :], o[:])

    nc.compile()
    return nc


_CACHE = {}


def build_in_maps(cfg):
    in_maps = []
    for p in range(cfg["ncores"]):
        m = dict(
            xT=cfg["xT"][p],
            idx1=cfg["rel"][1]["prep"]["idx"][p],
            seg1=cfg["rel"][1]["prep"]["seg"][p],
            idx2=cfg["rel"][2]["prep"]["idx"][p],
            seg2=cfg["rel"][2]["prep"]["seg"][p],
            dinvn1=cfg["rel"][1]["dinv_n"][p],
            dinvn2=cfg["rel"][2]["dinv_n"][p],
            idxr1=cfg["ro"][1]["prep"]["idx"][p],
            segr1=cfg["ro"][1]["prep"]["seg"][p],
            idxr2=cfg["ro"][2]["prep"]["idx"][p],
            segr2=cfg["ro"][2]["prep"]["seg"][p],
        )
        m.update({k: v for k, v in cfg["w"].items()})
        in_maps.append(m)
    return in_maps


def kernel(**inputs) -> np.ndarray:
    cfg = host_prep(inputs)
    key = (
        cfg["t_nodes"], cfg["f_in"], cfg["dim"], cfg["ncls"], cfg["n_bins"],
        tuple(tuple(cfg["rel"][r]["prep"][k]) for r in (1, 2)
              for k in ("nb_lo", "nb_hi")),
        tuple(tuple(cfg["ro"][i]["prep"][k]) for i in (1, 2)
              for k in ("nb_lo", "nb_hi")),
    )
    if key not in _CACHE:
        _CACHE[key] = build_program(cfg)
    nc = _CACHE[key]

    from concourse.bass_utils import run_bass_kernel_spmd

    in_maps = build_in_maps(cfg)
    res = run_bass_kernel_spmd(nc, in_maps, list(range(cfg["ncores"])))
    outs = [res.results[p]["out"][: cfg["bpc"]] for p in range(cfg["ncores"])]
    return np.ascontiguousarray(np.concatenate(outs, axis=0), np.float32)


# revision 42
# speedup vs baseline: 1.8943x; 1.0218x over previous
"""Trainium2 Bass kernel for nn_Net_50620484551136 (gnn_message_passing).

Network (see problem reference):
  h  = MLP(x)                     # 4652 -> 256 -> 256
  h1 = relu(GCN(h, e1)); h2 = relu(GCN(h, e2))
  h  = MLP([h1, h2])              # 512 -> 256 -> 256
  h1 = relu(GCN(h, e1)); h2 = relu(GCN(h, e2))
  h  = MLP([h1, h2])
  r1 = scatter_mean(h, index_1, N); r2 = scatter_mean(h, index_2, N)
  out = log_softmax(MLP([r1, r2]))

Strategy (8 NeuronCores, SPMD single program):
  - Tuple nodes sharded contiguously across cores (6250/core, padded to 6272).
  - All dense matmuls run feature-major (h^T: [feat, node]) in bf16, fp32 PSUM.
  - GCN: matmul commutes with aggregation, so we aggregate g = h * dinv[src]
    (node-major, bf16) and apply the conv weight after.  Each round: write
    g1|g2 locally, AllGather (4 node-range chunks, overlapped with the MLP
    that produces them) to a full [50176, 512] chunk-major buffer, then each
    core gathers its incoming-edge rows (sorted by dst) with
    gpsimd.dma_gather and segment-sums them with PE matmuls against one-hot
    SEG blocks built ON DEVICE from compact (dd, scale) pairs via
    DVE iota==dd * scale (SEG carries dinv[dst]).
  - dma_gather indices are int16, so gathers are split into a low range
    (rows < 32768) and a high range; block counts are per-tile (max over the
    8 cores) so one static program serves all cores with minimal padding.
  - Scatter-mean readout: output bins sharded across cores (625/core, padded
    to 640); same gather+SEG machinery against the AllGathered final h, with
    1/count folded into the SEG scale.  Final MLP + log_softmax on device;
    host concatenates the 8 output shards.
"""

import numpy as np
import ml_dtypes

BF16 = ml_dtypes.bfloat16
F8 = ml_dtypes.float8_e4m3

# Problem constants (hardcoded per harness contract).
T = 50000
N_BINS = 5000
F_IN = 4652
DIM = 256
N_CLASSES = 5
NCORES = 8
SPLIT = 32768  # int16 gather index limit
NCHUNK = 4     # AllGather chunks per buffer


def _ceil_to(x, m):
    return (x + m - 1) // m * m


def _wrap_idx(v):
    """int16 index vector (len % 16 == 0) -> [128, len/16] wrapped layout."""
    assert len(v) % 16 == 0
    w = v.reshape(-1, 16).T.astype(np.int16)  # [16, len/16]
    return np.tile(w, (8, 1))  # [128, len/16]


def _chunk_weight(w, dtype=BF16):
    """[K, M] -> [128, ceil(K/128), M] (partition = k%128, block = k//128)."""
    k, m = w.shape
    kp = _ceil_to(k, 128)
    wp = np.zeros((kp, m), np.float32)
    wp[:k] = w
    return np.ascontiguousarray(
        wp.reshape(kp // 128, 128, m).transpose(1, 0, 2)
    ).astype(dtype)


def _chunk_bias(b):
    """[M] -> [128, ceil(M/128)] f32 (partition = m%128, col = m//128)."""
    m = len(b)
    mp = _ceil_to(m, 128)
    bp = np.zeros(mp, np.float32)
    bp[:m] = b
    return np.ascontiguousarray(bp.reshape(mp // 128, 128).T).astype(np.float32)


def _chunk_widths(pad):
    """Split `pad` (multiple of 128) into NCHUNK widths, each mult of 128."""
    ntile = pad // 128
    per = ntile // NCHUNK
    ws = [per * 128] * (NCHUNK - 1)
    ws.append(pad - sum(ws))
    return ws


def _cm_rows(src, spc, spad, ncores):
    """Chunk-major global row id for each source node (vectorized).

    Layout: for chunk c (widths from _chunk_widths(spad)), rows
    [ncores*cum[c], ncores*cum[c+1]) hold [rank0 rows, rank1 rows, ...].
    """
    ws = _chunk_widths(spad)
    cum = np.cumsum([0] + ws)  # [NCHUNK+1]
    p = src // spc
    l = src % spc
    c = np.minimum(np.searchsorted(cum, l, side="right") - 1, NCHUNK - 1)
    return ncores * cum[c] + p * np.array(ws)[c] + (l - cum[c])


def _prep_edges(src, dst, dpc, dpad, spc, spad, ncores, seg_scale,
                seg_dtype):
    """Per-core gather indices + host-built one-hot SEG blocks for one
    (src -> dst) relation.  dst space is sharded dpc-per-core (padded dpad);
    src rows live in a chunk-major AllGathered buffer (see _cm_rows).
    Aggregation output for dst d is sum over edges e with dst==d of
    seg_scale[d] * g[src_e].

    Per-tile block counts are variable (max over cores).  Returns dict with
    per-core idx/seg arrays plus global per-tile nb_lo/nb_hi lists.
    """
    nt = dpad // 128
    order = np.argsort(dst, kind="stable")
    src = src[order]
    dst = dst[order]
    core_of = dst // dpc
    gsrc = _cm_rows(src, spc, spad, ncores)

    per_core = []  # [p][t] = (lo_gs, hi_gs, lo_dd, hi_dd)
    cnt_lo = np.zeros((ncores, nt), np.int64)
    cnt_hi = np.zeros((ncores, nt), np.int64)
    for p in range(ncores):
        sel = core_of == p
        sp = gsrc[sel]
        ld = dst[sel] - p * dpc
        tiles = []
        for t in range(nt):
            m = (ld // 128) == t
            st = sp[m]
            dd = (ld[m] - t * 128).astype(np.int64)
            lo = st < SPLIT
            tiles.append((st[lo], st[~lo] - SPLIT, dd[lo], dd[~lo]))
            cnt_lo[p, t] = lo.sum()
            cnt_hi[p, t] = (~lo).sum()
        per_core.append(tiles)

    nb_lo = [int(_ceil_to(max(cnt_lo[:, t].max(), 1), 128) // 128)
             for t in range(nt)]
    nb_hi = [int(_ceil_to(cnt_hi[:, t].max(), 128) // 128) for t in range(nt)]
    nb_tot = [nb_lo[t] + nb_hi[t] for t in range(nt)]
    off_nb = np.cumsum([0] + nb_tot).tolist()  # per-tile block offset
    tot_nb = off_nb[-1]

    idx_arrs = []
    seg_arrs = []
    for p in range(ncores):
        idx_a = np.zeros((128, tot_nb * 8), np.int16)
        seg_f = np.zeros((128, tot_nb * 128), np.float32)
        for t in range(nt):
            lo_gs, hi_gs, lo_dd, hi_dd = per_core[p][t]
            ob = off_nb[t]
            li = np.zeros(nb_lo[t] * 128, np.int64)
            li[: len(lo_gs)] = lo_gs
            idx_a[:, ob * 8: (ob + nb_lo[t]) * 8] = _wrap_idx(
                li.astype(np.int16))
            if nb_hi[t]:
                hi = np.zeros(nb_hi[t] * 128, np.int64)
                hi[: len(hi_gs)] = hi_gs
                idx_a[:, (ob + nb_lo[t]) * 8: (ob + nb_tot[t]) * 8] = \
                    _wrap_idx(hi.astype(np.int16))
            base = p * dpc + t * 128
            for boff, dd_list in ((0, lo_dd), (nb_lo[t], hi_dd)):
                i = np.arange(len(dd_list))
                b = ob + boff + i // 128
                e = i % 128
                seg_f[e, b * 128 + dd_list] = seg_scale[base + dd_list]
        idx_arrs.append(idx_a)
        seg_arrs.append(np.ascontiguousarray(seg_f.astype(seg_dtype)))
    return dict(nb_lo=nb_lo, nb_hi=nb_hi, off_nb=off_nb, tot_nb=tot_nb,
                idx=idx_arrs, seg=seg_arrs)


def host_prep(inputs, ncores=NCORES, n_bins=None):
    """Pure-numpy preprocessing: sharding, edge sorting, idx/ddsc
    construction, weight layout.  Only index arithmetic + data movement."""
    x = np.asarray(inputs["x"], np.float32)
    t_nodes, f_in = x.shape
    dim = np.asarray(inputs["W_i2"]).shape[0]
    ncls = np.asarray(inputs["b_fb"]).shape[0]
    if n_bins is None:
        if t_nodes == T and f_in == F_IN:
            n_bins = N_BINS
        else:
            n_bins = int(np.asarray(inputs["index_1"]).max()) + 1

    assert t_nodes % ncores == 0, (t_nodes, ncores)
    tpc = t_nodes // ncores
    tpad = _ceil_to(tpc, 128)
    nt = tpad // 128
    kin = _ceil_to(f_in, 128)
    assert n_bins % ncores == 0, (n_bins, ncores)
    bpc = n_bins // ncores
    bpad = _ceil_to(bpc, 128)
    bt = bpad // 128

    cfg = dict(
        t_nodes=t_nodes, f_in=f_in, dim=dim, ncls=ncls, n_bins=n_bins,
        ncores=ncores, tpc=tpc, tpad=tpad, nt=nt, kin=kin, kc=kin // 128,
        bpc=bpc, bpad=bpad, bt=bt, g_rows=ncores * tpad,
    )

    # ---- edge relations, degree norm.  Self-loops count toward the degree
    # but are excluded from the gather lists: their contribution
    # dinv[d]^2 * h[d] is added on-device from the local g shard.
    rel = {}
    for r, key in ((1, "edge_index_1"), (2, "edge_index_2")):
        ei = np.asarray(inputs[key]).astype(np.int64)
        loop = np.arange(t_nodes, dtype=np.int64)
        d_all = np.concatenate([ei[1], loop])
        deg = np.bincount(d_all, minlength=t_nodes).astype(np.float64)
        dinv = (1.0 / np.sqrt(np.maximum(deg, 1.0))).astype(np.float32)
        rel[r] = dict(
            prep=_prep_edges(ei[0], ei[1], tpc, tpad, tpc, tpad, ncores,
                             dinv, F8),
            dinv=dinv,
        )
    cfg["rel"] = rel

    # ---- readout (scatter-mean): treat (node -> bin) as edges
    ro = {}
    for i, key in ((1, "index_1"), (2, "index_2")):
        idx = np.asarray(inputs[key]).astype(np.int64)
        cnt = np.bincount(idx, minlength=n_bins).astype(np.float64)
        invc = (1.0 / np.maximum(cnt, 1.0)).astype(np.float32)
        nodes = np.arange(t_nodes, dtype=np.int64)
        ro[i] = dict(
            prep=_prep_edges(nodes, idx, bpc, bpad, tpc, tpad, ncores, invc,
                             BF16),
        )
    cfg["ro"] = ro

    # ---- per-core x^T slices (bf16) in sub-chunked layout
    # [128, nsub, kc, SUBW]: partition = k%128, sub-chunk of SUBW node
    # columns, contiguous per (partition, sub) for a single fat DMA.
    SUBW = 256
    nsub = _ceil_to(tpad, SUBW) // SUBW
    cfg["subw"] = SUBW
    cfg["nsub"] = nsub
    kc = kin // 128
    xT = []
    for p in range(ncores):
        xs = np.zeros((kin, nsub * SUBW), np.float32)
        xs[:f_in, :tpc] = x[p * tpc: (p + 1) * tpc].T
        # [kc, 128, nsub, SUBW] -> [128, nsub, kc, SUBW]
        a = xs.reshape(kc, 128, nsub, SUBW).transpose(1, 2, 0, 3)
        xT.append(np.ascontiguousarray(a).astype(F8))
    cfg["xT"] = xT

    # ---- dinv per-node tiles [128, nt] f32 per relation per core
    for r in (1, 2):
        dn = []
        dinv = rel[r]["dinv"]
        for p in range(ncores):
            a = np.zeros((128, nt), np.float32)
            vp = np.zeros(tpad, np.float32)
            vp[:tpc] = dinv[p * tpc: (p + 1) * tpc]
            a[:, :] = vp.reshape(nt, 128).T
            dn.append(a)
        rel[r]["dinv_n"] = dn

    # ---- weights
    w = {}
    w["wi1"] = _chunk_weight(np.asarray(inputs["W_i1"], np.float32), F8)
    w["wi2"] = _chunk_weight(np.asarray(inputs["W_i2"], np.float32))
    for nm, src in (("wc11", "Wc11"), ("wc12", "Wc12"),
                    ("wc21", "Wc21"), ("wc22", "Wc22"),
                    ("wm1a", "W_m1a"), ("wm1b", "W_m1b"),
                    ("wm2a", "W_m2a"), ("wm2b", "W_m2b"),
                    ("wfa", "W_fa"), ("wfb", "W_fb")):
        w[nm] = _chunk_weight(np.asarray(inputs[src], np.float32))
    for nm, src in (("bi1", "b_i1"), ("bi2", "b_i2"),
                    ("bc11", "bc11"), ("bc12", "bc12"),
                    ("bc21", "bc21"), ("bc22", "bc22"),
                    ("bm1a", "b_m1a"), ("bm1b", "b_m1b"),
                    ("bm2a", "b_m2a"), ("bm2b", "b_m2b"),
                    ("bfa", "b_fa"), ("bfb", "b_fb")):
        w[nm] = _chunk_bias(np.asarray(inputs[src], np.float32))
    w["ident16"] = np.eye(128, dtype=BF16)
    w["ident32"] = np.eye(128, dtype=np.float32)
    cfg["w"] = w
    return cfg


def _nchunks(total, step, base=0):
    out = []
    o = 0
    while o < total:
        out.append((base + o, min(step, total - o)))
        o += step
    return out


def build_program(cfg):
    """Build the SPMD bass program (one program, 8 cores)."""
    import concourse.bass as bass
    import concourse.mybir as mybir
    import concourse.tile as tile
    from concourse import bacc

    dt = mybir.dt
    AF = mybir.ActivationFunctionType
    ALU = mybir.AluOpType

    nt, tpad, kc = cfg["nt"], cfg["tpad"], cfg["kc"]
    bt, bpad = cfg["bt"], cfg["bpad"]
    dim, ncls = cfg["dim"], cfg["ncls"]
    dc = dim // 128
    g_rows = cfg["g_rows"]
    ncores = cfg["ncores"]
    rel, ro = cfg["rel"], cfg["ro"]
    rg = [list(range(ncores))]
    SUBW, nsub = cfg["subw"], cfg["nsub"]

    cw = _chunk_widths(tpad)           # node-range chunk widths
    cum = np.cumsum([0] + cw).tolist()  # local row offsets
    ctiles = [range(cum[c] // 128, cum[c + 1] // 128) for c in range(NCHUNK)]

    nc = bacc.Bacc("TRN2", target_bir_lowering=False, debug=False,
                   num_devices=ncores, num_swdge_queues=4)
    qstate = [0]

    def next_q():
        q = qstate[0]
        qstate[0] = (q + 1) % 4
        return q

    # ---------------- I/O declarations ----------------
    xT = nc.dram_tensor("xT", [128, nsub, kc, SUBW], dt.float8e4,
                        kind="ExternalInput")
    idx_in, seg_in, dinvn_in = {}, {}, {}
    for r in (1, 2):
        pr = rel[r]["prep"]
        idx_in[r] = nc.dram_tensor(f"idx{r}", [128, pr["tot_nb"] * 8],
                                   dt.int16, kind="ExternalInput")
        seg_in[r] = nc.dram_tensor(f"seg{r}", [128, pr["tot_nb"] * 128],
                                   dt.float8e4, kind="ExternalInput")
        dinvn_in[r] = nc.dram_tensor(f"dinvn{r}", [128, nt], dt.float32,
                                     kind="ExternalInput")
    idxr_in, segr_in = {}, {}
    for i in (1, 2):
        pr = ro[i]["prep"]
        idxr_in[i] = nc.dram_tensor(f"idxr{i}", [128, pr["tot_nb"] * 8],
                                    dt.int16, kind="ExternalInput")
        segr_in[i] = nc.dram_tensor(f"segr{i}", [128, pr["tot_nb"] * 128],
                                    dt.bfloat16, kind="ExternalInput")

    wnames_f8 = dict(wi1=[128, kc, dim])
    wnames_bf = dict(
        wi2=[128, dc, dim],
        wc11=[128, dc, dim], wc12=[128, dc, dim],
        wc21=[128, dc, dim], wc22=[128, dc, dim],
        wm1a=[128, 2 * dc, dim], wm1b=[128, dc, dim],
        wm2a=[128, 2 * dc, dim], wm2b=[128, dc, dim],
        wfa=[128, 2 * dc, dim], wfb=[128, dc, ncls],
        ident16=[128, 128],
    )
    wnames_f32 = dict(
        bi1=[128, dc], bi2=[128, dc],
        bc11=[128, dc], bc12=[128, dc], bc21=[128, dc], bc22=[128, dc],
        bm1a=[128, dc], bm1b=[128, dc], bm2a=[128, dc], bm2b=[128, dc],
        bfa=[128, dc], bfb=[128, 1],
        ident32=[128, 128],
    )
    win = {}
    for nm, shp in wnames_f8.items():
        win[nm] = nc.dram_tensor(nm, shp, dt.float8e4, kind="ExternalInput")
    for nm, shp in wnames_bf.items():
        win[nm] = nc.dram_tensor(nm, shp, dt.bfloat16, kind="ExternalInput")
    for nm, shp in wnames_f32.items():
        win[nm] = nc.dram_tensor(nm, shp, dt.float32, kind="ExternalInput")

    out_dram = nc.dram_tensor("out", [bpad, ncls], dt.float32,
                              kind="ExternalOutput")

    nb_max = max(rel[r]["prep"]["nb_lo"][t] + rel[r]["prep"]["nb_hi"][t]
                 for r in (1, 2) for t in range(nt))
    nb_ro_max = max(ro[i]["prep"]["nb_lo"][t] + ro[i]["prep"]["nb_hi"][t]
                    for i in (1, 2) for t in range(bt))
    nb_max = max(nb_max, nb_ro_max)

    with tile.TileContext(nc) as tc:
        with (
            tc.tile_pool(name="wpool", bufs=1) as wpool,
            tc.tile_pool(name="hpool", bufs=1) as hpool,
            tc.tile_pool(name="xpool", bufs=2) as xpool,
            tc.tile_pool(name="rpool", bufs=4) as rpool,
            tc.tile_pool(name="edpool", bufs=8) as edpool,
            tc.tile_pool(name="segpool", bufs=6) as segpool,
            tc.tile_pool(name="idxpool", bufs=8) as idxpool,
            tc.tile_pool(name="apool", bufs=4) as apool,
            tc.tile_pool(name="gpool", bufs=3) as gpool,
            tc.tile_pool(name="mpool", bufs=4) as mpool,
            tc.tile_pool(name="spool", bufs=1) as spool,
            tc.tile_pool(name="pbig", bufs=3, space="PSUM") as pbig,
            tc.tile_pool(name="pagg", bufs=2, space="PSUM") as pagg,
            tc.tile_pool(name="pcnv", bufs=3, space="PSUM") as pcnv,
            tc.tile_pool(name="dpool", bufs=1, space="DRAM") as dpool,
        ):
            # ---- resident weights
            wsb = {}
            for nm in list(wnames_f8) + list(wnames_bf) + list(wnames_f32):
                shp = (wnames_f8.get(nm) or wnames_bf.get(nm)
                       or wnames_f32[nm])
                dtyp = (dt.float8e4 if nm in wnames_f8 else
                        dt.bfloat16 if nm in wnames_bf else dt.float32)
                wt = wpool.tile(shp, dtyp, name=f"sb_{nm}", tag=f"w_{nm}")
                nc.sync.dma_start(wt[:], win[nm][:])
                wsb[nm] = wt
            dinvn_sb = {}
            for r in (1, 2):
                dv = wpool.tile([128, nt], dt.float32, name=f"sb_dinvn{r}",
                                tag=f"w_dinvn{r}")
                nc.sync.dma_start(dv[:], dinvn_in[r][:])
                dinvn_sb[r] = dv

            def a_step(h_src, t, gt_dst):
                """Transpose h tile t to node-major and scale by dinv."""
                trp = []
                for f in range(dc):
                    tp = pcnv.tile([128, 128], dt.bfloat16, name="trp",
                                   tag="cnv")
                    nc.tensor.transpose(
                        tp[:], h_src[:, f, t * 128:(t + 1) * 128],
                        wsb["ident16"][:])
                    trp.append(tp)
                for r in (1, 2):
                    for f in range(dc):
                        nc.vector.tensor_scalar_mul(
                            gt_dst[:, (r - 1) * dim + f * 128:
                                   (r - 1) * dim + (f + 1) * 128],
                            trp[f][:], dinvn_sb[r][:, t:t + 1])

            def conv_tile(pr, g_full, wc, bc, hout, r, t, gl):
                """One (relation, dst-tile) conv step: gather + SEG + W."""
                nbl, nbh = pr["nb_lo"][t], pr["nb_hi"][t]
                nb = nbl + nbh
                ob = pr["off_nb"][t]
                idxt = idxpool.tile([128, nb_max * 8], dt.int16,
                                    name="idxt", tag="idx")
                nc.sync.dma_start(idxt[:, :nb * 8],
                                  idx_in[r][:, ob * 8:(ob + nb) * 8])
                segt = segpool.tile([128, nb_max, 128], dt.float8e4,
                                    name="segt", tag="seg")
                nc.scalar.dma_start(segt[:, :nb, :],
                                    seg_in[r][:, ob * 128:(ob + nb) * 128])
                ed = edpool.tile([128, nb_max, dim], dt.float8e4,
                                 name="ed", tag="ed")
                nc.gpsimd.dma_gather(
                    ed[:, 0:nbl, :],
                    g_full[:, (r - 1) * dim:r * dim],
                    idxt[:, 0:nbl * 8],
                    nbl * 128, nbl * 128, dim,
                    elem_step=2 * dim, single_packet=False,
                    queue_num=next_q())
                if nbh:
                    nc.gpsimd.dma_gather(
                        ed[:, nbl:nb, :],
                        g_full[SPLIT:g_rows, (r - 1) * dim:r * dim],
                        idxt[:, nbl * 8:nb * 8],
                        nbh * 128, nbh * 128, dim,
                        elem_step=2 * dim, single_packet=False,
                        queue_num=next_q())
                agg = pagg.tile([128, dim], dt.float32, name="agg", tag="agg")
                npair = nb // 2
                for p_ in range(npair):
                    b = 2 * p_
                    nc.tensor.matmul(
                        agg[:],
                        lhsT=segt[:, b:b + 2, :],
                        rhs=ed[:, b:b + 2, :],
                        start=(b == 0), stop=(b + 2 == nb),
                        perf_mode=mybir.MatmulPerfMode.DoubleRow)
                if nb % 2:
                    nc.tensor.matmul(
                        agg[:],
                        lhsT=segt[:, nb - 1, :],
                        rhs=ed[:, nb - 1, :],
                        start=(nb == 1), stop=True)
                # fold in the self-loop term dinv[d]^2 h[d] = dinv[d] g[d]
                aggs = mpool.tile([128, dim], dt.bfloat16, name="aggs",
                                  tag="aggs")
                nc.vector.scalar_tensor_tensor(
                    aggs[:], gl[:, (r - 1) * dim:r * dim],
                    dinvn_sb[r][:, t:t + 1], agg[:], ALU.mult, ALU.add)
                aggT = mpool.tile([128, dim], dt.bfloat16, name="aggT",
                                  tag="aggT")
                for f in range(dc):
                    tp = pcnv.tile([128, 128], dt.bfloat16, name="tpc",
                                   tag="cnv")
                    nc.tensor.transpose(
                        tp[:], aggs[:, f * 128:(f + 1) * 128],
                        wsb["ident16"][:])
                    nc.vector.tensor_copy(
                        aggT[:, f * 128:(f + 1) * 128], tp[:])
                cps_f = [pcnv.tile([128, 128], dt.float32,
                                   name=f"cps{f}", tag="cnv")
                         for f in range(dc)]
                for f2 in range(dc):
                    for k in range(dc):
                        nc.tensor.matmul(
                            cps_f[f2][:],
                            lhsT=wc[:, k, f2 * 128:(f2 + 1) * 128],
                            rhs=aggT[:, k * 128:(k + 1) * 128],
                            start=(k == 0), stop=(k == dc - 1))
                hstage = gpool.tile([128, dc, 128], dt.bfloat16,
                                    name="hstage", tag="hstage")
                for f2 in range(dc):
                    nc.scalar.activation(hstage[:, f2, :], cps_f[f2][:],
                                         AF.Relu, bias=bc[:, f2:f2 + 1])
                nc.scalar.dma_start(hout[:, :, t * 128:(t + 1) * 128],
                                    hstage[:])

            # =========== Phase 1: input MLP  h0 = relu(x@Wi1+bi1)@Wi2+bi2
            # interleaved per AllGather chunk; AG1_c fires when chunk done.
            h_cur = hpool.tile([128, dc, tpad], dt.bfloat16, name="h0T",
                               tag="hT")
            g_loc1 = dpool.tile([tpad, 2 * dim], dt.float8e4, name="g_loc1",
                                tag="g_loc1")
            g_full1 = dpool.tile([g_rows, 2 * dim], dt.float8e4,
                                 name="g_full1", tag="g_full1")
            for c in range(NCHUNK):
                subs = [s for s in range(nsub)
                        if cum[c] <= s * SUBW < cum[c + 1]]
                for s in subs:
                    n0 = s * SUBW
                    nw = min(SUBW, tpad - n0)
                    xt = xpool.tile([128, kc, SUBW], dt.float8e4, name="xt",
                                    tag="xt")
                    nc.sync.dma_start(xt[:], xT[:, s])
                    ps1 = []
                    for f in range(dc):
                        p_ = pbig.tile([128, 512], dt.float32, name="ps1",
                                       tag="mlp")
                        ps1.append(p_)
                        for k in range(0, kc - 1, 2):
                            nc.tensor.matmul(
                                p_[:, :nw],
                                lhsT=wsb["wi1"][:, k:k + 2,
                                                f * 128:(f + 1) * 128],
                                rhs=xt[:, k:k + 2, :nw],
                                start=(k == 0), stop=(k + 2 == kc),
                                perf_mode=mybir.MatmulPerfMode.DoubleRow)
                        if kc % 2:
                            nc.tensor.matmul(
                                p_[:, :nw],
                                lhsT=wsb["wi1"][:, kc - 1,
                                                f * 128:(f + 1) * 128],
                                rhs=xt[:, kc - 1, :nw],
                                start=(kc == 1), stop=True)
                    a1 = []
                    for f in range(dc):
                        a_ = apool.tile([128, 512], dt.bfloat16, name="a1",
                                        tag="a1")
                        nc.scalar.activation(a_[:, :nw], ps1[f][:, :nw],
                                             AF.Relu, bias=wsb["bi1"][:, f:f + 1])
                        a1.append(a_)
                    for f2 in range(dc):
                        p2 = pbig.tile([128, 512], dt.float32, name="ps2",
                                       tag="mlp")
                        for k2 in range(dc):
                            nc.tensor.matmul(
                                p2[:, :nw],
                                lhsT=wsb["wi2"][:, k2, f2 * 128:(f2 + 1) * 128],
                                rhs=a1[k2][:, :nw],
                                start=(k2 == 0), stop=(k2 == dc - 1))
                        nc.vector.tensor_scalar(
                            h_cur[:, f2, n0:n0 + nw], p2[:, :nw],
                            wsb["bi2"][:, f2:f2 + 1], None, ALU.add)
                for t in ctiles[c]:
                    gt = gpool.tile([128, 2 * dim], dt.float8e4, name="gt",
                                    tag="gt")
                    a_step(h_cur, t, gt)
                    nc.sync.dma_start(g_loc1[t * 128:(t + 1) * 128, :], gt[:])
                nc.gpsimd.collective_compute(
                    "AllGather", ALU.bypass, replica_groups=rg,
                    ins=[g_loc1[cum[c]:cum[c + 1], :]],
                    outs=[g_full1[ncores * cum[c]:ncores * cum[c + 1], :]])

            # =========== Phase 2: two GCN rounds
            g_fulls = {1: g_full1}
            hf_loc = dpool.tile([tpad, dim], dt.bfloat16, name="hf_loc",
                                tag="hf_loc")
            hf_full = dpool.tile([g_rows, dim], dt.bfloat16, name="hf_full",
                                 tag="hf_full")
            for rnd in (1, 2):
                g_full = g_fulls[rnd]
                wma = wsb[f"wm{rnd}a"]
                wmb = wsb[f"wm{rnd}b"]
                bma = wsb[f"bm{rnd}a"]
                bmb = wsb[f"bm{rnd}b"]
                houts = [dpool.tile([128, dc, tpad], dt.bfloat16,
                                    name=f"h{r}T", tag=f"h12_{rnd}{r}")
                         for r in (1, 2)]
                h_next = hpool.tile([128, dc, tpad], dt.bfloat16,
                                    name=f"hm{rnd}T", tag="hT")
                if rnd == 1:
                    g_loc2 = dpool.tile([tpad, 2 * dim], dt.float8e4,
                                        name="g_loc2", tag="g_loc2")
                    g_full2 = dpool.tile([g_rows, 2 * dim], dt.float8e4,
                                         name="g_full2", tag="g_full2")
                    g_fulls[2] = g_full2
                # software pipeline: chunk c's conv tiles, then chunk c's
                # MLP + a-step + AllGather (overlaps chunk c+1's conv)
                g_loc_cur = g_loc1 if rnd == 1 else g_loc2
                for c in range(NCHUNK):
                    for t in ctiles[c]:
                        gl = rpool.tile([128, 2 * dim], dt.float8e4,
                                        name="gl", tag="gl")
                        nc.sync.dma_start(gl[:],
                                          g_loc_cur[t * 128:(t + 1) * 128, :])
                        for r in (1, 2):
                            conv_tile(rel[r]["prep"], g_full,
                                      wsb[f"wc{rnd}{r}"], wsb[f"bc{rnd}{r}"],
                                      houts[r - 1], r, t, gl)
                    for (n0, nw) in _nchunks(cw[c], 512, base=cum[c]):
                        ps1 = []
                        for f in range(dc):
                            p_ = pbig.tile([128, 512], dt.float32, name="psm1",
                                           tag="mlp")
                            ps1.append(p_)
                        for k in range(2 * dc):
                            rhs_src = houts[0] if k < dc else houts[1]
                            rhs_t = rpool.tile([128, 512], dt.bfloat16,
                                               name="ht", tag="ht")
                            nc.sync.dma_start(rhs_t[:, :nw],
                                              rhs_src[:, k % dc, n0:n0 + nw])
                            for f in range(dc):
                                nc.tensor.matmul(
                                    ps1[f][:, :nw],
                                    lhsT=wma[:, k, f * 128:(f + 1) * 128],
                                    rhs=rhs_t[:, :nw],
                                    start=(k == 0), stop=(k == 2 * dc - 1))
                        am = []
                        for f in range(dc):
                            a_ = apool.tile([128, 512], dt.bfloat16, name="am",
                                            tag="a1")
                            nc.scalar.activation(a_[:, :nw], ps1[f][:, :nw],
                                                 AF.Relu, bias=bma[:, f:f + 1])
                            am.append(a_)
                        for f2 in range(dc):
                            p2 = pbig.tile([128, 512], dt.float32, name="psm2",
                                           tag="mlp")
                            for k2 in range(dc):
                                nc.tensor.matmul(
                                    p2[:, :nw],
                                    lhsT=wmb[:, k2, f2 * 128:(f2 + 1) * 128],
                                    rhs=am[k2][:, :nw],
                                    start=(k2 == 0), stop=(k2 == dc - 1))
                            nc.vector.tensor_scalar(
                                h_next[:, f2, n0:n0 + nw], p2[:, :nw],
                                bmb[:, f2:f2 + 1], None, ALU.add)
                    if rnd == 1:
                        for t in ctiles[c]:
                            gt = gpool.tile([128, 2 * dim], dt.float8e4,
                                            name="gt", tag="gt")
                            a_step(h_next, t, gt)
                            nc.sync.dma_start(
                                g_loc2[t * 128:(t + 1) * 128, :], gt[:])
                        nc.gpsimd.collective_compute(
                            "AllGather", ALU.bypass, replica_groups=rg,
                            ins=[g_loc2[cum[c]:cum[c + 1], :]],
                            outs=[g_full2[ncores * cum[c]:
                                          ncores * cum[c + 1], :]])
                    else:
                        # final h: transpose only (no dinv scaling)
                        for t in ctiles[c]:
                            gt = gpool.tile([128, 2 * dim], dt.bfloat16,
                                            name="gtf", tag="gt")
                            for f in range(dc):
                                tp = pcnv.tile([128, 128], dt.bfloat16,
                                               name="trpf", tag="cnv")
                                nc.tensor.transpose(
                                    tp[:], h_next[:, f, t * 128:(t + 1) * 128],
                                    wsb["ident16"][:])
                                nc.vector.tensor_copy(
                                    gt[:, f * 128:(f + 1) * 128], tp[:])
                            nc.sync.dma_start(
                                hf_loc[t * 128:(t + 1) * 128, :], gt[:, :dim])
                        nc.gpsimd.collective_compute(
                            "AllGather", ALU.bypass, replica_groups=rg,
                            ins=[hf_loc[cum[c]:cum[c + 1], :]],
                            outs=[hf_full[ncores * cum[c]:
                                          ncores * cum[c + 1], :]])
                h_cur = h_next

            # =========== Phase 3: readout (bin-sharded scatter-mean)
            rcat = spool.tile([128, 2 * dc, bpad], dt.bfloat16, name="rcat",
                              tag="rcat")
            for i in (1, 2):
                pr = ro[i]["prep"]
                for t in range(bt):
                    nbl, nbh = pr["nb_lo"][t], pr["nb_hi"][t]
                    nb = nbl + nbh
                    ob = pr["off_nb"][t]
                    idxt = idxpool.tile([128, nb_max * 8], dt.int16,
                                        name="idxtr", tag="idx")
                    nc.sync.dma_start(idxt[:, :nb * 8],
                                      idxr_in[i][:, ob * 8:(ob + nb) * 8])
                    segt = segpool.tile([128, nb_ro_max * 128], dt.bfloat16,
                                        name="segtr", tag="segr")
                    nc.sync.dma_start(segt[:, :nb * 128],
                                      segr_in[i][:, ob * 128:(ob + nb) * 128])
                    ed = edpool.tile([128, nb_max, dim], dt.bfloat16,
                                     name="edr", tag="ed")
                    nc.gpsimd.dma_gather(
                        ed[:, 0:nbl, :], hf_full[:],
                        idxt[:, 0:nbl * 8],
                        nbl * 128, nbl * 128, dim,
                        single_packet=False, queue_num=next_q())
                    if nbh:
                        nc.gpsimd.dma_gather(
                            ed[:, nbl:nb, :], hf_full[SPLIT:g_rows, :],
                            idxt[:, nbl * 8:nb * 8],
                            nbh * 128, nbh * 128, dim,
                            single_packet=False, queue_num=next_q())
                    agg = pagg.tile([128, dim], dt.float32, name="aggr",
                                    tag="agg")
                    for b in range(nb):
                        nc.tensor.matmul(
                            agg[:],
                            lhsT=segt[:, b * 128:(b + 1) * 128],
                            rhs=ed[:, b, :],
                            start=(b == 0), stop=(b == nb - 1))
                    aggs = mpool.tile([128, dim], dt.bfloat16, name="aggsr",
                                      tag="aggs")
                    nc.vector.tensor_copy(aggs[:], agg[:])
                    for f in range(dc):
                        tp = pcnv.tile([128, 128], dt.bfloat16,
                                       name="tpr", tag="cnv")
                        nc.tensor.transpose(
                            tp[:], aggs[:, f * 128:(f + 1) * 128],
                            wsb["ident16"][:])
                        nc.vector.tensor_copy(
                            rcat[:, (i - 1) * dc + f, t * 128:(t + 1) * 128],
                            tp[:])

            # ---- final MLP + log_softmax
            logitsT = spool.tile([128, bpad], dt.float32, name="logitsT",
                                 tag="logitsT")
            nc.vector.memset(logitsT[:], 0.0)
            for (n0, nw) in _nchunks(bpad, 512):
                ps1 = []
                for f in range(dc):
                    p_ = pbig.tile([128, 512], dt.float32, name="psf1",
                                   tag="mlp")
                    ps1.append(p_)
                for k in range(2 * dc):
                    for f in range(dc):
                        nc.tensor.matmul(
                            ps1[f][:, :nw],
                            lhsT=wsb["wfa"][:, k, f * 128:(f + 1) * 128],
                            rhs=rcat[:, k, n0:n0 + nw],
                            start=(k == 0), stop=(k == 2 * dc - 1))
                af = []
                for f in range(dc):
                    a_ = apool.tile([128, 512], dt.bfloat16, name="af",
                                    tag="a1")
                    nc.scalar.activation(a_[:, :nw], ps1[f][:, :nw], AF.Relu,
                                         bias=wsb["bfa"][:, f:f + 1])
                    af.append(a_)
                pl = pbig.tile([128, 512], dt.float32, name="psl", tag="mlp")
                for k2 in range(dc):
                    nc.tensor.matmul(
                        pl[:ncls, :nw],
                        lhsT=wsb["wfb"][:, k2, :ncls],
                        rhs=af[k2][:, :nw],
                        start=(k2 == 0), stop=(k2 == dc - 1))
                nc.vector.tensor_scalar(
                    logitsT[:ncls, n0:n0 + nw], pl[:ncls, :nw],
                    wsb["bfb"][:ncls, 0:1], None, ALU.add)

            for t in range(bt):
                ltp = pcnv.tile([128, 128], dt.float32, name="ltp", tag="cnv")
                nc.tensor.transpose(
                    ltp[:], logitsT[:, t * 128:(t + 1) * 128],
                    wsb["ident32"][:])
                mx = mpool.tile([128, 1], dt.float32, name="mx", tag="mx")
                nc.vector.tensor_reduce(mx[:], ltp[:, :ncls],
                                        mybir.AxisListType.X, ALU.max)
                z = mpool.tile([128, ncls], dt.float32, name="z", tag="z")
                nc.vector.tensor_scalar(z[:], ltp[:, :ncls], mx[:, 0:1], None,
                                        ALU.subtract)
                ez = mpool.tile([128, ncls], dt.float32, name="ez", tag="z")
                nc.scalar.activation(ez[:], z[:], AF.Exp)
                sm = mpool.tile([128, 1], dt.float32, name="sm", tag="mx")
                nc.vector.tensor_reduce(sm[:], ez[:], mybir.AxisListType.X,
                                        ALU.add)
                ls = mpool.tile([128, 1], dt.float32, name="ls", tag="mx")
                nc.scalar.activation(ls[:], sm[:], AF.Ln)
                o = mpool.tile([128, ncls], dt.float32, name="o", tag="z")
                nc.vector.tensor_scalar(o[:], z[:], ls[:, 0:1], None,
                                        ALU.subtract)
                nc.sync.dma_start(out_dram[t * 128:(t + 1) * 128, :], o[:])

    nc.compile()
    return nc


_CACHE = {}


def build_in_maps(cfg):
    in_maps = []
    for p in range(cfg["ncores"]):
        m = dict(
            xT=cfg["xT"][p],
            idx1=cfg["rel"][1]["prep"]["idx"][p],
            seg1=cfg["rel"][1]["prep"]["seg"][p],
            idx2=cfg["rel"][2]["prep"]["idx"][p],
            seg2=cfg["rel"][2]["prep"]["seg"][p],
            dinvn1=cfg["rel"][1]["dinv_n"][p],
            dinvn2=cfg["rel"][2]["dinv_n"][p],
            idxr1=cfg["ro"][1]["prep"]["idx"][p],
            segr1=cfg["ro"][1]["prep"]["seg"][p],
            idxr2=cfg["ro"][2]["prep"]["idx"][p],
            segr2=cfg["ro"][2]["prep"]["seg"][p],
        )
        m.update({k: v for k, v in cfg["w"].items()})
        in_maps.append(m)
    return in_maps


def kernel(**inputs) -> np.ndarray:
    cfg = host_prep(inputs)
    key = (
        cfg["t_nodes"], cfg["f_in"], cfg["dim"], cfg["ncls"], cfg["n_bins"],
        tuple(tuple(cfg["rel"][r]["prep"][k]) for r in (1, 2)
              for k in ("nb_lo", "nb_hi")),
        tuple(tuple(cfg["ro"][i]["prep"][k]) for i in (1, 2)
              for k in ("nb_lo", "nb_hi")),
    )
    if key not in _CACHE:
        _CACHE[key] = build_program(cfg)
    nc = _CACHE[key]

    from concourse.bass_utils import run_bass_kernel_spmd

    in_maps = build_in_maps(cfg)
    res = run_bass_kernel_spmd(nc, in_maps, list(range(cfg["ncores"])))
    outs = [res.results[p]["out"][: cfg["bpc"]] for p in range(cfg["ncores"])]
    return np.ascontiguousarray(np.concatenate(outs, axis=0), np.float32)
